# revision 3
# baseline (speedup 1.0000x reference)
"""Trainium2 Bass kernel for nn_CrossAttention2d (B=32, C=256, INNER=128, H=W=32).

Sharding: pure data parallel — batch 32 split as 4 items per core across 8
NeuronCores; all weights replicated. No collectives.

Per item (N = H*W = 1024 tokens, C = 256 channels, D = 128 inner):
  attention for output stream s (s=0 -> fs side, s=1 -> fi side):
      q = wq[1-s] @ f[1-s]   (D x N)
      k = wk[s]   @ f[s]     (D x N)
      vT[m, c] = (wv[s] @ f[s]).T  -- computed directly as f.T @ wv.T slices
      S^T[m, n] = sum_d k[d, m] q[d, n]        (PE, m-tiles of 128)
      E = exp(S^T / sqrt(D))                   (ACT, psum -> sbuf bf16)
      O_un[c, n] = sum_m vT[m, c] E[m, n]      (PE, accumulate 8 chunks)
      den[n] = sum_m E[m, n]  (DVE tree add + PE ones-colsum broadcast)
      attn = O_un * (1/den)                    (DVE)
  fuse: g = relu(Wf @ [f[s]; attn] + b)        (PE + ACT)
  h = g + f32[s]; LayerNorm over all (C,N) of h (stats via DVE/ACT accum +
  PE partition-reduce), out = h * A + B        (DVE tensor_scalar)

Matmul convention: out[M, N] = lhsT.T @ rhs, lhsT = [K<=128, M<=128] (K on
partitions), rhs = [K, N<=512], out in PSUM f32.
"""

import numpy as np
import ml_dtypes

import concourse.bacc as bacc
import concourse.bass as bass
import concourse.tile as tile
from concourse import mybir
from concourse.bass_utils import run_bass_kernel_spmd

F32 = mybir.dt.float32
BF16 = mybir.dt.bfloat16
AF = mybir.ActivationFunctionType
OP = mybir.AluOpType

B, C, D, N = 32, 256, 128, 1024
NCORES = 8
IPC = B // NCORES  # items per core = 4
SCALE = 1.0 / float(np.sqrt(D))  # 0.08838834764831845
EPS = 1e-5
NTOT = float(C * N)  # layernorm element count per item/stream

# test.py can set {"trace": True}; harness path leaves this empty.
RUN_KWARGS = {}
LAST_RESULT = None


def _build():
    nc = bacc.Bacc("TRN2", target_bir_lowering=False, debug=False,
                   num_devices=NCORES)

    # ---- DRAM I/O (per-core shapes) ----
    fb_d = [nc.dram_tensor(n_, [IPC, 2, 128, N], BF16, kind="ExternalInput")
            for n_ in ("fsb", "fib")]
    f32_d = [nc.dram_tensor(n_, [IPC, 2, 128, N], F32, kind="ExternalInput")
             for n_ in ("fs32", "fi32")]
    wq_d = [nc.dram_tensor(n_, [128, 2, 128], BF16, kind="ExternalInput")
            for n_ in ("wq0", "wq1")]
    wk_d = [nc.dram_tensor(n_, [128, 2, 128], BF16, kind="ExternalInput")
            for n_ in ("wk0", "wk1")]
    wv_d = [nc.dram_tensor(n_, [128, 2, 256], BF16, kind="ExternalInput")
            for n_ in ("wv0", "wv1")]
    wf_d = nc.dram_tensor("wfuse", [128, 4, 256], BF16, kind="ExternalInput")
    fb_bias_d = nc.dram_tensor("fuseb", [128, 2], F32, kind="ExternalInput")
    lnw_d = nc.dram_tensor("lnw", [128, 2, 2], F32, kind="ExternalInput")
    lnb_d = nc.dram_tensor("lnb", [128, 2, 2], F32, kind="ExternalInput")
    out_d = [nc.dram_tensor(n_, [IPC, 2, 128, N], F32, kind="ExternalOutput")
             for n_ in ("out0", "out1")]

    with tile.TileContext(nc) as tc:
        consts = tc.alloc_tile_pool(name="consts", bufs=1)
        inp = tc.alloc_tile_pool(name="inp", bufs=2)
        work = tc.alloc_tile_pool(name="work", bufs=2)
        psum = tc.alloc_tile_pool(name="psum", bufs=2, space="PSUM")

        # ---- load constants ----
        wq = [consts.tile([128, 2, 128], BF16, name=f"wq{s}", tag=f"wq{s}")
              for s in range(2)]
        wk = [consts.tile([128, 2, 128], BF16, name=f"wk{s}", tag=f"wk{s}")
              for s in range(2)]
        wv = [consts.tile([128, 2, 256], BF16, name=f"wv{s}", tag=f"wv{s}")
              for s in range(2)]
        wf = consts.tile([128, 4, 256], BF16, name="wf", tag="wf")
        fbias = consts.tile([128, 2], F32, name="fbias", tag="fbias")
        lnw = consts.tile([128, 2, 2], F32, name="lnw", tag="lnw")
        lnb = consts.tile([128, 2, 2], F32, name="lnb", tag="lnb")
        ones_bf = consts.tile([128, 128], BF16, name="ones_bf", tag="ones_bf")
        ones_col = consts.tile([128, 1], F32, name="ones_col", tag="ones_col")
        ones_row = consts.tile([1, 128], F32, name="ones_row", tag="ones_row")
        for s in range(2):
            nc.sync.dma_start(out=wq[s][:], in_=wq_d[s][:])
            nc.sync.dma_start(out=wk[s][:], in_=wk_d[s][:])
            nc.sync.dma_start(out=wv[s][:], in_=wv_d[s][:])
        nc.sync.dma_start(out=wf[:], in_=wf_d[:])
        nc.sync.dma_start(out=fbias[:], in_=fb_bias_d[:])
        nc.sync.dma_start(out=lnw[:], in_=lnw_d[:])
        nc.sync.dma_start(out=lnb[:], in_=lnb_d[:])
        nc.vector.memset(ones_bf[:], 1.0)
        nc.vector.memset(ones_col[:], 1.0)
        nc.vector.memset(ones_row[:], 1.0)

        def conv_qk(w_t, f_t, name):
            """[128, N] = w.T @ f  (K=256 via 2 chunks); returns bf16 sbuf."""
            ps = psum.tile([128, N], F32, name=f"ps_{name}", tag="work")
            for h in range(2):
                for kc in range(2):
                    nc.tensor.matmul(
                        ps[:, h * 512:(h + 1) * 512],
                        lhsT=w_t[:, kc, :],
                        rhs=f_t[:, kc, h * 512:(h + 1) * 512],
                        start=(kc == 0), stop=(kc == 1))
            sb = work.tile([128, N], BF16, name=name, tag=name)
            nc.vector.tensor_copy(out=sb[:], in_=ps[:])
            return sb

        for i in range(IPC):
            # ---- input DMAs ----
            fb = []
            f32t = []
            for s in range(2):
                t = inp.tile([128, 2, N], BF16, name=f"fb{s}", tag=f"fb{s}")
                nc.sync.dma_start(
                    out=t[:], in_=fb_d[s][i].rearrange("c p n -> p c n"))
                fb.append(t)
                t2 = inp.tile([128, 2, N], F32, name=f"f32_{s}", tag=f"f32_{s}")
                nc.sync.dma_start(
                    out=t2[:], in_=f32_d[s][i].rearrange("c p n -> p c n"))
                f32t.append(t2)

            stats = work.tile([128, 8], F32, name="stats", tag="stats")
            h_sb = []

            for s in range(2):
                # ================= attention for output stream s ==========
                q_sb = conv_qk(wq[1 - s], fb[1 - s], "q_sb")
                k_sb = conv_qk(wk[s], fb[s], "k_sb")

                # vT[m, c] computed directly: lhsT = f chunk slice, rhs = wv
                vt_sb = work.tile([128, 8, 256], BF16, name="vt_sb", tag="vt")
                for half in range(2):
                    ps_vt = psum.tile([128, N], F32, name="ps_vt", tag="work")
                    for jj in range(4):
                        j = half * 4 + jj
                        for kc in range(2):
                            nc.tensor.matmul(
                                ps_vt[:, jj * 256:(jj + 1) * 256],
                                lhsT=fb[s][:, kc, j * 128:(j + 1) * 128],
                                rhs=wv[s][:, kc, :],
                                start=(kc == 0), stop=(kc == 1))
                    nc.vector.tensor_copy(
                        out=vt_sb[:, half * 4:(half + 1) * 4, :]
                        .rearrange("p a b -> p (a b)"),
                        in_=ps_vt[:])

                # S^T -> exp -> PV accumulate, chunk by chunk
                pv_ps = [psum.tile([128, N], F32, name=f"pv{t}", tag="pv")
                         for t in range(2)]
                expS = work.tile([128, 8, N], BF16, name="expS", tag="expS")
                for j in range(8):
                    ps_s = psum.tile([128, N], F32, name="ps_s", tag="work")
                    for h in range(2):
                        nc.tensor.matmul(
                            ps_s[:, h * 512:(h + 1) * 512],
                            lhsT=k_sb[:, j * 128:(j + 1) * 128],
                            rhs=q_sb[:, h * 512:(h + 1) * 512],
                            start=True, stop=True)
                    nc.scalar.activation(
                        out=expS[:, j, :], in_=ps_s[:], func=AF.Exp,
                        scale=SCALE)
                    for t in range(2):
                        for h in range(2):
                            nc.tensor.matmul(
                                pv_ps[t][:, h * 512:(h + 1) * 512],
                                lhsT=vt_sb[:, j, t * 128:(t + 1) * 128],
                                rhs=expS[:, j, h * 512:(h + 1) * 512],
                                start=(j == 0), stop=(j == 7))

                # denominator: tree-add the 8 chunks, then ones-matmul
                # broadcasts the partition-sum to all 128 rows.
                dpar = [work.tile([128, N], BF16, name=f"dpar{a}",
                                  tag=f"dpar{a}") for a in range(4)]
                for a in range(4):
                    nc.vector.tensor_add(out=dpar[a][:], in0=expS[:, 2 * a, :],
                                         in1=expS[:, 2 * a + 1, :])
                nc.vector.tensor_add(out=dpar[0][:], in0=dpar[0][:],
                                     in1=dpar[1][:])
                nc.vector.tensor_add(out=dpar[2][:], in0=dpar[2][:],
                                     in1=dpar[3][:])
                nc.vector.tensor_add(out=dpar[0][:], in0=dpar[0][:],
                                     in1=dpar[2][:])
                ps_bc = psum.tile([128, N], F32, name="ps_bc", tag="work")
                for h in range(2):
                    nc.tensor.matmul(
                        ps_bc[:, h * 512:(h + 1) * 512],
                        lhsT=ones_bf[:],
                        rhs=dpar[0][:, h * 512:(h + 1) * 512],
                        start=True, stop=True)
                rden = work.tile([128, N], F32, name="rden", tag="rden")
                nc.vector.reciprocal_approx_fast(out=rden[:], in_=ps_bc[:])

                # normalize PV -> attn (bf16, feeds fuse matmul)
                attn_sb = work.tile([128, 2, N], BF16, name="attn_sb",
                                    tag="attn")
                for t in range(2):
                    nc.vector.tensor_tensor(
                        out=attn_sb[:, t, :], in0=pv_ps[t][:], in1=rden[:],
                        op=OP.mult)

                # ================= fuse + residual + LN stats =============
                h_t = work.tile([128, 2, N], F32, name="h_t", tag=f"h{s}",
                                bufs=2)
                for t in range(2):
                    ps_f = psum.tile([128, N], F32, name="ps_f", tag="work")
                    for h in range(2):
                        for kc in range(4):
                            rhs = (fb[s][:, kc, h * 512:(h + 1) * 512]
                                   if kc < 2 else
                                   attn_sb[:, kc - 2, h * 512:(h + 1) * 512])
                            nc.tensor.matmul(
                                ps_f[:, h * 512:(h + 1) * 512],
                                lhsT=wf[:, kc, t * 128:(t + 1) * 128],
                                rhs=rhs, start=(kc == 0), stop=(kc == 3))
                    g_t = work.tile([128, N], F32, name="g_t", tag="g_t")
                    nc.scalar.activation(
                        out=g_t[:], in_=ps_f[:], func=AF.Relu,
                        bias=fbias[:, t:t + 1], scale=1.0)
                    c0 = s * 4 + t * 2
                    nc.vector.scalar_tensor_tensor(
                        out=h_t[:, t, :], in0=g_t[:], scalar=1.0,
                        in1=f32t[s][:, t, :], op0=OP.mult, op1=OP.add,
                        accum_out=stats[:, c0:c0 + 1])
                    dum = work.tile([128, N], BF16, name="dum", tag="dum")
                    nc.scalar.activation(
                        out=dum[:], in_=h_t[:, t, :], func=AF.Square,
                        accum_out=stats[:, c0 + 1:c0 + 2])
                h_sb.append(h_t)

            # ================= LN finalize (both streams) =================
            ps_st = psum.tile([1, 8], F32, name="ps_st", tag="work")
            nc.tensor.matmul(ps_st[:], lhsT=ones_col[:], rhs=stats[:],
                             start=True, stop=True)
            st = work.tile([1, 8], F32, name="st", tag="st")
            nc.vector.tensor_copy(out=st[:], in_=ps_st[:])
            # cols: s*4 + t*2 + k (k=0 sum, k=1 sumsq) -> tot over t
            st_r = st[:].rearrange("p (a b) -> p a b", a=2)  # a=s, b=(t,k)
            tot = work.tile([1, 4], F32, name="tot", tag="tot")
            nc.vector.tensor_add(
                out=tot[:].rearrange("p (a b) -> p a b", a=2),
                in0=st_r[:, :, 0:2],
                in1=st_r[:, :, 2:4])
            # tot = [s0_sum, s0_sq, s1_sum, s1_sq] -> moments = tot / NTOT
            mom = work.tile([1, 4], F32, name="mom", tag="mom")
            nc.vector.tensor_scalar(out=mom[:], in0=tot[:],
                                    scalar1=1.0 / NTOT, scalar2=None,
                                    op0=OP.mult)
            # var = E[x^2] - mu^2 ; mr = [rstd0, rstd1, -mu0, -mu1]
            mom_r = mom[:].rearrange("p (a b) -> p a b", a=2)
            mu_ap = mom_r[:, :, 0]
            m2_ap = mom_r[:, :, 1]
            musq = work.tile([1, 2], F32, name="musq", tag="musq")
            nc.vector.tensor_tensor(out=musq[:], in0=mu_ap, in1=mu_ap,
                                    op=OP.mult)
            var = work.tile([1, 2], F32, name="var", tag="var")
            nc.vector.tensor_tensor(out=var[:], in0=m2_ap, in1=musq[:],
                                    op=OP.subtract)
            nc.vector.tensor_scalar(out=var[:], in0=var[:], scalar1=EPS,
                                    scalar2=None, op0=OP.add)
            sd = work.tile([1, 2], F32, name="sd", tag="sd")
            nc.scalar.activation(out=sd[:], in_=var[:], func=AF.Sqrt)
            mr = work.tile([1, 4], F32, name="mr", tag="mr")
            nc.vector.reciprocal(out=mr[:, 0:2], in_=sd[:])
            nc.vector.tensor_scalar(out=mr[:, 2:4], in0=mu_ap,
                                    scalar1=-1.0, scalar2=None, op0=OP.mult)
            # broadcast [1,4] -> [128,4] via K=1 f32 matmul
            ps_mr = psum.tile([128, 4], F32, name="ps_mr", tag="work")
            nc.tensor.matmul(ps_mr[:], lhsT=ones_row[:], rhs=mr[:],
                             start=True, stop=True)
            mrb = work.tile([128, 4], F32, name="mrb", tag="mrb")
            nc.vector.tensor_copy(out=mrb[:], in_=ps_mr[:])
            # A = lnw * rstd ; Bb = lnb + (-mu) * A ; out = h * A + Bb
            for s in range(2):
                Asb = work.tile([128, 2], F32, name="Asb", tag=f"A{s}")
                nc.vector.tensor_scalar(
                    out=Asb[:], in0=lnw[:, s, :], scalar1=mrb[:, s:s + 1],
                    scalar2=None, op0=OP.mult)
                Bsb = work.tile([128, 2], F32, name="Bsb", tag=f"B{s}")
                nc.vector.scalar_tensor_tensor(
                    out=Bsb[:], in0=Asb[:], scalar=mrb[:, 2 + s:3 + s],
                    in1=lnb[:, s, :], op0=OP.mult, op1=OP.add)
                for t in range(2):
                    o_t = work.tile([128, N], F32, name="o_t", tag="o_t",
                                    bufs=4)
                    nc.vector.tensor_scalar(
                        out=o_t[:], in0=h_sb[s][:, t, :],
                        scalar1=Asb[:, t:t + 1], scalar2=Bsb[:, t:t + 1],
                        op0=OP.mult, op1=OP.add)
                    nc.sync.dma_start(out=out_d[s][i, t], in_=o_t[:])

        psum.release()
        work.release()
        inp.release()
        consts.release()

    nc.compile()
    return nc


_NC_CACHE = None


def _get_nc():
    global _NC_CACHE
    if _NC_CACHE is None:
        _NC_CACHE = _build()
    return _NC_CACHE


def kernel(fs, fi, qs_w, ks_w, vs_w, qi_w, ki_w, vi_w,
           fuse_w, fuse_b, ln_s_w, ln_s_b, ln_i_w, ln_i_b):
    global LAST_RESULT
    fs = np.asarray(fs, np.float32)
    fi = np.asarray(fi, np.float32)

    def prep_f(x):
        # (B, C, H, W) -> per-core [IPC, 2, 128, N]
        x = x.reshape(NCORES, IPC, 2, 128, N)
        return x

    def prep_w_qk(w):  # (128, 256) -> lhsT layout [128p, 2kc, 128m]
        wt = np.ascontiguousarray(w.T)  # (256, 128)
        return np.ascontiguousarray(
            wt.reshape(2, 128, 128).transpose(1, 0, 2)).astype(
                ml_dtypes.bfloat16)

    def prep_w_v(w):  # (256, 256) -> rhs layout [128p, 2kc, 256c]
        wt = np.ascontiguousarray(np.asarray(w, np.float32).T)
        return np.ascontiguousarray(
            wt.reshape(2, 128, 256).transpose(1, 0, 2)).astype(
                ml_dtypes.bfloat16)

    fs_sh = prep_f(fs)
    fi_sh = prep_f(fi)
    fs_bf = fs_sh.astype(ml_dtypes.bfloat16)
    fi_bf = fi_sh.astype(ml_dtypes.bfloat16)

    wq0 = prep_w_qk(np.asarray(qs_w, np.float32))
    wq1 = prep_w_qk(np.asarray(qi_w, np.float32))
    wk0 = prep_w_qk(np.asarray(ks_w, np.float32))
    wk1 = prep_w_qk(np.asarray(ki_w, np.float32))
    wv0 = prep_w_v(vs_w)
    wv1 = prep_w_v(vi_w)
    wfuse = np.ascontiguousarray(
        np.asarray(fuse_w, np.float32).T.reshape(4, 128, 256)
        .transpose(1, 0, 2)).astype(ml_dtypes.bfloat16)
    fuseb = np.ascontiguousarray(
        np.asarray(fuse_b, np.float32).reshape(2, 128).T)
    lnw = np.ascontiguousarray(
        np.stack([np.asarray(ln_s_w, np.float32).reshape(256),
                  np.asarray(ln_i_w, np.float32).reshape(256)])
        .reshape(2, 2, 128).transpose(2, 0, 1))
    lnb = np.ascontiguousarray(
        np.stack([np.asarray(ln_s_b, np.float32).reshape(256),
                  np.asarray(ln_i_b, np.float32).reshape(256)])
        .reshape(2, 2, 128).transpose(2, 0, 1))

    in_maps = []
    for c in range(NCORES):
        in_maps.append({
            "fsb": np.ascontiguousarray(fs_bf[c]),
            "fib": np.ascontiguousarray(fi_bf[c]),
            "fs32": np.ascontiguousarray(fs_sh[c]),
            "fi32": np.ascontiguousarray(fi_sh[c]),
            "wq0": wq0, "wq1": wq1, "wk0": wk0, "wk1": wk1,
            "wv0": wv0, "wv1": wv1, "wfuse": wfuse, "fuseb": fuseb,
            "lnw": lnw, "lnb": lnb,
        })

    nc = _get_nc()
    res = run_bass_kernel_spmd(nc, in_maps, core_ids=list(range(NCORES)),
                               **RUN_KWARGS)
    LAST_RESULT = res

    fs_out = np.empty((NCORES, IPC, 2, 128, N), np.float32)
    fi_out = np.empty((NCORES, IPC, 2, 128, N), np.float32)
    for c in range(NCORES):
        fs_out[c] = res.results[c]["out0"]
        fi_out[c] = res.results[c]["out1"]
    fs_out = fs_out.reshape(B, C, 32, 32)
    fi_out = fi_out.reshape(B, C, 32, 32)
    return fs_out, fi_out


# revision 4
# speedup vs baseline: 1.1212x; 1.1212x over previous
"""Trainium2 Bass kernel for nn_CrossAttention2d (B=32, C=256, INNER=128, H=W=32).

Sharding: pure data parallel — batch 32 split as 4 items per core across 8
NeuronCores; all weights replicated. No collectives.

Per item (N = H*W = 1024 tokens, C = 256 channels, D = 128 inner):
  attention for output stream s (s=0 -> fs side, s=1 -> fi side):
      q = wq[1-s] @ f[1-s]   (D x N)
      k = wk[s]   @ f[s]     (D x N)
      vT[m, c] = (wv[s] @ f[s]).T  -- computed directly as f.T @ wv.T slices
      S^T[m, n] = sum_d k[d, m] q[d, n]        (PE, m-tiles of 128)
      E = exp(S^T / sqrt(D))                   (ACT, psum -> sbuf bf16)
      O_un[c, n] = sum_m vT[m, c] E[m, n]      (PE, accumulate 8 chunks)
      den[n] = sum_m E[m, n]  (DVE tree add + PE ones-colsum broadcast)
      attn = O_un * (1/den)                    (DVE)
  fuse: g = relu(Wf @ [f[s]; attn] + b)        (PE + ACT, bf16 out)
  h = g + f[s] (bf16 residual); LayerNorm over all (C,N) of h (stats via
  DVE/ACT accum + PE partition-reduce), out = h * A + B (DVE tensor_scalar)

Matmul convention: out[M, N] = lhsT.T @ rhs, lhsT = [K<=128, M<=128] (K on
partitions), rhs = [K, N<=512], out in PSUM f32 (one bank per matmul).
PSUM: "pv" tag 2x[128,1024] (4 banks) + "work" tag 4x[128,512] (4 banks).
"""

import numpy as np
import ml_dtypes

import concourse.bacc as bacc
import concourse.bass as bass
import concourse.tile as tile
from concourse import mybir
from concourse.bass_utils import run_bass_kernel_spmd

F32 = mybir.dt.float32
BF16 = mybir.dt.bfloat16
AF = mybir.ActivationFunctionType
OP = mybir.AluOpType

B, C, D, N = 32, 256, 128, 1024
NCORES = 8
IPC = B // NCORES  # items per core = 4
SCALE = 1.0 / float(np.sqrt(D))  # 0.08838834764831845
EPS = 1e-5
NTOT = float(C * N)  # layernorm element count per item/stream

# test.py can set {"trace": True}; harness path leaves this empty.
RUN_KWARGS = {}
LAST_RESULT = None


def _build():
    nc = bacc.Bacc("TRN2", target_bir_lowering=False, debug=False,
                   num_devices=NCORES)

    # ---- DRAM I/O (per-core shapes) ----
    fb_d = [nc.dram_tensor(n_, [IPC, 2, 128, N], BF16, kind="ExternalInput")
            for n_ in ("fsb", "fib")]
    wq_d = [nc.dram_tensor(n_, [128, 2, 128], BF16, kind="ExternalInput")
            for n_ in ("wq0", "wq1")]
    wk_d = [nc.dram_tensor(n_, [128, 2, 128], BF16, kind="ExternalInput")
            for n_ in ("wk0", "wk1")]
    wv_d = [nc.dram_tensor(n_, [128, 2, 256], BF16, kind="ExternalInput")
            for n_ in ("wv0", "wv1")]
    wf_d = nc.dram_tensor("wfuse", [128, 4, 256], BF16, kind="ExternalInput")
    fb_bias_d = nc.dram_tensor("fuseb", [128, 2], F32, kind="ExternalInput")
    lnw_d = nc.dram_tensor("lnw", [128, 2, 2], F32, kind="ExternalInput")
    lnb_d = nc.dram_tensor("lnb", [128, 2, 2], F32, kind="ExternalInput")
    out_d = [nc.dram_tensor(n_, [IPC, 2, 128, N], F32, kind="ExternalOutput")
             for n_ in ("out0", "out1")]

    with tile.TileContext(nc) as tc:
        consts = tc.alloc_tile_pool(name="consts", bufs=1)
        inp = tc.alloc_tile_pool(name="inp", bufs=2)
        work = tc.alloc_tile_pool(name="work", bufs=2)
        psum = tc.alloc_tile_pool(name="psum", bufs=2, space="PSUM")

        # ---- load constants ----
        wq = [consts.tile([128, 2, 128], BF16, name=f"wq{s}", tag=f"wq{s}")
              for s in range(2)]
        wk = [consts.tile([128, 2, 128], BF16, name=f"wk{s}", tag=f"wk{s}")
              for s in range(2)]
        wv = [consts.tile([128, 2, 256], BF16, name=f"wv{s}", tag=f"wv{s}")
              for s in range(2)]
        wf = consts.tile([128, 4, 256], BF16, name="wf", tag="wf")
        fbias = consts.tile([128, 2], F32, name="fbias", tag="fbias")
        lnw = consts.tile([128, 2, 2], F32, name="lnw", tag="lnw")
        lnb = consts.tile([128, 2, 2], F32, name="lnb", tag="lnb")
        ones_bf = consts.tile([128, 128], BF16, name="ones_bf", tag="ones_bf")
        ones_col = consts.tile([128, 1], F32, name="ones_col", tag="ones_col")
        ones_row = consts.tile([1, 128], F32, name="ones_row", tag="ones_row")
        for s in range(2):
            nc.sync.dma_start(out=wq[s][:], in_=wq_d[s][:])
            nc.sync.dma_start(out=wk[s][:], in_=wk_d[s][:])
            nc.sync.dma_start(out=wv[s][:], in_=wv_d[s][:])
        nc.sync.dma_start(out=wf[:], in_=wf_d[:])
        nc.sync.dma_start(out=fbias[:], in_=fb_bias_d[:])
        nc.sync.dma_start(out=lnw[:], in_=lnw_d[:])
        nc.sync.dma_start(out=lnb[:], in_=lnb_d[:])
        nc.vector.memset(ones_bf[:], 1.0)
        nc.vector.memset(ones_col[:], 1.0)
        nc.vector.memset(ones_row[:], 1.0)

        def conv_qk(w_t, f_t, name):
            """[128, N] = w.T @ f  (K=256 via 2 chunks); returns bf16 sbuf."""
            sb = work.tile([128, N], BF16, name=name, tag=name)
            for h in range(2):
                ps = psum.tile([128, 512], F32, name=f"ps_{name}", tag="work",
                               bufs=4)
                for kc in range(2):
                    nc.tensor.matmul(
                        ps[:],
                        lhsT=w_t[:, kc, :],
                        rhs=f_t[:, kc, h * 512:(h + 1) * 512],
                        start=(kc == 0), stop=(kc == 1))
                nc.vector.tensor_copy(out=sb[:, h * 512:(h + 1) * 512],
                                      in_=ps[:])
            return sb

        for i in range(IPC):
            # ---- input DMAs ----
            fb = []
            for s in range(2):
                t = inp.tile([128, 2, N], BF16, name=f"fb{s}", tag=f"fb{s}")
                nc.sync.dma_start(
                    out=t[:], in_=fb_d[s][i].rearrange("c p n -> p c n"))
                fb.append(t)

            stats = work.tile([128, 8], F32, name="stats", tag="stats")
            h_sb = []

            for s in range(2):
                # ================= attention for output stream s ==========
                q_sb = conv_qk(wq[1 - s], fb[1 - s], "q_sb")
                k_sb = conv_qk(wk[s], fb[s], "k_sb")

                # vT[m, c] computed directly: lhsT = f chunk slice, rhs = wv
                vt_sb = work.tile([128, 8, 256], BF16, name="vt_sb", tag="vt")
                for half in range(4):
                    ps_vt = psum.tile([128, 512], F32, name="ps_vt",
                                      tag="work", bufs=4)
                    for jj in range(2):
                        j = half * 2 + jj
                        for kc in range(2):
                            nc.tensor.matmul(
                                ps_vt[:, jj * 256:(jj + 1) * 256],
                                lhsT=fb[s][:, kc, j * 128:(j + 1) * 128],
                                rhs=wv[s][:, kc, :],
                                start=(kc == 0), stop=(kc == 1))
                    nc.vector.tensor_copy(
                        out=vt_sb[:, half * 2:(half + 1) * 2, :]
                        .rearrange("p a b -> p (a b)"),
                        in_=ps_vt[:])

                # S^T -> exp -> PV accumulate, chunk by chunk (per n-half)
                pv_ps = [psum.tile([128, N], F32, name=f"pv{t}", tag="pv")
                         for t in range(2)]
                expS = work.tile([128, 8, N], BF16, name="expS", tag="expS")
                for j in range(8):
                    for h in range(2):
                        ps_s = psum.tile([128, 512], F32, name="ps_s",
                                         tag="work", bufs=4)
                        nc.tensor.matmul(
                            ps_s[:],
                            lhsT=k_sb[:, j * 128:(j + 1) * 128],
                            rhs=q_sb[:, h * 512:(h + 1) * 512],
                            start=True, stop=True)
                        nc.scalar.activation(
                            out=expS[:, j, h * 512:(h + 1) * 512],
                            in_=ps_s[:], func=AF.Exp, scale=SCALE)
                    for t in range(2):
                        for h in range(2):
                            nc.tensor.matmul(
                                pv_ps[t][:, h * 512:(h + 1) * 512],
                                lhsT=vt_sb[:, j, t * 128:(t + 1) * 128],
                                rhs=expS[:, j, h * 512:(h + 1) * 512],
                                start=(j == 0), stop=(j == 7))

                # denominator: tree-add the 8 chunks, then ones-matmul
                # broadcasts the partition-sum to all 128 rows.
                dpar = [work.tile([128, N], BF16, name=f"dpar{a}",
                                  tag=f"dpar{a}") for a in range(4)]
                for a in range(4):
                    nc.vector.tensor_add(out=dpar[a][:], in0=expS[:, 2 * a, :],
                                         in1=expS[:, 2 * a + 1, :])
                nc.vector.tensor_add(out=dpar[0][:], in0=dpar[0][:],
                                     in1=dpar[1][:])
                nc.vector.tensor_add(out=dpar[2][:], in0=dpar[2][:],
                                     in1=dpar[3][:])
                nc.vector.tensor_add(out=dpar[0][:], in0=dpar[0][:],
                                     in1=dpar[2][:])
                rden = work.tile([128, N], F32, name="rden", tag="rden")
                for h in range(2):
                    ps_bc = psum.tile([128, 512], F32, name="ps_bc",
                                      tag="work", bufs=4)
                    nc.tensor.matmul(
                        ps_bc[:],
                        lhsT=ones_bf[:],
                        rhs=dpar[0][:, h * 512:(h + 1) * 512],
                        start=True, stop=True)
                    nc.vector.reciprocal_approx_fast(
                        out=rden[:, h * 512:(h + 1) * 512], in_=ps_bc[:])

                # normalize PV -> attn (bf16, feeds fuse matmul)
                attn_sb = work.tile([128, 2, N], BF16, name="attn_sb",
                                    tag="attn")
                for t in range(2):
                    nc.vector.tensor_tensor(
                        out=attn_sb[:, t, :], in0=pv_ps[t][:], in1=rden[:],
                        op=OP.mult)

                # ================= fuse + residual + LN stats =============
                h_t = work.tile([128, 2, N], BF16, name="h_t", tag=f"h{s}",
                                bufs=2)
                g_t = work.tile([128, 2, N], BF16, name="g_t", tag="g_t")
                for t in range(2):
                    for h in range(2):
                        ps_f = psum.tile([128, 512], F32, name="ps_f",
                                         tag="work", bufs=4)
                        for kc in range(4):
                            rhs = (fb[s][:, kc, h * 512:(h + 1) * 512]
                                   if kc < 2 else
                                   attn_sb[:, kc - 2, h * 512:(h + 1) * 512])
                            nc.tensor.matmul(
                                ps_f[:],
                                lhsT=wf[:, kc, t * 128:(t + 1) * 128],
                                rhs=rhs, start=(kc == 0), stop=(kc == 3))
                        nc.scalar.activation(
                            out=g_t[:, t, h * 512:(h + 1) * 512], in_=ps_f[:],
                            func=AF.Relu, bias=fbias[:, t:t + 1], scale=1.0)
                # residual (bf16) + LN sum accumulation, one op per c-tile
                for t in range(2):
                    c0 = s * 4 + t * 2
                    nc.vector.scalar_tensor_tensor(
                        out=h_t[:, t, :], in0=g_t[:, t, :], scalar=1.0,
                        in1=fb[s][:, t, :], op0=OP.mult, op1=OP.add,
                        accum_out=stats[:, c0:c0 + 1])
                # sum of squares (ACT, batched to limit act-table reloads)
                for t in range(2):
                    c0 = s * 4 + t * 2
                    dum = work.tile([128, N], BF16, name="dum", tag="dum")
                    nc.scalar.activation(
                        out=dum[:], in_=h_t[:, t, :], func=AF.Square,
                        accum_out=stats[:, c0 + 1:c0 + 2])
                h_sb.append(h_t)

            # ================= LN finalize (both streams) =================
            ps_st = psum.tile([1, 8], F32, name="ps_st", tag="work", bufs=4)
            nc.tensor.matmul(ps_st[:], lhsT=ones_col[:], rhs=stats[:],
                             start=True, stop=True)
            st = work.tile([1, 8], F32, name="st", tag="st")
            nc.vector.tensor_copy(out=st[:], in_=ps_st[:])
            # cols: s*4 + t*2 + k (k=0 sum, k=1 sumsq) -> tot over t
            st_r = st[:].rearrange("p (a b) -> p a b", a=2)  # a=s, b=(t,k)
            tot = work.tile([1, 4], F32, name="tot", tag="tot")
            nc.vector.tensor_add(
                out=tot[:].rearrange("p (a b) -> p a b", a=2),
                in0=st_r[:, :, 0:2],
                in1=st_r[:, :, 2:4])
            # tot = [s0_sum, s0_sq, s1_sum, s1_sq] -> moments = tot / NTOT
            mom = work.tile([1, 4], F32, name="mom", tag="mom")
            nc.vector.tensor_scalar(out=mom[:], in0=tot[:],
                                    scalar1=1.0 / NTOT, scalar2=None,
                                    op0=OP.mult)
            # var = E[x^2] - mu^2 ; mr = [rstd0, rstd1, -mu0, -mu1]
            mom_r = mom[:].rearrange("p (a b) -> p a b", a=2)
            mu_ap = mom_r[:, :, 0]
            m2_ap = mom_r[:, :, 1]
            musq = work.tile([1, 2], F32, name="musq", tag="musq")
            nc.vector.tensor_tensor(out=musq[:], in0=mu_ap, in1=mu_ap,
                                    op=OP.mult)
            var = work.tile([1, 2], F32, name="var", tag="var")
            nc.vector.tensor_tensor(out=var[:], in0=m2_ap, in1=musq[:],
                                    op=OP.subtract)
            nc.vector.tensor_scalar(out=var[:], in0=var[:], scalar1=EPS,
                                    scalar2=None, op0=OP.add)
            sd = work.tile([1, 2], F32, name="sd", tag="sd")
            nc.scalar.activation(out=sd[:], in_=var[:], func=AF.Sqrt)
            mr = work.tile([1, 4], F32, name="mr", tag="mr")
            nc.vector.reciprocal(out=mr[:, 0:2], in_=sd[:])
            nc.vector.tensor_scalar(out=mr[:, 2:4], in0=mu_ap,
                                    scalar1=-1.0, scalar2=None, op0=OP.mult)
            # broadcast [1,4] -> [128,4] via K=1 f32 matmul
            ps_mr = psum.tile([128, 4], F32, name="ps_mr", tag="work", bufs=4)
            nc.tensor.matmul(ps_mr[:], lhsT=ones_row[:], rhs=mr[:],
                             start=True, stop=True)
            mrb = work.tile([128, 4], F32, name="mrb", tag="mrb")
            nc.vector.tensor_copy(out=mrb[:], in_=ps_mr[:])
            # A = lnw * rstd ; Bb = lnb + (-mu) * A ; out = h * A + Bb
            for s in range(2):
                Asb = work.tile([128, 2], F32, name="Asb", tag=f"A{s}")
                nc.vector.tensor_scalar(
                    out=Asb[:], in0=lnw[:, s, :], scalar1=mrb[:, s:s + 1],
                    scalar2=None, op0=OP.mult)
                Bsb = work.tile([128, 2], F32, name="Bsb", tag=f"B{s}")
                nc.vector.scalar_tensor_tensor(
                    out=Bsb[:], in0=Asb[:], scalar=mrb[:, 2 + s:3 + s],
                    in1=lnb[:, s, :], op0=OP.mult, op1=OP.add)
                for t in range(2):
                    o_t = work.tile([128, N], F32, name="o_t", tag="o_t",
                                    bufs=4)
                    nc.vector.tensor_scalar(
                        out=o_t[:], in0=h_sb[s][:, t, :],
                        scalar1=Asb[:, t:t + 1], scalar2=Bsb[:, t:t + 1],
                        op0=OP.mult, op1=OP.add)
                    nc.sync.dma_start(out=out_d[s][i, t], in_=o_t[:])

        psum.release()
        work.release()
        inp.release()
        consts.release()

    nc.compile()
    return nc


_NC_CACHE = None


def _get_nc():
    global _NC_CACHE
    if _NC_CACHE is None:
        _NC_CACHE = _build()
    return _NC_CACHE


def kernel(fs, fi, qs_w, ks_w, vs_w, qi_w, ki_w, vi_w,
           fuse_w, fuse_b, ln_s_w, ln_s_b, ln_i_w, ln_i_b):
    global LAST_RESULT
    fs = np.asarray(fs, np.float32)
    fi = np.asarray(fi, np.float32)

    def prep_f(x):
        # (B, C, H, W) -> per-core [IPC, 2, 128, N]
        return x.reshape(NCORES, IPC, 2, 128, N)

    def prep_w_qk(w):  # (128, 256) -> lhsT layout [128p, 2kc, 128m]
        wt = np.ascontiguousarray(np.asarray(w, np.float32).T)  # (256, 128)
        return np.ascontiguousarray(
            wt.reshape(2, 128, 128).transpose(1, 0, 2)).astype(
                ml_dtypes.bfloat16)

    def prep_w_v(w):  # (256, 256) -> rhs layout [128p, 2kc, 256c]
        wt = np.ascontiguousarray(np.asarray(w, np.float32).T)
        return np.ascontiguousarray(
            wt.reshape(2, 128, 256).transpose(1, 0, 2)).astype(
                ml_dtypes.bfloat16)

    fs_sh = prep_f(fs)
    fi_sh = prep_f(fi)
    fs_bf = fs_sh.astype(ml_dtypes.bfloat16)
    fi_bf = fi_sh.astype(ml_dtypes.bfloat16)

    wq0 = prep_w_qk(qs_w)
    wq1 = prep_w_qk(qi_w)
    wk0 = prep_w_qk(ks_w)
    wk1 = prep_w_qk(ki_w)
    wv0 = prep_w_v(vs_w)
    wv1 = prep_w_v(vi_w)
    wfuse = np.ascontiguousarray(
        np.asarray(fuse_w, np.float32).T.reshape(4, 128, 256)
        .transpose(1, 0, 2)).astype(ml_dtypes.bfloat16)
    fuseb = np.ascontiguousarray(
        np.asarray(fuse_b, np.float32).reshape(2, 128).T)
    lnw = np.ascontiguousarray(
        np.stack([np.asarray(ln_s_w, np.float32).reshape(256),
                  np.asarray(ln_i_w, np.float32).reshape(256)])
        .reshape(2, 2, 128).transpose(2, 0, 1))
    lnb = np.ascontiguousarray(
        np.stack([np.asarray(ln_s_b, np.float32).reshape(256),
                  np.asarray(ln_i_b, np.float32).reshape(256)])
        .reshape(2, 2, 128).transpose(2, 0, 1))

    in_maps = []
    for c in range(NCORES):
        in_maps.append({
            "fsb": np.ascontiguousarray(fs_bf[c]),
            "fib": np.ascontiguousarray(fi_bf[c]),
            "wq0": wq0, "wq1": wq1, "wk0": wk0, "wk1": wk1,
            "wv0": wv0, "wv1": wv1, "wfuse": wfuse, "fuseb": fuseb,
            "lnw": lnw, "lnb": lnb,
        })

    nc = _get_nc()
    res = run_bass_kernel_spmd(nc, in_maps, core_ids=list(range(NCORES)),
                               **RUN_KWARGS)
    LAST_RESULT = res

    fs_out = np.empty((NCORES, IPC, 2, 128, N), np.float32)
    fi_out = np.empty((NCORES, IPC, 2, 128, N), np.float32)
    for c in range(NCORES):
        fs_out[c] = res.results[c]["out0"]
        fi_out[c] = res.results[c]["out1"]
    fs_out = fs_out.reshape(B, C, 32, 32)
    fi_out = fi_out.reshape(B, C, 32, 32)
    return fs_out, fi_out


# revision 6
# speedup vs baseline: 1.2029x; 1.0729x over previous
"""Trainium2 Bass kernel for nn_CrossAttention2d (B=32, C=256, INNER=128, H=W=32).

Sharding: pure data parallel — batch 32 split as 4 items per core across 8
NeuronCores; all weights replicated. No collectives.

Per item (N = H*W = 1024 tokens, C = 256 channels, D = 128 inner):
  attention for output stream s (s=0 -> fs side, s=1 -> fi side):
      q = wq[1-s] @ f[1-s], k = wk[s] @ f[s]   (fp8 DoubleRow, x32 prescale)
      vT[m, c] = (wv[s] @ f[s]).T   -- computed directly via DoubleRow with
                 f-slices as the stationary operand
      S^T[m, n] = sum_d k[d, m] q[d, n]        (bf16 PE, m-tiles of 128)
      E = exp(S^T / (1024 sqrt(D)))            (ACT, psum -> fp8 sbuf)
      O_un[c, n] = sum_m vT[m, c] E[m, n]      (fp8 DoubleRow, 4 chunk-pairs)
      den[n] via ones.T @ E (fp8 DoubleRow) broadcast to 128 rows
      attn = O_un * (1/32) * (1/den)           (DVE scalar_tensor_tensor)
  fuse: g = relu(Wf @ [f[s]; attn] + b)        (bf16 PE + ACT)
  h = g + f[s] (bf16 residual); LayerNorm over all (C,N) of h; LN stats via
  DVE accum_out + PE partition-reduce; out = h * A + B (DVE tensor_scalar).
  The LN scalar epilogue of item i is emitted inside item i+1 (software
  pipelining) so its serial tiny-op chain hides behind PE work.

Matmul convention: out[M, N] = lhsT.T @ rhs, lhsT = [K<=128, M<=128] (K on
partitions), rhs = [K, N<=512], out in PSUM f32 (one bank per matmul).
DoubleRow: lhsT [Ki, 2, M], rhs [Ki, 2, N] fp8 -> contracts 2*Ki.
PSUM: "pv" tag 2x[128,1024] (4 banks) + "work" tag 4x[128,512] (4 banks).
"""

import numpy as np
import ml_dtypes

import concourse.bacc as bacc
import concourse.bass as bass
import concourse.tile as tile
from concourse import mybir
from concourse.bass_utils import run_bass_kernel_spmd

F32 = mybir.dt.float32
BF16 = mybir.dt.bfloat16
FP8 = mybir.dt.float8e4
DR = mybir.MatmulPerfMode.DoubleRow
AF = mybir.ActivationFunctionType
OP = mybir.AluOpType

B, C, D, N = 32, 256, 128, 1024
NCORES = 8
IPC = B // NCORES  # items per core = 4
WSCALE = 32.0  # fp8 weight prescale (w*32 keeps N(0,0.02) in e4m3 range)
EXP_SCALE = (1.0 / float(np.sqrt(D))) / (WSCALE * WSCALE)
EPS = 1e-5
NTOT = float(C * N)  # layernorm element count per item/stream

# test.py can set {"trace": True}; harness path leaves this empty.
RUN_KWARGS = {}
LAST_RESULT = None


def _build():
    nc = bacc.Bacc("TRN2", target_bir_lowering=False, debug=False,
                   num_devices=NCORES)

    # ---- DRAM I/O (per-core shapes) ----
    fb_d = [nc.dram_tensor(n_, [IPC, 2, 128, N], BF16, kind="ExternalInput")
            for n_ in ("fsb", "fib")]
    f8_d = [nc.dram_tensor(n_, [IPC, 2, 128, N], FP8, kind="ExternalInput")
            for n_ in ("fs8", "fi8")]
    wq_d = [nc.dram_tensor(n_, [128, 2, 128], FP8, kind="ExternalInput")
            for n_ in ("wq0", "wq1")]
    wk_d = [nc.dram_tensor(n_, [128, 2, 128], FP8, kind="ExternalInput")
            for n_ in ("wk0", "wk1")]
    wv_d = [nc.dram_tensor(n_, [128, 2, 256], FP8, kind="ExternalInput")
            for n_ in ("wv0", "wv1")]
    wf_d = nc.dram_tensor("wfuse", [128, 4, 256], BF16, kind="ExternalInput")
    fb_bias_d = nc.dram_tensor("fuseb", [128, 2], F32, kind="ExternalInput")
    lnw_d = nc.dram_tensor("lnw", [128, 2, 2], F32, kind="ExternalInput")
    lnb_d = nc.dram_tensor("lnb", [128, 2, 2], F32, kind="ExternalInput")
    out_d = [nc.dram_tensor(n_, [IPC, 2, 128, N], F32, kind="ExternalOutput")
             for n_ in ("out0", "out1")]

    with tile.TileContext(nc) as tc:
        consts = tc.alloc_tile_pool(name="consts", bufs=1)
        inp = tc.alloc_tile_pool(name="inp", bufs=2)
        work = tc.alloc_tile_pool(name="work", bufs=2)
        psum = tc.alloc_tile_pool(name="psum", bufs=2, space="PSUM")

        # ---- load constants ----
        wq = [consts.tile([128, 2, 128], FP8, name=f"wq{s}", tag=f"wq{s}")
              for s in range(2)]
        wk = [consts.tile([128, 2, 128], FP8, name=f"wk{s}", tag=f"wk{s}")
              for s in range(2)]
        wv = [consts.tile([128, 2, 256], FP8, name=f"wv{s}", tag=f"wv{s}")
              for s in range(2)]
        wf = consts.tile([128, 4, 256], BF16, name="wf", tag="wf")
        fbias = consts.tile([128, 2], F32, name="fbias", tag="fbias")
        lnw = consts.tile([128, 2, 2], F32, name="lnw", tag="lnw")
        lnb = consts.tile([128, 2, 2], F32, name="lnb", tag="lnb")
        ones8 = consts.tile([128, 2, 128], FP8, name="ones8", tag="ones8")
        ones_col = consts.tile([128, 1], F32, name="ones_col", tag="ones_col")
        ones_row = consts.tile([1, 128], F32, name="ones_row", tag="ones_row")
        for s in range(2):
            nc.sync.dma_start(out=wq[s][:], in_=wq_d[s][:])
            nc.sync.dma_start(out=wk[s][:], in_=wk_d[s][:])
            nc.sync.dma_start(out=wv[s][:], in_=wv_d[s][:])
        nc.sync.dma_start(out=wf[:], in_=wf_d[:])
        nc.sync.dma_start(out=fbias[:], in_=fb_bias_d[:])
        nc.sync.dma_start(out=lnw[:], in_=lnw_d[:])
        nc.sync.dma_start(out=lnb[:], in_=lnb_d[:])
        nc.vector.memset(ones8[:], 1.0)
        nc.vector.memset(ones_col[:], 1.0)
        nc.vector.memset(ones_row[:], 1.0)

        def conv_qk(w_t, f8_t, name):
            """[128, N] = (32w).T @ f via fp8 DoubleRow; bf16 sbuf out."""
            sb = work.tile([128, N], BF16, name=name, tag=name)
            for h in range(2):
                ps = psum.tile([128, 512], F32, name=f"ps_{name}", tag="work",
                               bufs=4)
                nc.tensor.matmul(
                    ps[:], lhsT=w_t[:],
                    rhs=f8_t[:, :, h * 512:(h + 1) * 512],
                    start=True, stop=True, perf_mode=DR)
                nc.vector.tensor_copy(out=sb[:, h * 512:(h + 1) * 512],
                                      in_=ps[:])
            return sb

        # -------- per-item state carried into the next item (LN epilogue)
        pend = []  # list of (i, stats, h_sb)

        def ln_epilogue():
            if not pend:
                return
            i, stats, h_sb = pend.pop()
            ps_st = psum.tile([1, 8], F32, name="ps_st", tag="work", bufs=4)
            nc.tensor.matmul(ps_st[:], lhsT=ones_col[:], rhs=stats[:],
                             start=True, stop=True)
            st = work.tile([1, 8], F32, name="st", tag="st")
            nc.vector.tensor_copy(out=st[:], in_=ps_st[:])
            # cols: s*4 + t*2 + k (k=0 sum, k=1 sumsq) -> tot over t
            st_r = st[:].rearrange("p (a b) -> p a b", a=2)  # a=s, b=(t,k)
            tot = work.tile([1, 4], F32, name="tot", tag="tot")
            nc.vector.tensor_add(
                out=tot[:].rearrange("p (a b) -> p a b", a=2),
                in0=st_r[:, :, 0:2],
                in1=st_r[:, :, 2:4])
            # tot = [s0_sum, s0_sq, s1_sum, s1_sq] -> moments = tot / NTOT
            mom = work.tile([1, 4], F32, name="mom", tag="mom")
            nc.vector.tensor_scalar(out=mom[:], in0=tot[:],
                                    scalar1=1.0 / NTOT, scalar2=None,
                                    op0=OP.mult)
            # var = E[x^2] - mu^2 ; mr = [rstd0, rstd1, -mu0, -mu1]
            mom_r = mom[:].rearrange("p (a b) -> p a b", a=2)
            mu_ap = mom_r[:, :, 0]
            m2_ap = mom_r[:, :, 1]
            musq = work.tile([1, 2], F32, name="musq", tag="musq")
            nc.vector.tensor_tensor(out=musq[:], in0=mu_ap, in1=mu_ap,
                                    op=OP.mult)
            var = work.tile([1, 2], F32, name="var", tag="var")
            nc.vector.tensor_tensor(out=var[:], in0=m2_ap, in1=musq[:],
                                    op=OP.subtract)
            nc.vector.tensor_scalar(out=var[:], in0=var[:], scalar1=EPS,
                                    scalar2=None, op0=OP.add)
            sd = work.tile([1, 2], F32, name="sd", tag="sd")
            nc.scalar.activation(out=sd[:], in_=var[:], func=AF.Sqrt)
            mr = work.tile([1, 4], F32, name="mr", tag="mr")
            nc.vector.reciprocal(out=mr[:, 0:2], in_=sd[:])
            nc.vector.tensor_scalar(out=mr[:, 2:4], in0=mu_ap,
                                    scalar1=-1.0, scalar2=None, op0=OP.mult)
            # broadcast [1,4] -> [128,4] via K=1 f32 matmul
            ps_mr = psum.tile([128, 4], F32, name="ps_mr", tag="work", bufs=4)
            nc.tensor.matmul(ps_mr[:], lhsT=ones_row[:], rhs=mr[:],
                             start=True, stop=True)
            mrb = work.tile([128, 4], F32, name="mrb", tag="mrb")
            nc.vector.tensor_copy(out=mrb[:], in_=ps_mr[:])
            # A = lnw * rstd ; Bb = lnb + (-mu) * A ; out = h * A + Bb
            for s in range(2):
                Asb = work.tile([128, 2], F32, name="Asb", tag=f"A{s}")
                nc.vector.tensor_scalar(
                    out=Asb[:], in0=lnw[:, s, :], scalar1=mrb[:, s:s + 1],
                    scalar2=None, op0=OP.mult)
                Bsb = work.tile([128, 2], F32, name="Bsb", tag=f"B{s}")
                nc.vector.scalar_tensor_tensor(
                    out=Bsb[:], in0=Asb[:], scalar=mrb[:, 2 + s:3 + s],
                    in1=lnb[:, s, :], op0=OP.mult, op1=OP.add)
                for t in range(2):
                    o_t = work.tile([128, N], F32, name="o_t", tag="o_t",
                                    bufs=4)
                    nc.vector.tensor_scalar(
                        out=o_t[:], in0=h_sb[s][:, t, :],
                        scalar1=Asb[:, t:t + 1], scalar2=Bsb[:, t:t + 1],
                        op0=OP.mult, op1=OP.add)
                    nc.sync.dma_start(out=out_d[s][i, t], in_=o_t[:])

        for i in range(IPC):
            # ---- input DMAs ----
            fb = []
            f8 = []
            for s in range(2):
                t = inp.tile([128, 2, N], BF16, name=f"fb{s}", tag=f"fb{s}")
                nc.sync.dma_start(
                    out=t[:], in_=fb_d[s][i].rearrange("c p n -> p c n"))
                fb.append(t)
                t8 = inp.tile([128, 2, N], FP8, name=f"f8_{s}", tag=f"f8_{s}")
                nc.sync.dma_start(
                    out=t8[:], in_=f8_d[s][i].rearrange("c p n -> p c n"))
                f8.append(t8)

            stats = work.tile([128, 8], F32, name="stats", tag="stats")
            h_sb = []

            for s in range(2):
                # ================= attention for output stream s ==========
                q_sb = conv_qk(wq[1 - s], f8[1 - s], "q_sb")
                k_sb = conv_qk(wk[s], f8[s], "k_sb")

                # vT[m, c] via DoubleRow: stationary = f8 slice pair
                vt_sb = work.tile([128, 8, 256], FP8, name="vt_sb", tag="vt")
                for half in range(4):
                    ps_vt = psum.tile([128, 512], F32, name="ps_vt",
                                      tag="work", bufs=4)
                    for jj in range(2):
                        j = half * 2 + jj
                        nc.tensor.matmul(
                            ps_vt[:, jj * 256:(jj + 1) * 256],
                            lhsT=f8[s][:, :, j * 128:(j + 1) * 128],
                            rhs=wv[s][:],
                            start=True, stop=True, perf_mode=DR)
                    nc.vector.tensor_copy(
                        out=vt_sb[:, half * 2:(half + 1) * 2, :]
                        .rearrange("p a b -> p (a b)"),
                        in_=ps_vt[:])

                # S^T -> exp(fp8) ; PV accumulates DoubleRow chunk-pairs
                pv_ps = [psum.tile([128, N], F32, name=f"pv{t}", tag="pv")
                         for t in range(2)]
                expS = work.tile([128, 8, N], FP8, name="expS", tag="expS")
                for j in range(8):
                    for h in range(2):
                        ps_s = psum.tile([128, 512], F32, name="ps_s",
                                         tag="work", bufs=4)
                        nc.tensor.matmul(
                            ps_s[:],
                            lhsT=k_sb[:, j * 128:(j + 1) * 128],
                            rhs=q_sb[:, h * 512:(h + 1) * 512],
                            start=True, stop=True)
                        nc.scalar.activation(
                            out=expS[:, j, h * 512:(h + 1) * 512],
                            in_=ps_s[:], func=AF.Exp, scale=EXP_SCALE)
                    if j % 2 == 1:
                        jp = j // 2  # chunk pair (2jp, 2jp+1) ready
                        for t in range(2):
                            for h in range(2):
                                nc.tensor.matmul(
                                    pv_ps[t][:, h * 512:(h + 1) * 512],
                                    lhsT=vt_sb[:, 2 * jp:2 * jp + 2,
                                               t * 128:(t + 1) * 128],
                                    rhs=expS[:, 2 * jp:2 * jp + 2,
                                             h * 512:(h + 1) * 512],
                                    start=(jp == 0), stop=(jp == 3),
                                    perf_mode=DR)

                # denominator: ones.T @ E accumulated over chunk pairs,
                # result rows are all equal to den[n]; then reciprocal.
                rden = work.tile([128, N], F32, name="rden", tag="rden")
                for h in range(2):
                    ps_bc = psum.tile([128, 512], F32, name="ps_bc",
                                      tag="work", bufs=4)
                    for jp in range(4):
                        nc.tensor.matmul(
                            ps_bc[:],
                            lhsT=ones8[:],
                            rhs=expS[:, 2 * jp:2 * jp + 2,
                                     h * 512:(h + 1) * 512],
                            start=(jp == 0), stop=(jp == 3), perf_mode=DR)
                    nc.vector.reciprocal_approx_fast(
                        out=rden[:, h * 512:(h + 1) * 512], in_=ps_bc[:])

                # normalize PV -> attn (bf16, feeds fuse matmul)
                attn_sb = work.tile([128, 2, N], BF16, name="attn_sb",
                                    tag="attn")
                for t in range(2):
                    nc.vector.scalar_tensor_tensor(
                        out=attn_sb[:, t, :], in0=pv_ps[t][:],
                        scalar=1.0 / WSCALE, in1=rden[:],
                        op0=OP.mult, op1=OP.mult)

                # ================= fuse + residual + LN stats =============
                # f-half chunks first (no attn dependency), then attn half.
                ps_f = {}
                for t in range(2):
                    for h in range(2):
                        p = psum.tile([128, 512], F32, name="ps_f",
                                      tag="work", bufs=4)
                        ps_f[(t, h)] = p
                        for kc in range(2):
                            nc.tensor.matmul(
                                p[:],
                                lhsT=wf[:, kc, t * 128:(t + 1) * 128],
                                rhs=fb[s][:, kc, h * 512:(h + 1) * 512],
                                start=(kc == 0), stop=False)
                h_t = work.tile([128, 2, N], BF16, name="h_t", tag=f"h{s}",
                                bufs=2)
                g_t = work.tile([128, 2, N], BF16, name="g_t", tag="g_t")
                for t in range(2):
                    for h in range(2):
                        p = ps_f[(t, h)]
                        for kc in range(2, 4):
                            nc.tensor.matmul(
                                p[:],
                                lhsT=wf[:, kc, t * 128:(t + 1) * 128],
                                rhs=attn_sb[:, kc - 2, h * 512:(h + 1) * 512],
                                start=False, stop=(kc == 3))
                        nc.scalar.activation(
                            out=g_t[:, t, h * 512:(h + 1) * 512], in_=p[:],
                            func=AF.Relu, bias=fbias[:, t:t + 1], scale=1.0)
                # residual (bf16) + LN sum accum; sumsq on DVE
                for t in range(2):
                    c0 = s * 4 + t * 2
                    nc.vector.scalar_tensor_tensor(
                        out=h_t[:, t, :], in0=g_t[:, t, :], scalar=1.0,
                        in1=fb[s][:, t, :], op0=OP.mult, op1=OP.add,
                        accum_out=stats[:, c0:c0 + 1])
                    dum = work.tile([128, N], BF16, name="dum", tag="dum")
                    nc.scalar.activation(
                        out=dum[:], in_=h_t[:, t, :], func=AF.Square,
                        accum_out=stats[:, c0 + 1:c0 + 2])
                h_sb.append(h_t)

            pend.append((i, stats, h_sb))
            ln_epilogue()

        ln_epilogue()

        psum.release()
        work.release()
        inp.release()
        consts.release()

    nc.compile()
    return nc


_NC_CACHE = None


def _get_nc():
    global _NC_CACHE
    if _NC_CACHE is None:
        _NC_CACHE = _build()
    return _NC_CACHE


def kernel(fs, fi, qs_w, ks_w, vs_w, qi_w, ki_w, vi_w,
           fuse_w, fuse_b, ln_s_w, ln_s_b, ln_i_w, ln_i_b):
    global LAST_RESULT
    fs = np.asarray(fs, np.float32)
    fi = np.asarray(fi, np.float32)

    def prep_f(x):
        # (B, C, H, W) -> per-core [IPC, 2, 128, N]
        return x.reshape(NCORES, IPC, 2, 128, N)

    def prep_w_qk(w):  # (128, 256) -> lhsT layout [128p, 2kc, 128m] * 32
        wt = np.ascontiguousarray(np.asarray(w, np.float32).T) * WSCALE
        return np.ascontiguousarray(
            wt.reshape(2, 128, 128).transpose(1, 0, 2)).astype(
                ml_dtypes.float8_e4m3)

    def prep_w_v(w):  # (256, 256) -> rhs layout [128p, 2kc, 256c] * 32
        wt = np.ascontiguousarray(np.asarray(w, np.float32).T) * WSCALE
        return np.ascontiguousarray(
            wt.reshape(2, 128, 256).transpose(1, 0, 2)).astype(
                ml_dtypes.float8_e4m3)

    fs_sh = prep_f(fs)
    fi_sh = prep_f(fi)
    fs_bf = fs_sh.astype(ml_dtypes.bfloat16)
    fi_bf = fi_sh.astype(ml_dtypes.bfloat16)
    fs_q8 = fs_sh.astype(ml_dtypes.float8_e4m3)
    fi_q8 = fi_sh.astype(ml_dtypes.float8_e4m3)

    wq0 = prep_w_qk(qs_w)
    wq1 = prep_w_qk(qi_w)
    wk0 = prep_w_qk(ks_w)
    wk1 = prep_w_qk(ki_w)
    wv0 = prep_w_v(vs_w)
    wv1 = prep_w_v(vi_w)
    wfuse = np.ascontiguousarray(
        np.asarray(fuse_w, np.float32).T.reshape(4, 128, 256)
        .transpose(1, 0, 2)).astype(ml_dtypes.bfloat16)
    fuseb = np.ascontiguousarray(
        np.asarray(fuse_b, np.float32).reshape(2, 128).T)
    lnw = np.ascontiguousarray(
        np.stack([np.asarray(ln_s_w, np.float32).reshape(256),
                  np.asarray(ln_i_w, np.float32).reshape(256)])
        .reshape(2, 2, 128).transpose(2, 0, 1))
    lnb = np.ascontiguousarray(
        np.stack([np.asarray(ln_s_b, np.float32).reshape(256),
                  np.asarray(ln_i_b, np.float32).reshape(256)])
        .reshape(2, 2, 128).transpose(2, 0, 1))

    in_maps = []
    for c in range(NCORES):
        in_maps.append({
            "fsb": np.ascontiguousarray(fs_bf[c]),
            "fib": np.ascontiguousarray(fi_bf[c]),
            "fs8": np.ascontiguousarray(fs_q8[c]),
            "fi8": np.ascontiguousarray(fi_q8[c]),
            "wq0": wq0, "wq1": wq1, "wk0": wk0, "wk1": wk1,
            "wv0": wv0, "wv1": wv1, "wfuse": wfuse, "fuseb": fuseb,
            "lnw": lnw, "lnb": lnb,
        })

    nc = _get_nc()
    res = run_bass_kernel_spmd(nc, in_maps, core_ids=list(range(NCORES)),
                               **RUN_KWARGS)
    LAST_RESULT = res

    fs_out = np.empty((NCORES, IPC, 2, 128, N), np.float32)
    fi_out = np.empty((NCORES, IPC, 2, 128, N), np.float32)
    for c in range(NCORES):
        fs_out[c] = res.results[c]["out0"]
        fi_out[c] = res.results[c]["out1"]
    fs_out = fs_out.reshape(B, C, 32, 32)
    fi_out = fi_out.reshape(B, C, 32, 32)
    return fs_out, fi_out


# revision 7
# speedup vs baseline: 1.2061x; 1.0027x over previous
"""Trainium2 Bass kernel for nn_CrossAttention2d (B=32, C=256, INNER=128, H=W=32).

Sharding: pure data parallel — batch 32 split as 4 items per core across 8
NeuronCores; all weights replicated. No collectives.

Per item (N = H*W = 1024 tokens, C = 256 channels, D = 128 inner):
  attention for output stream s (s=0 -> fs side, s=1 -> fi side):
      q = wq[1-s] @ f[1-s], k = wk[s] @ f[s]   (fp8 DoubleRow, x32 prescale)
      vT[m, c] = (wv[s] @ f[s]).T   -- computed directly via DoubleRow with
                 f-slices as the stationary operand
      S^T[m, n] = sum_d k[d, m] q[d, n]        (bf16 PE, m-tiles of 128)
      E = exp(S^T / (1024 sqrt(D)))            (ACT, psum -> fp8 sbuf)
      O_un[c, n] = sum_m vT[m, c] E[m, n]      (fp8 DoubleRow, 4 chunk-pairs)
      den[n] via ones.T @ E (fp8 DoubleRow) broadcast to 128 rows
      attn = O_un * (1/32) * (1/den)           (DVE scalar_tensor_tensor)
  fuse: g = relu(Wf @ [f[s]; attn] + b)        (bf16 PE + ACT)
  h = g + f[s] (bf16 residual); LayerNorm over all (C,N) of h; LN stats via
  DVE accum_out + PE partition-reduce; out = h * A + B (DVE tensor_scalar).
  The LN scalar epilogue of item i is emitted inside item i+1 (software
  pipelining) so its serial tiny-op chain hides behind PE work.

Matmul convention: out[M, N] = lhsT.T @ rhs, lhsT = [K<=128, M<=128] (K on
partitions), rhs = [K, N<=512], out in PSUM f32 (one bank per matmul).
DoubleRow: lhsT [Ki, 2, M], rhs [Ki, 2, N] fp8 -> contracts 2*Ki.
PSUM: "pv" tag 2x[128,1024] (4 banks) + "work" tag 4x[128,512] (4 banks).
"""

import numpy as np
import ml_dtypes

import concourse.bacc as bacc
import concourse.bass as bass
import concourse.tile as tile
from concourse import mybir
from concourse.bass_utils import run_bass_kernel_spmd

F32 = mybir.dt.float32
BF16 = mybir.dt.bfloat16
FP8 = mybir.dt.float8e4
DR = mybir.MatmulPerfMode.DoubleRow
AF = mybir.ActivationFunctionType
OP = mybir.AluOpType

B, C, D, N = 32, 256, 128, 1024
NCORES = 8
IPC = B // NCORES  # items per core = 4
WSCALE = 32.0  # fp8 weight prescale (w*32 keeps N(0,0.02) in e4m3 range)
EXP_SCALE = (1.0 / float(np.sqrt(D))) / (WSCALE * WSCALE)
EPS = 1e-5
NTOT = float(C * N)  # layernorm element count per item/stream

# test.py can set {"trace": True}; harness path leaves this empty.
RUN_KWARGS = {}
LAST_RESULT = None


def _build():
    nc = bacc.Bacc("TRN2", target_bir_lowering=False, debug=False,
                   num_devices=NCORES)

    # ---- DRAM I/O (per-core shapes) ----
    fb_d = [nc.dram_tensor(n_, [IPC, 2, 128, N], BF16, kind="ExternalInput")
            for n_ in ("fsb", "fib")]
    f8_d = [nc.dram_tensor(n_, [IPC, 2, 128, N], FP8, kind="ExternalInput")
            for n_ in ("fs8", "fi8")]
    wq_d = [nc.dram_tensor(n_, [128, 2, 128], FP8, kind="ExternalInput")
            for n_ in ("wq0", "wq1")]
    wk_d = [nc.dram_tensor(n_, [128, 2, 128], FP8, kind="ExternalInput")
            for n_ in ("wk0", "wk1")]
    wv_d = [nc.dram_tensor(n_, [128, 2, 256], FP8, kind="ExternalInput")
            for n_ in ("wv0", "wv1")]
    wf_d = nc.dram_tensor("wfuse", [128, 4, 256], BF16, kind="ExternalInput")
    fb_bias_d = nc.dram_tensor("fuseb", [128, 2], F32, kind="ExternalInput")
    lnw_d = nc.dram_tensor("lnw", [128, 2, 2], F32, kind="ExternalInput")
    lnb_d = nc.dram_tensor("lnb", [128, 2, 2], F32, kind="ExternalInput")
    out_d = [nc.dram_tensor(n_, [IPC, 2, 128, N], F32, kind="ExternalOutput")
             for n_ in ("out0", "out1")]

    with tile.TileContext(nc) as tc:
        consts = tc.alloc_tile_pool(name="consts", bufs=1)
        inp = tc.alloc_tile_pool(name="inp", bufs=2)
        work = tc.alloc_tile_pool(name="work", bufs=2)
        psum = tc.alloc_tile_pool(name="psum", bufs=2, space="PSUM")

        # ---- load constants ----
        wq = [consts.tile([128, 2, 128], FP8, name=f"wq{s}", tag=f"wq{s}")
              for s in range(2)]
        wk = [consts.tile([128, 2, 128], FP8, name=f"wk{s}", tag=f"wk{s}")
              for s in range(2)]
        wv = [consts.tile([128, 2, 256], FP8, name=f"wv{s}", tag=f"wv{s}")
              for s in range(2)]
        wf = consts.tile([128, 4, 256], BF16, name="wf", tag="wf")
        fbias = consts.tile([128, 2], F32, name="fbias", tag="fbias")
        lnw = consts.tile([128, 2, 2], F32, name="lnw", tag="lnw")
        lnb = consts.tile([128, 2, 2], F32, name="lnb", tag="lnb")
        ones8 = consts.tile([128, 2, 128], FP8, name="ones8", tag="ones8")
        ones_col = consts.tile([128, 1], F32, name="ones_col", tag="ones_col")
        ones_row = consts.tile([1, 128], F32, name="ones_row", tag="ones_row")
        for s in range(2):
            nc.sync.dma_start(out=wq[s][:], in_=wq_d[s][:])
            nc.sync.dma_start(out=wk[s][:], in_=wk_d[s][:])
            nc.sync.dma_start(out=wv[s][:], in_=wv_d[s][:])
        nc.sync.dma_start(out=wf[:], in_=wf_d[:])
        nc.sync.dma_start(out=fbias[:], in_=fb_bias_d[:])
        nc.sync.dma_start(out=lnw[:], in_=lnw_d[:])
        nc.sync.dma_start(out=lnb[:], in_=lnb_d[:])
        nc.vector.memset(ones8[:], 1.0)
        nc.vector.memset(ones_col[:], 1.0)
        nc.vector.memset(ones_row[:], 1.0)

        def conv_qk(w_t, f8_t, name):
            """[128, N] = (32w).T @ f via fp8 DoubleRow; bf16 sbuf out."""
            sb = work.tile([128, N], BF16, name=name, tag=name)
            for h in range(2):
                ps = psum.tile([128, 512], F32, name=f"ps_{name}", tag="work",
                               bufs=4)
                nc.tensor.matmul(
                    ps[:], lhsT=w_t[:],
                    rhs=f8_t[:, :, h * 512:(h + 1) * 512],
                    start=True, stop=True, perf_mode=DR)
                nc.vector.tensor_copy(out=sb[:, h * 512:(h + 1) * 512],
                                      in_=ps[:])
            return sb

        # -------- per-item state carried into the next item (LN epilogue)
        pend = []  # list of (i, stats, h_sb)

        def ln_epilogue():
            if not pend:
                return
            i, stats, h_sb = pend.pop()
            ps_st = psum.tile([1, 8], F32, name="ps_st", tag="work", bufs=4)
            nc.tensor.matmul(ps_st[:], lhsT=ones_col[:], rhs=stats[:],
                             start=True, stop=True)
            st = work.tile([1, 8], F32, name="st", tag="st")
            nc.vector.tensor_copy(out=st[:], in_=ps_st[:])
            # cols: s*4 + t*2 + k (k=0 sum, k=1 sumsq) -> tot over t
            st_r = st[:].rearrange("p (a b) -> p a b", a=2)  # a=s, b=(t,k)
            tot = work.tile([1, 4], F32, name="tot", tag="tot")
            nc.vector.tensor_add(
                out=tot[:].rearrange("p (a b) -> p a b", a=2),
                in0=st_r[:, :, 0:2],
                in1=st_r[:, :, 2:4])
            # tot = [s0_sum, s0_sq, s1_sum, s1_sq] -> moments = tot / NTOT
            mom = work.tile([1, 4], F32, name="mom", tag="mom")
            nc.vector.tensor_scalar(out=mom[:], in0=tot[:],
                                    scalar1=1.0 / NTOT, scalar2=None,
                                    op0=OP.mult)
            # var = E[x^2] - mu^2 ; mr = [rstd0, rstd1, -mu0, -mu1]
            mom_r = mom[:].rearrange("p (a b) -> p a b", a=2)
            mu_ap = mom_r[:, :, 0]
            m2_ap = mom_r[:, :, 1]
            musq = work.tile([1, 2], F32, name="musq", tag="musq")
            nc.vector.tensor_tensor(out=musq[:], in0=mu_ap, in1=mu_ap,
                                    op=OP.mult)
            var = work.tile([1, 2], F32, name="var", tag="var")
            nc.vector.tensor_tensor(out=var[:], in0=m2_ap, in1=musq[:],
                                    op=OP.subtract)
            nc.vector.tensor_scalar(out=var[:], in0=var[:], scalar1=EPS,
                                    scalar2=None, op0=OP.add)
            sd = work.tile([1, 2], F32, name="sd", tag="sd")
            nc.scalar.activation(out=sd[:], in_=var[:], func=AF.Sqrt)
            mr = work.tile([1, 4], F32, name="mr", tag="mr")
            nc.vector.reciprocal(out=mr[:, 0:2], in_=sd[:])
            nc.vector.tensor_scalar(out=mr[:, 2:4], in0=mu_ap,
                                    scalar1=-1.0, scalar2=None, op0=OP.mult)
            # broadcast [1,4] -> [128,4] via K=1 f32 matmul
            ps_mr = psum.tile([128, 4], F32, name="ps_mr", tag="work", bufs=4)
            nc.tensor.matmul(ps_mr[:], lhsT=ones_row[:], rhs=mr[:],
                             start=True, stop=True)
            mrb = work.tile([128, 4], F32, name="mrb", tag="mrb")
            nc.vector.tensor_copy(out=mrb[:], in_=ps_mr[:])
            # A = lnw * rstd ; Bb = lnb + (-mu) * A ; out = h * A + Bb
            for s in range(2):
                Asb = work.tile([128, 2], F32, name="Asb", tag=f"A{s}")
                nc.vector.tensor_scalar(
                    out=Asb[:], in0=lnw[:, s, :], scalar1=mrb[:, s:s + 1],
                    scalar2=None, op0=OP.mult)
                Bsb = work.tile([128, 2], F32, name="Bsb", tag=f"B{s}")
                nc.vector.scalar_tensor_tensor(
                    out=Bsb[:], in0=Asb[:], scalar=mrb[:, 2 + s:3 + s],
                    in1=lnb[:, s, :], op0=OP.mult, op1=OP.add)
                for t in range(2):
                    o_t = work.tile([128, N], F32, name="o_t", tag="o_t",
                                    bufs=4)
                    nc.vector.tensor_scalar(
                        out=o_t[:], in0=h_sb[s][:, t, :],
                        scalar1=Asb[:, t:t + 1], scalar2=Bsb[:, t:t + 1],
                        op0=OP.mult, op1=OP.add)
                    nc.sync.dma_start(out=out_d[s][i, t], in_=o_t[:])

        for i in range(IPC):
            # ---- input DMAs ----
            fb = []
            f8 = []
            for s in range(2):
                t = inp.tile([128, 2, N], BF16, name=f"fb{s}", tag=f"fb{s}")
                nc.sync.dma_start(
                    out=t[:], in_=fb_d[s][i].rearrange("c p n -> p c n"))
                fb.append(t)
                t8 = inp.tile([128, 2, N], FP8, name=f"f8_{s}", tag=f"f8_{s}")
                nc.sync.dma_start(
                    out=t8[:], in_=f8_d[s][i].rearrange("c p n -> p c n"))
                f8.append(t8)

            stats = work.tile([128, 8], F32, name="stats", tag="stats")
            h_sb = []

            for s in range(2):
                # ================= attention for output stream s ==========
                q_sb = conv_qk(wq[1 - s], f8[1 - s], "q_sb")
                k_sb = conv_qk(wk[s], f8[s], "k_sb")

                # vT[m, c] via DoubleRow: stationary = f8 slice pair
                vt_sb = work.tile([128, 8, 256], FP8, name="vt_sb", tag="vt")
                for half in range(4):
                    ps_vt = psum.tile([128, 512], F32, name="ps_vt",
                                      tag="work", bufs=4)
                    for jj in range(2):
                        j = half * 2 + jj
                        nc.tensor.matmul(
                            ps_vt[:, jj * 256:(jj + 1) * 256],
                            lhsT=f8[s][:, :, j * 128:(j + 1) * 128],
                            rhs=wv[s][:],
                            start=True, stop=True, perf_mode=DR)
                    nc.vector.tensor_copy(
                        out=vt_sb[:, half * 2:(half + 1) * 2, :]
                        .rearrange("p a b -> p (a b)"),
                        in_=ps_vt[:])

                # S^T -> exp(fp8) ; PV accumulates DoubleRow chunk-pairs
                pv_ps = [psum.tile([128, N], F32, name=f"pv{t}", tag="pv")
                         for t in range(2)]
                expS = work.tile([128, 8, N], FP8, name="expS", tag="expS")
                for j in range(8):
                    for h in range(2):
                        ps_s = psum.tile([128, 512], F32, name="ps_s",
                                         tag="work", bufs=4)
                        nc.tensor.matmul(
                            ps_s[:],
                            lhsT=k_sb[:, j * 128:(j + 1) * 128],
                            rhs=q_sb[:, h * 512:(h + 1) * 512],
                            start=True, stop=True)
                        nc.scalar.activation(
                            out=expS[:, j, h * 512:(h + 1) * 512],
                            in_=ps_s[:], func=AF.Exp, scale=EXP_SCALE)
                    if j % 2 == 1:
                        jp = j // 2  # chunk pair (2jp, 2jp+1) ready
                        for t in range(2):
                            for h in range(2):
                                nc.tensor.matmul(
                                    pv_ps[t][:, h * 512:(h + 1) * 512],
                                    lhsT=vt_sb[:, 2 * jp:2 * jp + 2,
                                               t * 128:(t + 1) * 128],
                                    rhs=expS[:, 2 * jp:2 * jp + 2,
                                             h * 512:(h + 1) * 512],
                                    start=(jp == 0), stop=(jp == 3),
                                    perf_mode=DR)

                # denominator: ones.T @ E accumulated over chunk pairs,
                # result rows are all equal to den[n]; then reciprocal.
                rden = work.tile([128, N], F32, name="rden", tag="rden")
                for h in range(2):
                    ps_bc = psum.tile([128, 512], F32, name="ps_bc",
                                      tag="work", bufs=4)
                    for jp in range(4):
                        nc.tensor.matmul(
                            ps_bc[:],
                            lhsT=ones8[:],
                            rhs=expS[:, 2 * jp:2 * jp + 2,
                                     h * 512:(h + 1) * 512],
                            start=(jp == 0), stop=(jp == 3), perf_mode=DR)
                    nc.vector.reciprocal_approx_fast(
                        out=rden[:, h * 512:(h + 1) * 512], in_=ps_bc[:])

                # normalize PV -> attn (bf16, feeds fuse matmul)
                attn_sb = work.tile([128, 2, N], BF16, name="attn_sb",
                                    tag="attn")
                for t in range(2):
                    nc.vector.scalar_tensor_tensor(
                        out=attn_sb[:, t, :], in0=pv_ps[t][:],
                        scalar=1.0 / WSCALE, in1=rden[:],
                        op0=OP.mult, op1=OP.mult)

                # ================= fuse + residual + LN stats =============
                # f-half chunks first (no attn dependency), then attn half.
                ps_f = {}
                for t in range(2):
                    for h in range(2):
                        p = psum.tile([128, 512], F32, name="ps_f",
                                      tag="work", bufs=4)
                        ps_f[(t, h)] = p
                        for kc in range(2):
                            nc.tensor.matmul(
                                p[:],
                                lhsT=wf[:, kc, t * 128:(t + 1) * 128],
                                rhs=fb[s][:, kc, h * 512:(h + 1) * 512],
                                start=(kc == 0), stop=False)
                h_t = work.tile([128, 2, N], BF16, name="h_t", tag=f"h{s}",
                                bufs=2)
                g_t = work.tile([128, 2, N], BF16, name="g_t", tag="g_t")
                for t in range(2):
                    for h in range(2):
                        p = ps_f[(t, h)]
                        for kc in range(2, 4):
                            nc.tensor.matmul(
                                p[:],
                                lhsT=wf[:, kc, t * 128:(t + 1) * 128],
                                rhs=attn_sb[:, kc - 2, h * 512:(h + 1) * 512],
                                start=False, stop=(kc == 3))
                        nc.scalar.activation(
                            out=g_t[:, t, h * 512:(h + 1) * 512], in_=p[:],
                            func=AF.Relu, bias=fbias[:, t:t + 1], scale=1.0)
                # residual (bf16) + LN sum accum; sumsq on DVE
                for t in range(2):
                    c0 = s * 4 + t * 2
                    nc.vector.scalar_tensor_tensor(
                        out=h_t[:, t, :], in0=g_t[:, t, :], scalar=1.0,
                        in1=fb[s][:, t, :], op0=OP.mult, op1=OP.add,
                        accum_out=stats[:, c0:c0 + 1])
                    dum = work.tile([128, N], BF16, name="dum", tag="dum")
                    nc.scalar.activation(
                        out=dum[:], in_=h_t[:, t, :], func=AF.Square,
                        accum_out=stats[:, c0 + 1:c0 + 2])
                h_sb.append(h_t)

                if s == 0:
                    # previous item's LN epilogue hides behind this item's
                    # stream-1 attention PE work
                    ln_epilogue()

            pend.append((i, stats, h_sb))

        ln_epilogue()

        psum.release()
        work.release()
        inp.release()
        consts.release()

    nc.compile()
    return nc


_NC_CACHE = None


def _get_nc():
    global _NC_CACHE
    if _NC_CACHE is None:
        _NC_CACHE = _build()
    return _NC_CACHE


def kernel(fs, fi, qs_w, ks_w, vs_w, qi_w, ki_w, vi_w,
           fuse_w, fuse_b, ln_s_w, ln_s_b, ln_i_w, ln_i_b):
    global LAST_RESULT
    fs = np.asarray(fs, np.float32)
    fi = np.asarray(fi, np.float32)

    def prep_f(x):
        # (B, C, H, W) -> per-core [IPC, 2, 128, N]
        return x.reshape(NCORES, IPC, 2, 128, N)

    def prep_w_qk(w):  # (128, 256) -> lhsT layout [128p, 2kc, 128m] * 32
        wt = np.ascontiguousarray(np.asarray(w, np.float32).T) * WSCALE
        return np.ascontiguousarray(
            wt.reshape(2, 128, 128).transpose(1, 0, 2)).astype(
                ml_dtypes.float8_e4m3)

    def prep_w_v(w):  # (256, 256) -> rhs layout [128p, 2kc, 256c] * 32
        wt = np.ascontiguousarray(np.asarray(w, np.float32).T) * WSCALE
        return np.ascontiguousarray(
            wt.reshape(2, 128, 256).transpose(1, 0, 2)).astype(
                ml_dtypes.float8_e4m3)

    fs_sh = prep_f(fs)
    fi_sh = prep_f(fi)
    fs_bf = fs_sh.astype(ml_dtypes.bfloat16)
    fi_bf = fi_sh.astype(ml_dtypes.bfloat16)
    fs_q8 = fs_sh.astype(ml_dtypes.float8_e4m3)
    fi_q8 = fi_sh.astype(ml_dtypes.float8_e4m3)

    wq0 = prep_w_qk(qs_w)
    wq1 = prep_w_qk(qi_w)
    wk0 = prep_w_qk(ks_w)
    wk1 = prep_w_qk(ki_w)
    wv0 = prep_w_v(vs_w)
    wv1 = prep_w_v(vi_w)
    wfuse = np.ascontiguousarray(
        np.asarray(fuse_w, np.float32).T.reshape(4, 128, 256)
        .transpose(1, 0, 2)).astype(ml_dtypes.bfloat16)
    fuseb = np.ascontiguousarray(
        np.asarray(fuse_b, np.float32).reshape(2, 128).T)
    lnw = np.ascontiguousarray(
        np.stack([np.asarray(ln_s_w, np.float32).reshape(256),
                  np.asarray(ln_i_w, np.float32).reshape(256)])
        .reshape(2, 2, 128).transpose(2, 0, 1))
    lnb = np.ascontiguousarray(
        np.stack([np.asarray(ln_s_b, np.float32).reshape(256),
                  np.asarray(ln_i_b, np.float32).reshape(256)])
        .reshape(2, 2, 128).transpose(2, 0, 1))

    in_maps = []
    for c in range(NCORES):
        in_maps.append({
            "fsb": np.ascontiguousarray(fs_bf[c]),
            "fib": np.ascontiguousarray(fi_bf[c]),
            "fs8": np.ascontiguousarray(fs_q8[c]),
            "fi8": np.ascontiguousarray(fi_q8[c]),
            "wq0": wq0, "wq1": wq1, "wk0": wk0, "wk1": wk1,
            "wv0": wv0, "wv1": wv1, "wfuse": wfuse, "fuseb": fuseb,
            "lnw": lnw, "lnb": lnb,
        })

    nc = _get_nc()
    res = run_bass_kernel_spmd(nc, in_maps, core_ids=list(range(NCORES)),
                               **RUN_KWARGS)
    LAST_RESULT = res

    fs_out = np.empty((NCORES, IPC, 2, 128, N), np.float32)
    fi_out = np.empty((NCORES, IPC, 2, 128, N), np.float32)
    for c in range(NCORES):
        fs_out[c] = res.results[c]["out0"]
        fi_out[c] = res.results[c]["out1"]
    fs_out = fs_out.reshape(B, C, 32, 32)
    fi_out = fi_out.reshape(B, C, 32, 32)
    return fs_out, fi_out


# revision 9
# speedup vs baseline: 1.3146x; 1.0900x over previous
"""Trainium2 Bass kernel for nn_CrossAttention2d (B=32, C=256, INNER=128, H=W=32).

Sharding: pure data parallel — batch 32 split as 4 items per core across 8
NeuronCores; all weights replicated. No collectives.

Per item (N = H*W = 1024 tokens, C = 256 channels, D = 128 inner):
  attention for output stream s (s=0 -> fs side, s=1 -> fi side):
      q = wq[1-s] @ f[1-s], k = wk[s] @ f[s]   (fp8 DoubleRow, x32 prescale)
      vT[m, c] = (wv[s] @ f[s]).T   -- computed directly via DoubleRow with
                 f-slices as the stationary operand
      S^T[m, n] = sum_d k[d, m] q[d, n]        (bf16 PE, m-tiles of 128)
      E = exp(S^T / (1024 sqrt(D)))            (ACT, psum -> fp8 sbuf)
      O_un[c, n] = sum_m vT[m, c] E[m, n]      (fp8 DoubleRow, 4 chunk-pairs)
      den[n] via ones.T @ E (fp8 DoubleRow) broadcast to 128 rows
      attn = O_un * (1/32) * (1/den)           (DVE scalar_tensor_tensor)
  fuse: g = relu(Wf @ [f[s]; attn] + b)        (bf16 PE + ACT)
  h = g + f[s] (bf16 residual); LayerNorm over all (C,N) of h; LN stats via
  DVE accum_out + PE partition-reduce; out = h * A + B (DVE tensor_scalar).
  The LN scalar epilogue of item i is emitted inside item i+1 (software
  pipelining) so its serial tiny-op chain hides behind PE work.

Matmul convention: out[M, N] = lhsT.T @ rhs, lhsT = [K<=128, M<=128] (K on
partitions), rhs = [K, N<=512], out in PSUM f32 (one bank per matmul).
DoubleRow: lhsT [Ki, 2, M], rhs [Ki, 2, N] fp8 -> contracts 2*Ki.
PSUM: "pv" tag 2x[128,1024] (4 banks) + "work" tag 4x[128,512] (4 banks).
"""

import numpy as np
import ml_dtypes

import concourse.bacc as bacc
import concourse.bass as bass
import concourse.tile as tile
from concourse import mybir
from concourse.bass_utils import run_bass_kernel_spmd

F32 = mybir.dt.float32
BF16 = mybir.dt.bfloat16
FP8 = mybir.dt.float8e4
DR = mybir.MatmulPerfMode.DoubleRow
AF = mybir.ActivationFunctionType
OP = mybir.AluOpType

B, C, D, N = 32, 256, 128, 1024
NCORES = 8
IPC = B // NCORES  # items per core = 4
WSCALE = 32.0  # fp8 weight prescale (w*32 keeps N(0,0.02) in e4m3 range)
EXP_SCALE = (1.0 / float(np.sqrt(D))) / (WSCALE * WSCALE)
EPS = 1e-5
NTOT = float(C * N)  # layernorm element count per item/stream

# test.py can set {"trace": True}; harness path leaves this empty.
RUN_KWARGS = {}
LAST_RESULT = None


def _build():
    nc = bacc.Bacc("TRN2", target_bir_lowering=False, debug=False,
                   num_devices=NCORES)

    # ---- DRAM I/O (per-core shapes) ----
    fb_d = [nc.dram_tensor(n_, [IPC, 2, 128, N], BF16, kind="ExternalInput")
            for n_ in ("fsb", "fib")]
    f8_d = [nc.dram_tensor(n_, [IPC, 2, 128, N], FP8, kind="ExternalInput")
            for n_ in ("fs8", "fi8")]
    wq_d = [nc.dram_tensor(n_, [128, 2, 128], FP8, kind="ExternalInput")
            for n_ in ("wq0", "wq1")]
    wk_d = [nc.dram_tensor(n_, [128, 2, 128], FP8, kind="ExternalInput")
            for n_ in ("wk0", "wk1")]
    wv_d = [nc.dram_tensor(n_, [128, 2, 256], FP8, kind="ExternalInput")
            for n_ in ("wv0", "wv1")]
    wf_d = nc.dram_tensor("wfuse", [128, 4, 256], BF16, kind="ExternalInput")
    fb_bias_d = nc.dram_tensor("fuseb", [128, 2], F32, kind="ExternalInput")
    lnw_d = nc.dram_tensor("lnw", [128, 2, 2], F32, kind="ExternalInput")
    lnb_d = nc.dram_tensor("lnb", [128, 2, 2], F32, kind="ExternalInput")
    out_d = [nc.dram_tensor(n_, [IPC, 2, 128, N], F32, kind="ExternalOutput")
             for n_ in ("out0", "out1")]

    with tile.TileContext(nc) as tc:
        consts = tc.alloc_tile_pool(name="consts", bufs=1)
        inp = tc.alloc_tile_pool(name="inp", bufs=2)
        work = tc.alloc_tile_pool(name="work", bufs=2)
        psum = tc.alloc_tile_pool(name="psum", bufs=2, space="PSUM")

        # ---- load constants ----
        wq = [consts.tile([128, 2, 128], FP8, name=f"wq{s}", tag=f"wq{s}")
              for s in range(2)]
        wk = [consts.tile([128, 2, 128], FP8, name=f"wk{s}", tag=f"wk{s}")
              for s in range(2)]
        wv = [consts.tile([128, 2, 256], FP8, name=f"wv{s}", tag=f"wv{s}")
              for s in range(2)]
        wf = consts.tile([128, 4, 256], BF16, name="wf", tag="wf")
        fbias = consts.tile([128, 2], F32, name="fbias", tag="fbias")
        lnw = consts.tile([128, 2, 2], F32, name="lnw", tag="lnw")
        lnb = consts.tile([128, 2, 2], F32, name="lnb", tag="lnb")
        ones8 = consts.tile([128, 2, 128], FP8, name="ones8", tag="ones8")
        ones_col = consts.tile([128, 1], F32, name="ones_col", tag="ones_col")
        ones_row = consts.tile([1, 128], F32, name="ones_row", tag="ones_row")
        for s in range(2):
            nc.sync.dma_start(out=wq[s][:], in_=wq_d[s][:])
            nc.sync.dma_start(out=wk[s][:], in_=wk_d[s][:])
            nc.sync.dma_start(out=wv[s][:], in_=wv_d[s][:])
        nc.sync.dma_start(out=wf[:], in_=wf_d[:])
        nc.sync.dma_start(out=fbias[:], in_=fb_bias_d[:])
        nc.sync.dma_start(out=lnw[:], in_=lnw_d[:])
        nc.sync.dma_start(out=lnb[:], in_=lnb_d[:])
        nc.vector.memset(ones8[:], 1.0)
        nc.vector.memset(ones_col[:], 1.0)
        nc.vector.memset(ones_row[:], 1.0)

        def conv_qk(w_t, f8_t, name):
            """[128, N] = (32w).T @ f via fp8 DoubleRow; bf16 sbuf out."""
            sb = work.tile([128, N], BF16, name=name, tag=name)
            for h in range(2):
                ps = psum.tile([128, 512], F32, name=f"ps_{name}", tag="work",
                               bufs=4)
                nc.tensor.matmul(
                    ps[:], lhsT=w_t[:],
                    rhs=f8_t[:, :, h * 512:(h + 1) * 512],
                    start=True, stop=True, perf_mode=DR)
                nc.vector.tensor_copy(out=sb[:, h * 512:(h + 1) * 512],
                                      in_=ps[:])
            return sb

        # -------- per-item state carried into the next item (LN epilogue)
        pend = []       # [(i, stats, h_sb)] awaiting the stats->A/B chain
        pend_apply = []  # [(i, h_sb, A, B)] awaiting LN apply + store

        def ln_epi_chain():
            """Stats -> mean/var -> rstd (DVE Newton) -> A/B. No ACT, and
            the only PE op (stats colsum) has its inputs long ready, so the
            PE stream never blocks on this chain."""
            if not pend:
                return
            i, stats, h_sb = pend.pop()
            ps_st = psum.tile([1, 8], F32, name="ps_st", tag="work", bufs=4)
            nc.tensor.matmul(ps_st[:], lhsT=ones_col[:], rhs=stats[:],
                             start=True, stop=True)
            st = work.tile([1, 8], F32, name="st", tag="st")
            nc.vector.tensor_copy(out=st[:], in_=ps_st[:])
            # cols: s*4 + t*2 + k (k=0 sum, k=1 sumsq) -> tot over t
            st_r = st[:].rearrange("p (a b) -> p a b", a=2)  # a=s, b=(t,k)
            tot = work.tile([1, 4], F32, name="tot", tag="tot")
            nc.vector.tensor_add(
                out=tot[:].rearrange("p (a b) -> p a b", a=2),
                in0=st_r[:, :, 0:2],
                in1=st_r[:, :, 2:4])
            # tot = [s0_sum, s0_sq, s1_sum, s1_sq] -> moments = tot / NTOT
            mom = work.tile([1, 4], F32, name="mom", tag="mom")
            nc.vector.tensor_scalar(out=mom[:], in0=tot[:],
                                    scalar1=1.0 / NTOT, scalar2=None,
                                    op0=OP.mult)
            mom_r = mom[:].rearrange("p (a b) -> p a b", a=2)
            mu_ap = mom_r[:, :, 0]
            m2_ap = mom_r[:, :, 1]
            musq = work.tile([1, 2], F32, name="musq", tag="musq")
            nc.vector.tensor_tensor(out=musq[:], in0=mu_ap, in1=mu_ap,
                                    op=OP.mult)
            var = work.tile([1, 2], F32, name="var", tag="var")
            nc.vector.scalar_tensor_tensor(
                out=var[:], in0=musq[:], scalar=-1.0, in1=m2_ap,
                op0=OP.mult, op1=OP.add)
            nc.vector.tensor_scalar(out=var[:], in0=var[:], scalar1=EPS,
                                    scalar2=None, op0=OP.add)
            # rstd = var^-0.5 via Newton (all-DVE; var is ~[0.3, 3] so the
            # constant seed converges: err 30% -> 6% -> 0.3% -> 1e-5)
            mr = work.tile([1, 4], F32, name="mr", tag="mr")
            y = mr[:, 0:2]
            nc.vector.memset(y, 0.92)
            t1 = work.tile([1, 2], F32, name="t1", tag="t1")
            for _ in range(3):
                nc.vector.tensor_tensor(out=t1[:], in0=y, in1=y, op=OP.mult)
                nc.vector.tensor_tensor(out=t1[:], in0=var[:], in1=t1[:],
                                        op=OP.mult)
                nc.vector.tensor_scalar(out=t1[:], in0=t1[:], scalar1=-0.5,
                                        scalar2=1.5, op0=OP.mult, op1=OP.add)
                nc.vector.tensor_tensor(out=y, in0=y, in1=t1[:], op=OP.mult)
            nc.vector.tensor_scalar(out=mr[:, 2:4], in0=mu_ap,
                                    scalar1=-1.0, scalar2=None, op0=OP.mult)
            # broadcast [1,4] -> [128,4] on GpSimd (PE stays out of it)
            mrb = work.tile([128, 4], F32, name="mrb", tag="mrb")
            nc.gpsimd.partition_broadcast(out_ap=mrb[:], in_ap=mr[:])
            # A = lnw * rstd ; Bb = lnb + (-mu) * A
            AB = []
            for s in range(2):
                Asb = work.tile([128, 2], F32, name="Asb", tag=f"A{s}")
                nc.vector.tensor_scalar(
                    out=Asb[:], in0=lnw[:, s, :], scalar1=mrb[:, s:s + 1],
                    scalar2=None, op0=OP.mult)
                Bsb = work.tile([128, 2], F32, name="Bsb", tag=f"B{s}")
                nc.vector.scalar_tensor_tensor(
                    out=Bsb[:], in0=Asb[:], scalar=mrb[:, 2 + s:3 + s],
                    in1=lnb[:, s, :], op0=OP.mult, op1=OP.add)
                AB.append((Asb, Bsb))
            pend_apply.append((i, h_sb, AB))

        def ln_epi_apply():
            if not pend_apply:
                return
            i, h_sb, AB = pend_apply.pop()
            for s in range(2):
                Asb, Bsb = AB[s]
                for t in range(2):
                    o_t = work.tile([128, N], F32, name="o_t", tag="o_t",
                                    bufs=4)
                    nc.vector.tensor_scalar(
                        out=o_t[:], in0=h_sb[s][:, t, :],
                        scalar1=Asb[:, t:t + 1], scalar2=Bsb[:, t:t + 1],
                        op0=OP.mult, op1=OP.add)
                    nc.sync.dma_start(out=out_d[s][i, t], in_=o_t[:])

        for i in range(IPC):
            # ---- input DMAs ----
            fb = []
            f8 = []
            for s in range(2):
                t = inp.tile([128, 2, N], BF16, name=f"fb{s}", tag=f"fb{s}")
                nc.sync.dma_start(
                    out=t[:], in_=fb_d[s][i].rearrange("c p n -> p c n"))
                fb.append(t)
                t8 = inp.tile([128, 2, N], FP8, name=f"f8_{s}", tag=f"f8_{s}")
                nc.sync.dma_start(
                    out=t8[:], in_=f8_d[s][i].rearrange("c p n -> p c n"))
                f8.append(t8)

            stats = work.tile([128, 8], F32, name="stats", tag="stats")
            h_sb = []

            for s in range(2):
                # ================= attention for output stream s ==========
                q_sb = conv_qk(wq[1 - s], f8[1 - s], "q_sb")
                k_sb = conv_qk(wk[s], f8[s], "k_sb")

                # vT[m, c] via DoubleRow: stationary = f8 slice pair
                vt_sb = work.tile([128, 8, 256], FP8, name="vt_sb", tag="vt")
                for half in range(4):
                    ps_vt = psum.tile([128, 512], F32, name="ps_vt",
                                      tag="work", bufs=4)
                    for jj in range(2):
                        j = half * 2 + jj
                        nc.tensor.matmul(
                            ps_vt[:, jj * 256:(jj + 1) * 256],
                            lhsT=f8[s][:, :, j * 128:(j + 1) * 128],
                            rhs=wv[s][:],
                            start=True, stop=True, perf_mode=DR)
                    nc.vector.tensor_copy(
                        out=vt_sb[:, half * 2:(half + 1) * 2, :]
                        .rearrange("p a b -> p (a b)"),
                        in_=ps_vt[:])

                if s == 1:
                    # previous item's LN apply lands between this attention's
                    # casts and its tail ops on the DVE queue
                    ln_epi_apply()

                # S^T -> exp(fp8) ; PV accumulates DoubleRow chunk-pairs
                pv_ps = [psum.tile([128, N], F32, name=f"pv{t}", tag="pv")
                         for t in range(2)]
                expS = work.tile([128, 8, N], FP8, name="expS", tag="expS")
                for j in range(8):
                    for h in range(2):
                        ps_s = psum.tile([128, 512], F32, name="ps_s",
                                         tag="work", bufs=4)
                        nc.tensor.matmul(
                            ps_s[:],
                            lhsT=k_sb[:, j * 128:(j + 1) * 128],
                            rhs=q_sb[:, h * 512:(h + 1) * 512],
                            start=True, stop=True)
                        nc.scalar.activation(
                            out=expS[:, j, h * 512:(h + 1) * 512],
                            in_=ps_s[:], func=AF.Exp, scale=EXP_SCALE)
                    if j % 2 == 1:
                        jp = j // 2  # chunk pair (2jp, 2jp+1) ready
                        for t in range(2):
                            for h in range(2):
                                nc.tensor.matmul(
                                    pv_ps[t][:, h * 512:(h + 1) * 512],
                                    lhsT=vt_sb[:, 2 * jp:2 * jp + 2,
                                               t * 128:(t + 1) * 128],
                                    rhs=expS[:, 2 * jp:2 * jp + 2,
                                             h * 512:(h + 1) * 512],
                                    start=(jp == 0), stop=(jp == 3),
                                    perf_mode=DR)

                # denominator: ones.T @ E accumulated over chunk pairs,
                # result rows are all equal to den[n]; then reciprocal.
                rden = work.tile([128, N], F32, name="rden", tag="rden")
                for h in range(2):
                    ps_bc = psum.tile([128, 512], F32, name="ps_bc",
                                      tag="work", bufs=4)
                    for jp in range(4):
                        nc.tensor.matmul(
                            ps_bc[:],
                            lhsT=ones8[:],
                            rhs=expS[:, 2 * jp:2 * jp + 2,
                                     h * 512:(h + 1) * 512],
                            start=(jp == 0), stop=(jp == 3), perf_mode=DR)
                    nc.vector.reciprocal_approx_fast(
                        out=rden[:, h * 512:(h + 1) * 512], in_=ps_bc[:])

                # normalize PV -> attn (bf16, feeds fuse matmul)
                attn_sb = work.tile([128, 2, N], BF16, name="attn_sb",
                                    tag="attn")
                for t in range(2):
                    nc.vector.scalar_tensor_tensor(
                        out=attn_sb[:, t, :], in0=pv_ps[t][:],
                        scalar=1.0 / WSCALE, in1=rden[:],
                        op0=OP.mult, op1=OP.mult)

                # ================= fuse + residual + LN stats =============
                # f-half chunks first (no attn dependency), then attn half.
                ps_f = {}
                for t in range(2):
                    for h in range(2):
                        p = psum.tile([128, 512], F32, name="ps_f",
                                      tag="work", bufs=4)
                        ps_f[(t, h)] = p
                        for kc in range(2):
                            nc.tensor.matmul(
                                p[:],
                                lhsT=wf[:, kc, t * 128:(t + 1) * 128],
                                rhs=fb[s][:, kc, h * 512:(h + 1) * 512],
                                start=(kc == 0), stop=False)
                h_t = work.tile([128, 2, N], BF16, name="h_t", tag=f"h{s}",
                                bufs=2)
                g_t = work.tile([128, 2, N], BF16, name="g_t", tag="g_t")
                for t in range(2):
                    for h in range(2):
                        p = ps_f[(t, h)]
                        for kc in range(2, 4):
                            nc.tensor.matmul(
                                p[:],
                                lhsT=wf[:, kc, t * 128:(t + 1) * 128],
                                rhs=attn_sb[:, kc - 2, h * 512:(h + 1) * 512],
                                start=False, stop=(kc == 3))
                        nc.scalar.activation(
                            out=g_t[:, t, h * 512:(h + 1) * 512], in_=p[:],
                            func=AF.Relu, bias=fbias[:, t:t + 1], scale=1.0)
                # residual (bf16) + LN sum accum; sumsq on DVE
                for t in range(2):
                    c0 = s * 4 + t * 2
                    nc.vector.scalar_tensor_tensor(
                        out=h_t[:, t, :], in0=g_t[:, t, :], scalar=1.0,
                        in1=fb[s][:, t, :], op0=OP.mult, op1=OP.add,
                        accum_out=stats[:, c0:c0 + 1])
                    dum = work.tile([128, N], BF16, name="dum", tag="dum")
                    nc.scalar.activation(
                        out=dum[:], in_=h_t[:, t, :], func=AF.Square,
                        accum_out=stats[:, c0 + 1:c0 + 2])
                h_sb.append(h_t)

                if s == 0:
                    # previous item's LN chain hides behind this item's work
                    ln_epi_chain()

            pend.append((i, stats, h_sb))

        ln_epi_chain()
        ln_epi_apply()

        psum.release()
        work.release()
        inp.release()
        consts.release()

    nc.compile()
    return nc


_NC_CACHE = None


def _get_nc():
    global _NC_CACHE
    if _NC_CACHE is None:
        _NC_CACHE = _build()
    return _NC_CACHE


def kernel(fs, fi, qs_w, ks_w, vs_w, qi_w, ki_w, vi_w,
           fuse_w, fuse_b, ln_s_w, ln_s_b, ln_i_w, ln_i_b):
    global LAST_RESULT
    fs = np.asarray(fs, np.float32)
    fi = np.asarray(fi, np.float32)

    def prep_f(x):
        # (B, C, H, W) -> per-core [IPC, 2, 128, N]
        return x.reshape(NCORES, IPC, 2, 128, N)

    def prep_w_qk(w):  # (128, 256) -> lhsT layout [128p, 2kc, 128m] * 32
        wt = np.ascontiguousarray(np.asarray(w, np.float32).T) * WSCALE
        return np.ascontiguousarray(
            wt.reshape(2, 128, 128).transpose(1, 0, 2)).astype(
                ml_dtypes.float8_e4m3)

    def prep_w_v(w):  # (256, 256) -> rhs layout [128p, 2kc, 256c] * 32
        wt = np.ascontiguousarray(np.asarray(w, np.float32).T) * WSCALE
        return np.ascontiguousarray(
            wt.reshape(2, 128, 256).transpose(1, 0, 2)).astype(
                ml_dtypes.float8_e4m3)

    fs_sh = prep_f(fs)
    fi_sh = prep_f(fi)
    fs_bf = fs_sh.astype(ml_dtypes.bfloat16)
    fi_bf = fi_sh.astype(ml_dtypes.bfloat16)
    fs_q8 = fs_sh.astype(ml_dtypes.float8_e4m3)
    fi_q8 = fi_sh.astype(ml_dtypes.float8_e4m3)

    wq0 = prep_w_qk(qs_w)
    wq1 = prep_w_qk(qi_w)
    wk0 = prep_w_qk(ks_w)
    wk1 = prep_w_qk(ki_w)
    wv0 = prep_w_v(vs_w)
    wv1 = prep_w_v(vi_w)
    wfuse = np.ascontiguousarray(
        np.asarray(fuse_w, np.float32).T.reshape(4, 128, 256)
        .transpose(1, 0, 2)).astype(ml_dtypes.bfloat16)
    fuseb = np.ascontiguousarray(
        np.asarray(fuse_b, np.float32).reshape(2, 128).T)
    lnw = np.ascontiguousarray(
        np.stack([np.asarray(ln_s_w, np.float32).reshape(256),
                  np.asarray(ln_i_w, np.float32).reshape(256)])
        .reshape(2, 2, 128).transpose(2, 0, 1))
    lnb = np.ascontiguousarray(
        np.stack([np.asarray(ln_s_b, np.float32).reshape(256),
                  np.asarray(ln_i_b, np.float32).reshape(256)])
        .reshape(2, 2, 128).transpose(2, 0, 1))

    in_maps = []
    for c in range(NCORES):
        in_maps.append({
            "fsb": np.ascontiguousarray(fs_bf[c]),
            "fib": np.ascontiguousarray(fi_bf[c]),
            "fs8": np.ascontiguousarray(fs_q8[c]),
            "fi8": np.ascontiguousarray(fi_q8[c]),
            "wq0": wq0, "wq1": wq1, "wk0": wk0, "wk1": wk1,
            "wv0": wv0, "wv1": wv1, "wfuse": wfuse, "fuseb": fuseb,
            "lnw": lnw, "lnb": lnb,
        })

    nc = _get_nc()
    res = run_bass_kernel_spmd(nc, in_maps, core_ids=list(range(NCORES)),
                               **RUN_KWARGS)
    LAST_RESULT = res

    fs_out = np.empty((NCORES, IPC, 2, 128, N), np.float32)
    fi_out = np.empty((NCORES, IPC, 2, 128, N), np.float32)
    for c in range(NCORES):
        fs_out[c] = res.results[c]["out0"]
        fi_out[c] = res.results[c]["out1"]
    fs_out = fs_out.reshape(B, C, 32, 32)
    fi_out = fi_out.reshape(B, C, 32, 32)
    return fs_out, fi_out


# revision 10
# speedup vs baseline: 1.3654x; 1.0386x over previous
"""Trainium2 Bass kernel for nn_CrossAttention2d (B=32, C=256, INNER=128, H=W=32).

Sharding: pure data parallel — batch 32 split as 4 items per core across 8
NeuronCores; all weights replicated. No collectives.

Per item (N = H*W = 1024 tokens, C = 256 channels, D = 128 inner):
  attention for output stream s (s=0 -> fs side, s=1 -> fi side):
      q = wq[1-s] @ f[1-s], k = wk[s] @ f[s]   (fp8 DoubleRow, x32 prescale)
      vT[m, c] = (wv[s] @ f[s]).T   -- computed directly via DoubleRow with
                 f-slices as the stationary operand
      S^T[m, n] = sum_d k[d, m] q[d, n]        (bf16 PE, m-tiles of 128)
      E = exp(S^T / (1024 sqrt(D)))            (ACT, psum -> fp8 sbuf)
      O_un[c, n] = sum_m vT[m, c] E[m, n]      (fp8 DoubleRow, 4 chunk-pairs)
      den[n] via ones.T @ E (fp8 DoubleRow) broadcast to 128 rows
      attn = O_un * (1/32) * (1/den)           (DVE scalar_tensor_tensor)
  fuse: g = relu(Wf @ [f[s]; attn] + b)        (bf16 PE + ACT)
  h = g + f[s] (bf16 residual); LayerNorm over all (C,N) of h; LN stats via
  DVE accum_out + PE partition-reduce; out = h * A + B (DVE tensor_scalar).
  The LN scalar epilogue of item i is emitted inside item i+1 (software
  pipelining) so its serial tiny-op chain hides behind PE work.

Matmul convention: out[M, N] = lhsT.T @ rhs, lhsT = [K<=128, M<=128] (K on
partitions), rhs = [K, N<=512], out in PSUM f32 (one bank per matmul).
DoubleRow: lhsT [Ki, 2, M], rhs [Ki, 2, N] fp8 -> contracts 2*Ki.
PSUM: "pv" tag 2x[128,1024] (4 banks) + "work" tag 4x[128,512] (4 banks).
"""

import numpy as np
import ml_dtypes

import concourse.bacc as bacc
import concourse.bass as bass
import concourse.tile as tile
from concourse import mybir
from concourse.bass_utils import run_bass_kernel_spmd

F32 = mybir.dt.float32
BF16 = mybir.dt.bfloat16
FP8 = mybir.dt.float8e4
DR = mybir.MatmulPerfMode.DoubleRow
AF = mybir.ActivationFunctionType
OP = mybir.AluOpType

B, C, D, N = 32, 256, 128, 1024
NCORES = 8
IPC = B // NCORES  # items per core = 4
WSCALE = 32.0  # fp8 weight prescale (w*32 keeps N(0,0.02) in e4m3 range)
EXP_SCALE = (1.0 / float(np.sqrt(D))) / (WSCALE * WSCALE)
EPS = 1e-5
NTOT = float(C * N)  # layernorm element count per item/stream

# test.py can set {"trace": True}; harness path leaves this empty.
RUN_KWARGS = {}
LAST_RESULT = None


def _build():
    nc = bacc.Bacc("TRN2", target_bir_lowering=False, debug=False,
                   num_devices=NCORES)

    # ---- DRAM I/O (per-core shapes) ----
    fb_d = [nc.dram_tensor(n_, [IPC, 2, 128, N], BF16, kind="ExternalInput")
            for n_ in ("fsb", "fib")]
    f8_d = [nc.dram_tensor(n_, [IPC, 2, 128, N], FP8, kind="ExternalInput")
            for n_ in ("fs8", "fi8")]
    wq_d = [nc.dram_tensor(n_, [128, 2, 128], FP8, kind="ExternalInput")
            for n_ in ("wq0", "wq1")]
    wk_d = [nc.dram_tensor(n_, [128, 2, 128], FP8, kind="ExternalInput")
            for n_ in ("wk0", "wk1")]
    wv_d = [nc.dram_tensor(n_, [128, 2, 256], FP8, kind="ExternalInput")
            for n_ in ("wv0", "wv1")]
    wf_d = nc.dram_tensor("wfuse", [128, 4, 256], BF16, kind="ExternalInput")
    fb_bias_d = nc.dram_tensor("fuseb", [128, 2], F32, kind="ExternalInput")
    lnw_d = nc.dram_tensor("lnw", [128, 2, 2], F32, kind="ExternalInput")
    lnb_d = nc.dram_tensor("lnb", [128, 2, 2], F32, kind="ExternalInput")
    out_d = [nc.dram_tensor(n_, [IPC, 2, 128, N], F32, kind="ExternalOutput")
             for n_ in ("out0", "out1")]

    with tile.TileContext(nc) as tc:
        consts = tc.alloc_tile_pool(name="consts", bufs=1)
        inp = tc.alloc_tile_pool(name="inp", bufs=2)
        work = tc.alloc_tile_pool(name="work", bufs=2)
        psum = tc.alloc_tile_pool(name="psum", bufs=2, space="PSUM")

        # ---- load constants ----
        wq = [consts.tile([128, 2, 128], FP8, name=f"wq{s}", tag=f"wq{s}")
              for s in range(2)]
        wk = [consts.tile([128, 2, 128], FP8, name=f"wk{s}", tag=f"wk{s}")
              for s in range(2)]
        wv = [consts.tile([128, 2, 256], FP8, name=f"wv{s}", tag=f"wv{s}")
              for s in range(2)]
        wf = consts.tile([128, 4, 256], BF16, name="wf", tag="wf")
        fbias = consts.tile([128, 2], F32, name="fbias", tag="fbias")
        lnw = consts.tile([128, 2, 2], F32, name="lnw", tag="lnw")
        lnb = consts.tile([128, 2, 2], F32, name="lnb", tag="lnb")
        ones8 = consts.tile([128, 2, 128], FP8, name="ones8", tag="ones8")
        ones_col = consts.tile([128, 1], F32, name="ones_col", tag="ones_col")
        ones_row = consts.tile([1, 128], F32, name="ones_row", tag="ones_row")
        for s in range(2):
            nc.sync.dma_start(out=wq[s][:], in_=wq_d[s][:])
            nc.sync.dma_start(out=wk[s][:], in_=wk_d[s][:])
            nc.sync.dma_start(out=wv[s][:], in_=wv_d[s][:])
        nc.sync.dma_start(out=wf[:], in_=wf_d[:])
        nc.sync.dma_start(out=fbias[:], in_=fb_bias_d[:])
        nc.sync.dma_start(out=lnw[:], in_=lnw_d[:])
        nc.sync.dma_start(out=lnb[:], in_=lnb_d[:])
        nc.vector.memset(ones8[:], 1.0)
        nc.vector.memset(ones_col[:], 1.0)
        nc.vector.memset(ones_row[:], 1.0)

        def conv_qk(w_t, f8_t, name):
            """[128, N] = (32w).T @ f via fp8 DoubleRow; bf16 sbuf out."""
            sb = work.tile([128, N], BF16, name=name, tag=name)
            for h in range(2):
                ps = psum.tile([128, 512], F32, name=f"ps_{name}", tag="work",
                               bufs=4)
                nc.tensor.matmul(
                    ps[:], lhsT=w_t[:],
                    rhs=f8_t[:, :, h * 512:(h + 1) * 512],
                    start=True, stop=True, perf_mode=DR)
                nc.vector.tensor_copy(out=sb[:, h * 512:(h + 1) * 512],
                                      in_=ps[:])
            return sb

        # -------- per-item state carried into the next item (LN epilogue)
        deferred_stats = []  # closures emitting residual-stt + square ops
        pend = []       # [(i, stats, h_sb)] awaiting the stats->A/B chain
        pend_apply = []  # [(i, h_sb, A, B)] awaiting LN apply + store

        def ln_epi_chain():
            """Stats -> mean/var -> rstd (DVE Newton) -> A/B. No ACT, and
            the only PE op (stats colsum) has its inputs long ready, so the
            PE stream never blocks on this chain."""
            if not pend:
                return
            i, stats, h_sb = pend.pop()
            ps_st = psum.tile([1, 8], F32, name="ps_st", tag="work", bufs=4)
            nc.tensor.matmul(ps_st[:], lhsT=ones_col[:], rhs=stats[:],
                             start=True, stop=True)
            st = work.tile([1, 8], F32, name="st", tag="st")
            nc.vector.tensor_copy(out=st[:], in_=ps_st[:])
            # cols: s*4 + t*2 + k (k=0 sum, k=1 sumsq) -> tot over t
            st_r = st[:].rearrange("p (a b) -> p a b", a=2)  # a=s, b=(t,k)
            tot = work.tile([1, 4], F32, name="tot", tag="tot")
            nc.vector.tensor_add(
                out=tot[:].rearrange("p (a b) -> p a b", a=2),
                in0=st_r[:, :, 0:2],
                in1=st_r[:, :, 2:4])
            # tot = [s0_sum, s0_sq, s1_sum, s1_sq] -> moments = tot / NTOT
            mom = work.tile([1, 4], F32, name="mom", tag="mom")
            nc.vector.tensor_scalar(out=mom[:], in0=tot[:],
                                    scalar1=1.0 / NTOT, scalar2=None,
                                    op0=OP.mult)
            mom_r = mom[:].rearrange("p (a b) -> p a b", a=2)
            mu_ap = mom_r[:, :, 0]
            m2_ap = mom_r[:, :, 1]
            musq = work.tile([1, 2], F32, name="musq", tag="musq")
            nc.vector.tensor_tensor(out=musq[:], in0=mu_ap, in1=mu_ap,
                                    op=OP.mult)
            var = work.tile([1, 2], F32, name="var", tag="var")
            nc.vector.scalar_tensor_tensor(
                out=var[:], in0=musq[:], scalar=-1.0, in1=m2_ap,
                op0=OP.mult, op1=OP.add)
            nc.vector.tensor_scalar(out=var[:], in0=var[:], scalar1=EPS,
                                    scalar2=None, op0=OP.add)
            # rstd = var^-0.5 via Newton (all-DVE; var is ~[0.3, 3] so the
            # constant seed converges: err 30% -> 6% -> 0.3% -> 1e-5)
            mr = work.tile([1, 4], F32, name="mr", tag="mr")
            y = mr[:, 0:2]
            nc.vector.memset(y, 0.92)
            t1 = work.tile([1, 2], F32, name="t1", tag="t1")
            for _ in range(3):
                nc.vector.tensor_tensor(out=t1[:], in0=y, in1=y, op=OP.mult)
                nc.vector.tensor_tensor(out=t1[:], in0=var[:], in1=t1[:],
                                        op=OP.mult)
                nc.vector.tensor_scalar(out=t1[:], in0=t1[:], scalar1=-0.5,
                                        scalar2=1.5, op0=OP.mult, op1=OP.add)
                nc.vector.tensor_tensor(out=y, in0=y, in1=t1[:], op=OP.mult)
            nc.vector.tensor_scalar(out=mr[:, 2:4], in0=mu_ap,
                                    scalar1=-1.0, scalar2=None, op0=OP.mult)
            # broadcast [1,4] -> [128,4] on GpSimd (PE stays out of it)
            mrb = work.tile([128, 4], F32, name="mrb", tag="mrb")
            nc.gpsimd.partition_broadcast(out_ap=mrb[:], in_ap=mr[:])
            # A = lnw * rstd ; Bb = lnb + (-mu) * A
            AB = []
            for s in range(2):
                Asb = work.tile([128, 2], F32, name="Asb", tag=f"A{s}")
                nc.vector.tensor_scalar(
                    out=Asb[:], in0=lnw[:, s, :], scalar1=mrb[:, s:s + 1],
                    scalar2=None, op0=OP.mult)
                Bsb = work.tile([128, 2], F32, name="Bsb", tag=f"B{s}")
                nc.vector.scalar_tensor_tensor(
                    out=Bsb[:], in0=Asb[:], scalar=mrb[:, 2 + s:3 + s],
                    in1=lnb[:, s, :], op0=OP.mult, op1=OP.add)
                AB.append((Asb, Bsb))
            pend_apply.append((i, h_sb, AB))

        def ln_epi_apply():
            if not pend_apply:
                return
            i, h_sb, AB = pend_apply.pop()
            for s in range(2):
                Asb, Bsb = AB[s]
                for t in range(2):
                    o_t = work.tile([128, N], F32, name="o_t", tag="o_t",
                                    bufs=4)
                    nc.vector.tensor_scalar(
                        out=o_t[:], in0=h_sb[s][:, t, :],
                        scalar1=Asb[:, t:t + 1], scalar2=Bsb[:, t:t + 1],
                        op0=OP.mult, op1=OP.add)
                    nc.sync.dma_start(out=out_d[s][i, t], in_=o_t[:])

        for i in range(IPC):
            # ---- input DMAs ----
            fb = []
            f8 = []
            for s in range(2):
                t = inp.tile([128, 2, N], BF16, name=f"fb{s}", tag=f"fb{s}")
                nc.sync.dma_start(
                    out=t[:], in_=fb_d[s][i].rearrange("c p n -> p c n"))
                fb.append(t)
                t8 = inp.tile([128, 2, N], FP8, name=f"f8_{s}", tag=f"f8_{s}")
                nc.sync.dma_start(
                    out=t8[:], in_=f8_d[s][i].rearrange("c p n -> p c n"))
                f8.append(t8)

            stats = work.tile([128, 8], F32, name="stats", tag="stats")
            h_sb = []

            for s in range(2):
                # ================= attention for output stream s ==========
                q_sb = conv_qk(wq[1 - s], f8[1 - s], "q_sb")
                k_sb = conv_qk(wk[s], f8[s], "k_sb")

                # flush previous section's residual/square stat ops here:
                # their DVE work lands after this section's q/k casts
                while deferred_stats:
                    deferred_stats.pop(0)()

                # vT[m, c] via DoubleRow: stationary = f8 slice pair
                vt_sb = work.tile([128, 8, 256], FP8, name="vt_sb", tag="vt")
                for half in range(4):
                    ps_vt = psum.tile([128, 512], F32, name="ps_vt",
                                      tag="work", bufs=4)
                    for jj in range(2):
                        j = half * 2 + jj
                        nc.tensor.matmul(
                            ps_vt[:, jj * 256:(jj + 1) * 256],
                            lhsT=f8[s][:, :, j * 128:(j + 1) * 128],
                            rhs=wv[s][:],
                            start=True, stop=True, perf_mode=DR)
                    nc.vector.tensor_copy(
                        out=vt_sb[:, half * 2:(half + 1) * 2, :]
                        .rearrange("p a b -> p (a b)"),
                        in_=ps_vt[:])

                if s == 1:
                    # previous item's LN apply lands between this attention's
                    # casts and its tail ops on the DVE queue
                    ln_epi_apply()

                # S^T -> exp(fp8) ; PV accumulates DoubleRow chunk-pairs
                pv_ps = [psum.tile([128, N], F32, name=f"pv{t}", tag="pv")
                         for t in range(2)]
                expS = work.tile([128, 8, N], FP8, name="expS", tag="expS")
                for j in range(8):
                    for h in range(2):
                        ps_s = psum.tile([128, 512], F32, name="ps_s",
                                         tag="work", bufs=4)
                        nc.tensor.matmul(
                            ps_s[:],
                            lhsT=k_sb[:, j * 128:(j + 1) * 128],
                            rhs=q_sb[:, h * 512:(h + 1) * 512],
                            start=True, stop=True)
                        nc.scalar.activation(
                            out=expS[:, j, h * 512:(h + 1) * 512],
                            in_=ps_s[:], func=AF.Exp, scale=EXP_SCALE)
                    if j % 2 == 1:
                        jp = j // 2  # chunk pair (2jp, 2jp+1) ready
                        for t in range(2):
                            for h in range(2):
                                nc.tensor.matmul(
                                    pv_ps[t][:, h * 512:(h + 1) * 512],
                                    lhsT=vt_sb[:, 2 * jp:2 * jp + 2,
                                               t * 128:(t + 1) * 128],
                                    rhs=expS[:, 2 * jp:2 * jp + 2,
                                             h * 512:(h + 1) * 512],
                                    start=(jp == 0), stop=(jp == 3),
                                    perf_mode=DR)

                # denominator: ones.T @ E accumulated over chunk pairs,
                # result rows are all equal to den[n]; then reciprocal.
                rden = work.tile([128, N], F32, name="rden", tag="rden")
                for h in range(2):
                    ps_bc = psum.tile([128, 512], F32, name="ps_bc",
                                      tag="work", bufs=4)
                    for jp in range(4):
                        nc.tensor.matmul(
                            ps_bc[:],
                            lhsT=ones8[:],
                            rhs=expS[:, 2 * jp:2 * jp + 2,
                                     h * 512:(h + 1) * 512],
                            start=(jp == 0), stop=(jp == 3), perf_mode=DR)
                    nc.vector.reciprocal_approx_fast(
                        out=rden[:, h * 512:(h + 1) * 512], in_=ps_bc[:])

                # normalize PV -> attn (bf16, feeds fuse matmul)
                attn_sb = work.tile([128, 2, N], BF16, name="attn_sb",
                                    tag="attn")
                for t in range(2):
                    for h in range(2):
                        sl = slice(h * 512, (h + 1) * 512)
                        nc.vector.scalar_tensor_tensor(
                            out=attn_sb[:, t, sl], in0=pv_ps[t][:, sl],
                            scalar=1.0 / WSCALE, in1=rden[:, sl],
                            op0=OP.mult, op1=OP.mult)

                # ================= fuse + residual + LN stats =============
                # f-half chunks first (no attn dependency), then attn half.
                ps_f = {}
                for t in range(2):
                    for h in range(2):
                        p = psum.tile([128, 512], F32, name="ps_f",
                                      tag="work", bufs=4)
                        ps_f[(t, h)] = p
                        for kc in range(2):
                            nc.tensor.matmul(
                                p[:],
                                lhsT=wf[:, kc, t * 128:(t + 1) * 128],
                                rhs=fb[s][:, kc, h * 512:(h + 1) * 512],
                                start=(kc == 0), stop=False)
                h_t = work.tile([128, 2, N], BF16, name="h_t", tag=f"h{s}",
                                bufs=2)
                g_t = work.tile([128, 2, N], BF16, name="g_t", tag="g_t",
                                bufs=3)
                for t in range(2):
                    for h in range(2):
                        p = ps_f[(t, h)]
                        for kc in range(2, 4):
                            nc.tensor.matmul(
                                p[:],
                                lhsT=wf[:, kc, t * 128:(t + 1) * 128],
                                rhs=attn_sb[:, kc - 2, h * 512:(h + 1) * 512],
                                start=False, stop=(kc == 3))
                        nc.scalar.activation(
                            out=g_t[:, t, h * 512:(h + 1) * 512], in_=p[:],
                            func=AF.Relu, bias=fbias[:, t:t + 1], scale=1.0)
                # residual (bf16) + LN sum/sumsq accum -- deferred past
                # the next section's conv casts so they don't clog the DVE
                # queue ahead of psum-freeing casts
                def emit_stats(s=s, g_t=g_t, h_t=h_t, fb_s=fb[s],
                               stats=stats):
                    for t in range(2):
                        c0 = s * 4 + t * 2
                        nc.vector.scalar_tensor_tensor(
                            out=h_t[:, t, :], in0=g_t[:, t, :], scalar=1.0,
                            in1=fb_s[:, t, :], op0=OP.mult, op1=OP.add,
                            accum_out=stats[:, c0:c0 + 1])
                        dum = work.tile([128, N], BF16, name="dum", tag="dum")
                        nc.scalar.activation(
                            out=dum[:], in_=h_t[:, t, :], func=AF.Square,
                            accum_out=stats[:, c0 + 1:c0 + 2])
                deferred_stats.append(emit_stats)
                h_sb.append(h_t)

                if s == 0:
                    # previous item's LN chain hides behind this item's work
                    ln_epi_chain()

            pend.append((i, stats, h_sb))

        while deferred_stats:
            deferred_stats.pop(0)()
        ln_epi_chain()
        ln_epi_apply()

        psum.release()
        work.release()
        inp.release()
        consts.release()

    nc.compile()
    return nc


_NC_CACHE = None


def _get_nc():
    global _NC_CACHE
    if _NC_CACHE is None:
        _NC_CACHE = _build()
    return _NC_CACHE


def kernel(fs, fi, qs_w, ks_w, vs_w, qi_w, ki_w, vi_w,
           fuse_w, fuse_b, ln_s_w, ln_s_b, ln_i_w, ln_i_b):
    global LAST_RESULT
    fs = np.asarray(fs, np.float32)
    fi = np.asarray(fi, np.float32)

    def prep_f(x):
        # (B, C, H, W) -> per-core [IPC, 2, 128, N]
        return x.reshape(NCORES, IPC, 2, 128, N)

    def prep_w_qk(w):  # (128, 256) -> lhsT layout [128p, 2kc, 128m] * 32
        wt = np.ascontiguousarray(np.asarray(w, np.float32).T) * WSCALE
        return np.ascontiguousarray(
            wt.reshape(2, 128, 128).transpose(1, 0, 2)).astype(
                ml_dtypes.float8_e4m3)

    def prep_w_v(w):  # (256, 256) -> rhs layout [128p, 2kc, 256c] * 32
        wt = np.ascontiguousarray(np.asarray(w, np.float32).T) * WSCALE
        return np.ascontiguousarray(
            wt.reshape(2, 128, 256).transpose(1, 0, 2)).astype(
                ml_dtypes.float8_e4m3)

    fs_sh = prep_f(fs)
    fi_sh = prep_f(fi)
    fs_bf = fs_sh.astype(ml_dtypes.bfloat16)
    fi_bf = fi_sh.astype(ml_dtypes.bfloat16)
    fs_q8 = fs_sh.astype(ml_dtypes.float8_e4m3)
    fi_q8 = fi_sh.astype(ml_dtypes.float8_e4m3)

    wq0 = prep_w_qk(qs_w)
    wq1 = prep_w_qk(qi_w)
    wk0 = prep_w_qk(ks_w)
    wk1 = prep_w_qk(ki_w)
    wv0 = prep_w_v(vs_w)
    wv1 = prep_w_v(vi_w)
    wfuse = np.ascontiguousarray(
        np.asarray(fuse_w, np.float32).T.reshape(4, 128, 256)
        .transpose(1, 0, 2)).astype(ml_dtypes.bfloat16)
    fuseb = np.ascontiguousarray(
        np.asarray(fuse_b, np.float32).reshape(2, 128).T)
    lnw = np.ascontiguousarray(
        np.stack([np.asarray(ln_s_w, np.float32).reshape(256),
                  np.asarray(ln_i_w, np.float32).reshape(256)])
        .reshape(2, 2, 128).transpose(2, 0, 1))
    lnb = np.ascontiguousarray(
        np.stack([np.asarray(ln_s_b, np.float32).reshape(256),
                  np.asarray(ln_i_b, np.float32).reshape(256)])
        .reshape(2, 2, 128).transpose(2, 0, 1))

    in_maps = []
    for c in range(NCORES):
        in_maps.append({
            "fsb": np.ascontiguousarray(fs_bf[c]),
            "fib": np.ascontiguousarray(fi_bf[c]),
            "fs8": np.ascontiguousarray(fs_q8[c]),
            "fi8": np.ascontiguousarray(fi_q8[c]),
            "wq0": wq0, "wq1": wq1, "wk0": wk0, "wk1": wk1,
            "wv0": wv0, "wv1": wv1, "wfuse": wfuse, "fuseb": fuseb,
            "lnw": lnw, "lnb": lnb,
        })

    nc = _get_nc()
    res = run_bass_kernel_spmd(nc, in_maps, core_ids=list(range(NCORES)),
                               **RUN_KWARGS)
    LAST_RESULT = res

    fs_out = np.empty((NCORES, IPC, 2, 128, N), np.float32)
    fi_out = np.empty((NCORES, IPC, 2, 128, N), np.float32)
    for c in range(NCORES):
        fs_out[c] = res.results[c]["out0"]
        fi_out[c] = res.results[c]["out1"]
    fs_out = fs_out.reshape(B, C, 32, 32)
    fi_out = fi_out.reshape(B, C, 32, 32)
    return fs_out, fi_out


# revision 11
# speedup vs baseline: 1.3763x; 1.0079x over previous
"""Trainium2 Bass kernel for nn_CrossAttention2d (B=32, C=256, INNER=128, H=W=32).

Sharding: pure data parallel — batch 32 split as 4 items per core across 8
NeuronCores; all weights replicated. No collectives.

Per item (N = H*W = 1024 tokens, C = 256 channels, D = 128 inner):
  attention for output stream s (s=0 -> fs side, s=1 -> fi side):
      q = wq[1-s] @ f[1-s], k = wk[s] @ f[s]   (fp8 DoubleRow, x32 prescale)
      vT[m, c] = (wv[s] @ f[s]).T   -- computed directly via DoubleRow with
                 f-slices as the stationary operand
      S^T[m, n] = sum_d k[d, m] q[d, n]        (bf16 PE, m-tiles of 128)
      E = exp(S^T / (1024 sqrt(D)))            (ACT, psum -> fp8 sbuf)
      O_un[c, n] = sum_m vT[m, c] E[m, n]      (fp8 DoubleRow, 4 chunk-pairs)
      den[n] via ones.T @ E (fp8 DoubleRow) broadcast to 128 rows
      attn = O_un * (1/32) * (1/den)           (DVE scalar_tensor_tensor)
  fuse: g = relu(Wf @ [f[s]; attn] + b)        (bf16 PE + ACT)
  h = g + f[s] (bf16 residual); LayerNorm over all (C,N) of h; LN stats via
  DVE accum_out + PE partition-reduce; out = h * A + B (DVE tensor_scalar).
  The LN scalar epilogue of item i is emitted inside item i+1 (software
  pipelining) so its serial tiny-op chain hides behind PE work.

Matmul convention: out[M, N] = lhsT.T @ rhs, lhsT = [K<=128, M<=128] (K on
partitions), rhs = [K, N<=512], out in PSUM f32 (one bank per matmul).
DoubleRow: lhsT [Ki, 2, M], rhs [Ki, 2, N] fp8 -> contracts 2*Ki.
PSUM: "pv" tag 2x[128,1024] (4 banks) + "work" tag 4x[128,512] (4 banks).
"""

import numpy as np
import ml_dtypes

import concourse.bacc as bacc
import concourse.bass as bass
import concourse.tile as tile
from concourse import mybir
from concourse.bass_utils import run_bass_kernel_spmd

F32 = mybir.dt.float32
BF16 = mybir.dt.bfloat16
FP8 = mybir.dt.float8e4
DR = mybir.MatmulPerfMode.DoubleRow
AF = mybir.ActivationFunctionType
OP = mybir.AluOpType

B, C, D, N = 32, 256, 128, 1024
NCORES = 8
IPC = B // NCORES  # items per core = 4
WSCALE = 32.0  # fp8 weight prescale (w*32 keeps N(0,0.02) in e4m3 range)
EXP_SCALE = (1.0 / float(np.sqrt(D))) / (WSCALE * WSCALE)
EPS = 1e-5
NTOT = float(C * N)  # layernorm element count per item/stream

# test.py can set {"trace": True}; harness path leaves this empty.
RUN_KWARGS = {}
LAST_RESULT = None


def _build():
    nc = bacc.Bacc("TRN2", target_bir_lowering=False, debug=False,
                   num_devices=NCORES)

    # ---- DRAM I/O (per-core shapes) ----
    fb_d = [nc.dram_tensor(n_, [IPC, 2, 128, N], BF16, kind="ExternalInput")
            for n_ in ("fsb", "fib")]
    f8_d = [nc.dram_tensor(n_, [IPC, 2, 128, N], FP8, kind="ExternalInput")
            for n_ in ("fs8", "fi8")]
    wq_d = [nc.dram_tensor(n_, [128, 2, 128], FP8, kind="ExternalInput")
            for n_ in ("wq0", "wq1")]
    wk_d = [nc.dram_tensor(n_, [128, 2, 128], FP8, kind="ExternalInput")
            for n_ in ("wk0", "wk1")]
    wv_d = [nc.dram_tensor(n_, [128, 2, 256], FP8, kind="ExternalInput")
            for n_ in ("wv0", "wv1")]
    wf_d = nc.dram_tensor("wfuse", [128, 4, 256], BF16, kind="ExternalInput")
    fb_bias_d = nc.dram_tensor("fuseb", [128, 2], F32, kind="ExternalInput")
    lnw_d = nc.dram_tensor("lnw", [128, 2, 2], F32, kind="ExternalInput")
    lnb_d = nc.dram_tensor("lnb", [128, 2, 2], F32, kind="ExternalInput")
    out_d = [nc.dram_tensor(n_, [IPC, 2, 128, N], F32, kind="ExternalOutput")
             for n_ in ("out0", "out1")]

    with tile.TileContext(nc) as tc:
        consts = tc.alloc_tile_pool(name="consts", bufs=1)
        inp = tc.alloc_tile_pool(name="inp", bufs=2)
        work = tc.alloc_tile_pool(name="work", bufs=2)
        psum = tc.alloc_tile_pool(name="psum", bufs=2, space="PSUM")

        # ---- load constants ----
        wq = [consts.tile([128, 2, 128], FP8, name=f"wq{s}", tag=f"wq{s}")
              for s in range(2)]
        wk = [consts.tile([128, 2, 128], FP8, name=f"wk{s}", tag=f"wk{s}")
              for s in range(2)]
        wv = [consts.tile([128, 2, 256], FP8, name=f"wv{s}", tag=f"wv{s}")
              for s in range(2)]
        wf = consts.tile([128, 4, 256], BF16, name="wf", tag="wf")
        fbias = consts.tile([128, 2], F32, name="fbias", tag="fbias")
        lnw = consts.tile([128, 2, 2], F32, name="lnw", tag="lnw")
        lnb = consts.tile([128, 2, 2], F32, name="lnb", tag="lnb")
        ones8 = consts.tile([128, 2, 128], FP8, name="ones8", tag="ones8")
        ones_col = consts.tile([128, 1], F32, name="ones_col", tag="ones_col")
        ones_row = consts.tile([1, 128], F32, name="ones_row", tag="ones_row")
        for s in range(2):
            nc.sync.dma_start(out=wq[s][:], in_=wq_d[s][:])
            nc.sync.dma_start(out=wk[s][:], in_=wk_d[s][:])
            nc.sync.dma_start(out=wv[s][:], in_=wv_d[s][:])
        nc.sync.dma_start(out=wf[:], in_=wf_d[:])
        nc.sync.dma_start(out=fbias[:], in_=fb_bias_d[:])
        nc.sync.dma_start(out=lnw[:], in_=lnw_d[:])
        nc.sync.dma_start(out=lnb[:], in_=lnb_d[:])
        nc.vector.memset(ones8[:], 1.0)
        nc.vector.memset(ones_col[:], 1.0)
        nc.vector.memset(ones_row[:], 1.0)

        def conv_qk(w_t, f8_t, name):
            """[128, N] = (32w).T @ f via fp8 DoubleRow; bf16 sbuf out."""
            sb = work.tile([128, N], BF16, name=name, tag=name)
            for h in range(2):
                ps = psum.tile([128, 512], F32, name=f"ps_{name}", tag="work",
                               bufs=4)
                nc.tensor.matmul(
                    ps[:], lhsT=w_t[:],
                    rhs=f8_t[:, :, h * 512:(h + 1) * 512],
                    start=True, stop=True, perf_mode=DR)
                nc.vector.tensor_copy(out=sb[:, h * 512:(h + 1) * 512],
                                      in_=ps[:])
            return sb

        # -------- per-item state carried into the next item (LN epilogue)
        deferred_stats = []  # closures emitting residual-stt + square ops
        pend = []       # [(i, stats, h_sb)] awaiting the stats->A/B chain
        pend_apply = []  # [(i, h_sb, A, B)] awaiting LN apply + store

        def ln_epi_chain():
            """Stats -> mean/var -> rstd (DVE Newton) -> A/B. No ACT, and
            the only PE op (stats colsum) has its inputs long ready, so the
            PE stream never blocks on this chain."""
            if not pend:
                return
            i, stats, h_sb = pend.pop()
            ps_st = psum.tile([1, 8], F32, name="ps_st", tag="work", bufs=4)
            nc.tensor.matmul(ps_st[:], lhsT=ones_col[:], rhs=stats[:],
                             start=True, stop=True)
            st = work.tile([1, 8], F32, name="st", tag="st")
            nc.vector.tensor_copy(out=st[:], in_=ps_st[:])
            # cols: s*4 + t*2 + k (k=0 sum, k=1 sumsq) -> tot over t
            st_r = st[:].rearrange("p (a b) -> p a b", a=2)  # a=s, b=(t,k)
            tot = work.tile([1, 4], F32, name="tot", tag="tot")
            nc.vector.tensor_add(
                out=tot[:].rearrange("p (a b) -> p a b", a=2),
                in0=st_r[:, :, 0:2],
                in1=st_r[:, :, 2:4])
            # tot = [s0_sum, s0_sq, s1_sum, s1_sq] -> moments = tot / NTOT
            mom = work.tile([1, 4], F32, name="mom", tag="mom")
            nc.vector.tensor_scalar(out=mom[:], in0=tot[:],
                                    scalar1=1.0 / NTOT, scalar2=None,
                                    op0=OP.mult)
            mom_r = mom[:].rearrange("p (a b) -> p a b", a=2)
            mu_ap = mom_r[:, :, 0]
            m2_ap = mom_r[:, :, 1]
            musq = work.tile([1, 2], F32, name="musq", tag="musq")
            nc.vector.tensor_tensor(out=musq[:], in0=mu_ap, in1=mu_ap,
                                    op=OP.mult)
            var = work.tile([1, 2], F32, name="var", tag="var")
            nc.vector.scalar_tensor_tensor(
                out=var[:], in0=musq[:], scalar=-1.0, in1=m2_ap,
                op0=OP.mult, op1=OP.add)
            nc.vector.tensor_scalar(out=var[:], in0=var[:], scalar1=EPS,
                                    scalar2=None, op0=OP.add)
            # rstd = var^-0.5 via Newton (all-DVE; var is ~[0.3, 3] so the
            # constant seed converges: err 30% -> 6% -> 0.3% -> 1e-5)
            mr = work.tile([1, 4], F32, name="mr", tag="mr")
            y = mr[:, 0:2]
            nc.vector.memset(y, 0.92)
            t1 = work.tile([1, 2], F32, name="t1", tag="t1")
            for _ in range(3):
                nc.vector.tensor_tensor(out=t1[:], in0=y, in1=y, op=OP.mult)
                nc.vector.tensor_tensor(out=t1[:], in0=var[:], in1=t1[:],
                                        op=OP.mult)
                nc.vector.tensor_scalar(out=t1[:], in0=t1[:], scalar1=-0.5,
                                        scalar2=1.5, op0=OP.mult, op1=OP.add)
                nc.vector.tensor_tensor(out=y, in0=y, in1=t1[:], op=OP.mult)
            nc.vector.tensor_scalar(out=mr[:, 2:4], in0=mu_ap,
                                    scalar1=-1.0, scalar2=None, op0=OP.mult)
            # broadcast [1,4] -> [128,4] on GpSimd (PE stays out of it)
            mrb = work.tile([128, 4], F32, name="mrb", tag="mrb")
            nc.gpsimd.partition_broadcast(out_ap=mrb[:], in_ap=mr[:])
            # A = lnw * rstd ; Bb = lnb + (-mu) * A
            AB = []
            for s in range(2):
                Asb = work.tile([128, 2], F32, name="Asb", tag=f"A{s}")
                nc.vector.tensor_scalar(
                    out=Asb[:], in0=lnw[:, s, :], scalar1=mrb[:, s:s + 1],
                    scalar2=None, op0=OP.mult)
                Bsb = work.tile([128, 2], F32, name="Bsb", tag=f"B{s}")
                nc.vector.scalar_tensor_tensor(
                    out=Bsb[:], in0=Asb[:], scalar=mrb[:, 2 + s:3 + s],
                    in1=lnb[:, s, :], op0=OP.mult, op1=OP.add)
                AB.append((Asb, Bsb))
            pend_apply.append((i, h_sb, AB))

        def ln_epi_apply():
            if not pend_apply:
                return
            i, h_sb, AB = pend_apply.pop()
            for s in range(2):
                Asb, Bsb = AB[s]
                for t in range(2):
                    o_t = work.tile([128, N], F32, name="o_t", tag="o_t",
                                    bufs=4)
                    nc.vector.tensor_scalar(
                        out=o_t[:], in0=h_sb[s][:, t, :],
                        scalar1=Asb[:, t:t + 1], scalar2=Bsb[:, t:t + 1],
                        op0=OP.mult, op1=OP.add)
                    nc.sync.dma_start(out=out_d[s][i, t], in_=o_t[:])

        for i in range(IPC):
            # ---- input DMAs ----
            fb = []
            f8 = []
            for s in range(2):
                t = inp.tile([128, 2, N], BF16, name=f"fb{s}", tag=f"fb{s}")
                nc.sync.dma_start(
                    out=t[:], in_=fb_d[s][i].rearrange("c p n -> p c n"))
                fb.append(t)
                t8 = inp.tile([128, 2, N], FP8, name=f"f8_{s}", tag=f"f8_{s}")
                nc.sync.dma_start(
                    out=t8[:], in_=f8_d[s][i].rearrange("c p n -> p c n"))
                f8.append(t8)

            stats = work.tile([128, 8], F32, name="stats", tag="stats")
            h_sb = []

            for s in range(2):
                # ================= attention for output stream s ==========
                q_sb = conv_qk(wq[1 - s], f8[1 - s], "q_sb")
                k_sb = conv_qk(wk[s], f8[s], "k_sb")

                # vT[m, c] via DoubleRow: stationary = f8 slice pair
                vt_sb = work.tile([128, 8, 256], FP8, name="vt_sb", tag="vt")
                for half in range(4):
                    ps_vt = psum.tile([128, 512], F32, name="ps_vt",
                                      tag="work", bufs=4)
                    for jj in range(2):
                        j = half * 2 + jj
                        nc.tensor.matmul(
                            ps_vt[:, jj * 256:(jj + 1) * 256],
                            lhsT=f8[s][:, :, j * 128:(j + 1) * 128],
                            rhs=wv[s][:],
                            start=True, stop=True, perf_mode=DR)
                    nc.vector.tensor_copy(
                        out=vt_sb[:, half * 2:(half + 1) * 2, :]
                        .rearrange("p a b -> p (a b)"),
                        in_=ps_vt[:])

                # flush the previous section's residual/square stat ops
                # here: the S^T/PV phase ahead has no DVE work, so theyueue
                # behind the vT casts harmlessly
                while deferred_stats:
                    deferred_stats.pop(0)()
                if s == 1:
                    # previous item's LN apply lands between this attention's
                    # casts and its tail ops on the DVE queue
                    ln_epi_apply()

                # S^T -> exp(fp8) ; PV accumulates DoubleRow chunk-pairs
                pv_ps = [psum.tile([128, N], F32, name=f"pv{t}", tag="pv")
                         for t in range(2)]
                expS = work.tile([128, 8, N], FP8, name="expS", tag="expS")
                for j in range(8):
                    for h in range(2):
                        ps_s = psum.tile([128, 512], F32, name="ps_s",
                                         tag="work", bufs=4)
                        nc.tensor.matmul(
                            ps_s[:],
                            lhsT=k_sb[:, j * 128:(j + 1) * 128],
                            rhs=q_sb[:, h * 512:(h + 1) * 512],
                            start=True, stop=True)
                        nc.scalar.activation(
                            out=expS[:, j, h * 512:(h + 1) * 512],
                            in_=ps_s[:], func=AF.Exp, scale=EXP_SCALE)
                    if j % 2 == 1:
                        jp = j // 2  # chunk pair (2jp, 2jp+1) ready
                        for t in range(2):
                            for h in range(2):
                                nc.tensor.matmul(
                                    pv_ps[t][:, h * 512:(h + 1) * 512],
                                    lhsT=vt_sb[:, 2 * jp:2 * jp + 2,
                                               t * 128:(t + 1) * 128],
                                    rhs=expS[:, 2 * jp:2 * jp + 2,
                                             h * 512:(h + 1) * 512],
                                    start=(jp == 0), stop=(jp == 3),
                                    perf_mode=DR)

                # denominator: ones.T @ E accumulated over chunk pairs,
                # result rows are all equal to den[n]; then reciprocal.
                rden = work.tile([128, N], F32, name="rden", tag="rden")
                for h in range(2):
                    ps_bc = psum.tile([128, 512], F32, name="ps_bc",
                                      tag="work", bufs=4)
                    for jp in range(4):
                        nc.tensor.matmul(
                            ps_bc[:],
                            lhsT=ones8[:],
                            rhs=expS[:, 2 * jp:2 * jp + 2,
                                     h * 512:(h + 1) * 512],
                            start=(jp == 0), stop=(jp == 3), perf_mode=DR)
                    nc.vector.reciprocal_approx_fast(
                        out=rden[:, h * 512:(h + 1) * 512], in_=ps_bc[:])

                # normalize PV -> attn (bf16, feeds fuse matmul)
                attn_sb = work.tile([128, 2, N], BF16, name="attn_sb",
                                    tag="attn")
                for t in range(2):
                    for h in range(2):
                        sl = slice(h * 512, (h + 1) * 512)
                        nc.vector.scalar_tensor_tensor(
                            out=attn_sb[:, t, sl], in0=pv_ps[t][:, sl],
                            scalar=1.0 / WSCALE, in1=rden[:, sl],
                            op0=OP.mult, op1=OP.mult)

                # ================= fuse + residual + LN stats =============
                # f-half chunks first (no attn dependency), then attn half.
                ps_f = {}
                for t in range(2):
                    for h in range(2):
                        p = psum.tile([128, 512], F32, name="ps_f",
                                      tag="work", bufs=4)
                        ps_f[(t, h)] = p
                        for kc in range(2):
                            nc.tensor.matmul(
                                p[:],
                                lhsT=wf[:, kc, t * 128:(t + 1) * 128],
                                rhs=fb[s][:, kc, h * 512:(h + 1) * 512],
                                start=(kc == 0), stop=False)
                h_t = work.tile([128, 2, N], BF16, name="h_t", tag=f"h{s}",
                                bufs=2)
                g_t = work.tile([128, 2, N], BF16, name="g_t", tag="g_t",
                                bufs=3)
                for t in range(2):
                    for h in range(2):
                        p = ps_f[(t, h)]
                        for kc in range(2, 4):
                            nc.tensor.matmul(
                                p[:],
                                lhsT=wf[:, kc, t * 128:(t + 1) * 128],
                                rhs=attn_sb[:, kc - 2, h * 512:(h + 1) * 512],
                                start=False, stop=(kc == 3))
                        nc.scalar.activation(
                            out=g_t[:, t, h * 512:(h + 1) * 512], in_=p[:],
                            func=AF.Relu, bias=fbias[:, t:t + 1], scale=1.0)
                # residual (bf16) + LN sum/sumsq accum -- deferred past
                # the next section's conv casts so they don't clog the DVE
                # queue ahead of psum-freeing casts
                def emit_stats(s=s, g_t=g_t, h_t=h_t, fb_s=fb[s],
                               stats=stats):
                    for t in range(2):
                        c0 = s * 4 + t * 2
                        nc.vector.scalar_tensor_tensor(
                            out=h_t[:, t, :], in0=g_t[:, t, :], scalar=1.0,
                            in1=fb_s[:, t, :], op0=OP.mult, op1=OP.add,
                            accum_out=stats[:, c0:c0 + 1])
                        dum = work.tile([128, N], BF16, name="dum", tag="dum")
                        nc.scalar.activation(
                            out=dum[:], in_=h_t[:, t, :], func=AF.Square,
                            accum_out=stats[:, c0 + 1:c0 + 2])
                deferred_stats.append(emit_stats)
                h_sb.append(h_t)

                if s == 0:
                    # previous item's LN chain hides behind this item's work
                    ln_epi_chain()

            pend.append((i, stats, h_sb))

        while deferred_stats:
            deferred_stats.pop(0)()
        ln_epi_chain()
        ln_epi_apply()

        psum.release()
        work.release()
        inp.release()
        consts.release()

    nc.compile()
    return nc


_NC_CACHE = None


def _get_nc():
    global _NC_CACHE
    if _NC_CACHE is None:
        _NC_CACHE = _build()
    return _NC_CACHE


def kernel(fs, fi, qs_w, ks_w, vs_w, qi_w, ki_w, vi_w,
           fuse_w, fuse_b, ln_s_w, ln_s_b, ln_i_w, ln_i_b):
    global LAST_RESULT
    fs = np.asarray(fs, np.float32)
    fi = np.asarray(fi, np.float32)

    def prep_f(x):
        # (B, C, H, W) -> per-core [IPC, 2, 128, N]
        return x.reshape(NCORES, IPC, 2, 128, N)

    def prep_w_qk(w):  # (128, 256) -> lhsT layout [128p, 2kc, 128m] * 32
        wt = np.ascontiguousarray(np.asarray(w, np.float32).T) * WSCALE
        return np.ascontiguousarray(
            wt.reshape(2, 128, 128).transpose(1, 0, 2)).astype(
                ml_dtypes.float8_e4m3)

    def prep_w_v(w):  # (256, 256) -> rhs layout [128p, 2kc, 256c] * 32
        wt = np.ascontiguousarray(np.asarray(w, np.float32).T) * WSCALE
        return np.ascontiguousarray(
            wt.reshape(2, 128, 256).transpose(1, 0, 2)).astype(
                ml_dtypes.float8_e4m3)

    fs_sh = prep_f(fs)
    fi_sh = prep_f(fi)
    fs_bf = fs_sh.astype(ml_dtypes.bfloat16)
    fi_bf = fi_sh.astype(ml_dtypes.bfloat16)
    fs_q8 = fs_sh.astype(ml_dtypes.float8_e4m3)
    fi_q8 = fi_sh.astype(ml_dtypes.float8_e4m3)

    wq0 = prep_w_qk(qs_w)
    wq1 = prep_w_qk(qi_w)
    wk0 = prep_w_qk(ks_w)
    wk1 = prep_w_qk(ki_w)
    wv0 = prep_w_v(vs_w)
    wv1 = prep_w_v(vi_w)
    wfuse = np.ascontiguousarray(
        np.asarray(fuse_w, np.float32).T.reshape(4, 128, 256)
        .transpose(1, 0, 2)).astype(ml_dtypes.bfloat16)
    fuseb = np.ascontiguousarray(
        np.asarray(fuse_b, np.float32).reshape(2, 128).T)
    lnw = np.ascontiguousarray(
        np.stack([np.asarray(ln_s_w, np.float32).reshape(256),
                  np.asarray(ln_i_w, np.float32).reshape(256)])
        .reshape(2, 2, 128).transpose(2, 0, 1))
    lnb = np.ascontiguousarray(
        np.stack([np.asarray(ln_s_b, np.float32).reshape(256),
                  np.asarray(ln_i_b, np.float32).reshape(256)])
        .reshape(2, 2, 128).transpose(2, 0, 1))

    in_maps = []
    for c in range(NCORES):
        in_maps.append({
            "fsb": np.ascontiguousarray(fs_bf[c]),
            "fib": np.ascontiguousarray(fi_bf[c]),
            "fs8": np.ascontiguousarray(fs_q8[c]),
            "fi8": np.ascontiguousarray(fi_q8[c]),
            "wq0": wq0, "wq1": wq1, "wk0": wk0, "wk1": wk1,
            "wv0": wv0, "wv1": wv1, "wfuse": wfuse, "fuseb": fuseb,
            "lnw": lnw, "lnb": lnb,
        })

    nc = _get_nc()
    res = run_bass_kernel_spmd(nc, in_maps, core_ids=list(range(NCORES)),
                               **RUN_KWARGS)
    LAST_RESULT = res

    fs_out = np.empty((NCORES, IPC, 2, 128, N), np.float32)
    fi_out = np.empty((NCORES, IPC, 2, 128, N), np.float32)
    for c in range(NCORES):
        fs_out[c] = res.results[c]["out0"]
        fi_out[c] = res.results[c]["out1"]
    fs_out = fs_out.reshape(B, C, 32, 32)
    fi_out = fi_out.reshape(B, C, 32, 32)
    return fs_out, fi_out


# revision 12
# speedup vs baseline: 1.5150x; 1.1008x over previous
"""Trainium2 Bass kernel for nn_CrossAttention2d (B=32, C=256, INNER=128, H=W=32).

Sharding: pure data parallel — batch 32 split as 4 items per core across 8
NeuronCores; all weights replicated. No collectives.

Per item (N = H*W = 1024 tokens, C = 256 channels, D = 128 inner):
  attention for output stream s (s=0 -> fs side, s=1 -> fi side):
      q = wq[1-s] @ f[1-s], k = wk[s] @ f[s]   (fp8 DoubleRow, x32 prescale)
      vT[m, c] = (wv[s] @ f[s]).T   -- computed directly via DoubleRow with
                 f-slices as the stationary operand
      S^T[m, n] = sum_d k[d, m] q[d, n]        (bf16 PE, m-tiles of 128)
      E = exp(S^T / (1024 sqrt(D)))            (ACT, psum -> fp8 sbuf)
      O_un[c, n] = sum_m vT[m, c] E[m, n]      (fp8 DoubleRow, 4 chunk-pairs)
      den[n] via ones.T @ E (fp8 DoubleRow) broadcast to 128 rows
      attn = O_un * (1/32) * (1/den)           (DVE scalar_tensor_tensor)
  fuse: g = relu(Wf @ [f[s]; attn] + b)        (bf16 PE + ACT)
  h = g + f[s] (bf16 residual); LayerNorm over all (C,N) of h; LN stats via
  DVE accum_out + PE partition-reduce; out = h * A + B (DVE tensor_scalar).
  The LN scalar epilogue of item i is emitted inside item i+1 (software
  pipelining) so its serial tiny-op chain hides behind PE work.

Matmul convention: out[M, N] = lhsT.T @ rhs, lhsT = [K<=128, M<=128] (K on
partitions), rhs = [K, N<=512], out in PSUM f32 (one bank per matmul).
DoubleRow: lhsT [Ki, 2, M], rhs [Ki, 2, N] fp8 -> contracts 2*Ki.
PSUM: "pv" tag 2x[128,1024] (4 banks) + "work" tag 4x[128,512] (4 banks).
"""

import numpy as np
import ml_dtypes

import concourse.bacc as bacc
import concourse.bass as bass
import concourse.tile as tile
from concourse import mybir
from concourse.bass_utils import run_bass_kernel_spmd

F32 = mybir.dt.float32
BF16 = mybir.dt.bfloat16
FP8 = mybir.dt.float8e4
DR = mybir.MatmulPerfMode.DoubleRow
AF = mybir.ActivationFunctionType
OP = mybir.AluOpType

B, C, D, N = 32, 256, 128, 1024
NCORES = 8
IPC = B // NCORES  # items per core = 4
WSCALE = 32.0  # fp8 weight prescale (w*32 keeps N(0,0.02) in e4m3 range)
EXP_SCALE = (1.0 / float(np.sqrt(D))) / (WSCALE * WSCALE)
EPS = 1e-5
NTOT = float(C * N)  # layernorm element count per item/stream

# test.py can set {"trace": True}; harness path leaves this empty.
RUN_KWARGS = {}
LAST_RESULT = None


def _build():
    nc = bacc.Bacc("TRN2", target_bir_lowering=False, debug=False,
                   num_devices=NCORES)

    # ---- DRAM I/O (per-core shapes) ----
    fb_d = [nc.dram_tensor(n_, [IPC, 2, 128, N], BF16, kind="ExternalInput")
            for n_ in ("fsb", "fib")]
    f8_d = [nc.dram_tensor(n_, [IPC, 2, 128, N], FP8, kind="ExternalInput")
            for n_ in ("fs8", "fi8")]
    wq_d = [nc.dram_tensor(n_, [128, 2, 128], FP8, kind="ExternalInput")
            for n_ in ("wq0", "wq1")]
    wk_d = [nc.dram_tensor(n_, [128, 2, 128], FP8, kind="ExternalInput")
            for n_ in ("wk0", "wk1")]
    wv_d = [nc.dram_tensor(n_, [128, 2, 256], FP8, kind="ExternalInput")
            for n_ in ("wv0", "wv1")]
    wf_d = nc.dram_tensor("wfuse", [128, 4, 256], BF16, kind="ExternalInput")
    fb_bias_d = nc.dram_tensor("fuseb", [128, 2], F32, kind="ExternalInput")
    lnw_d = nc.dram_tensor("lnw", [128, 2, 2], F32, kind="ExternalInput")
    lnb_d = nc.dram_tensor("lnb", [128, 2, 2], F32, kind="ExternalInput")
    out_d = [nc.dram_tensor(n_, [IPC, 2, 128, N], F32, kind="ExternalOutput")
             for n_ in ("out0", "out1")]

    with tile.TileContext(nc) as tc:
        consts = tc.alloc_tile_pool(name="consts", bufs=1)
        inp = tc.alloc_tile_pool(name="inp", bufs=2)
        work = tc.alloc_tile_pool(name="work", bufs=2)
        psum = tc.alloc_tile_pool(name="psum", bufs=2, space="PSUM")

        # ---- load constants ----
        wq = [consts.tile([128, 2, 128], FP8, name=f"wq{s}", tag=f"wq{s}")
              for s in range(2)]
        wk = [consts.tile([128, 2, 128], FP8, name=f"wk{s}", tag=f"wk{s}")
              for s in range(2)]
        wv = [consts.tile([128, 2, 256], FP8, name=f"wv{s}", tag=f"wv{s}")
              for s in range(2)]
        wf = consts.tile([128, 4, 256], BF16, name="wf", tag="wf")
        fbias = consts.tile([128, 2], F32, name="fbias", tag="fbias")
        lnw = consts.tile([128, 2, 2], F32, name="lnw", tag="lnw")
        lnb = consts.tile([128, 2, 2], F32, name="lnb", tag="lnb")
        ones8 = consts.tile([128, 2, 128], FP8, name="ones8", tag="ones8")
        ones_col = consts.tile([128, 1], F32, name="ones_col", tag="ones_col")
        ones_row = consts.tile([1, 128], F32, name="ones_row", tag="ones_row")
        for s in range(2):
            nc.sync.dma_start(out=wq[s][:], in_=wq_d[s][:])
            nc.sync.dma_start(out=wk[s][:], in_=wk_d[s][:])
            nc.sync.dma_start(out=wv[s][:], in_=wv_d[s][:])
        nc.sync.dma_start(out=wf[:], in_=wf_d[:])
        nc.sync.dma_start(out=fbias[:], in_=fb_bias_d[:])
        nc.sync.dma_start(out=lnw[:], in_=lnw_d[:])
        nc.sync.dma_start(out=lnb[:], in_=lnb_d[:])
        nc.vector.memset(ones8[:], 1.0)
        nc.vector.memset(ones_col[:], 1.0)
        nc.vector.memset(ones_row[:], 1.0)

        def conv_qk(w_t, f8_t, name):
            """[128, N] = (32w).T @ f via fp8 DoubleRow; bf16 sbuf out."""
            sb = work.tile([128, N], BF16, name=name, tag=name)
            for h in range(2):
                ps = psum.tile([128, 512], F32, name=f"ps_{name}", tag="work",
                               bufs=4)
                nc.tensor.matmul(
                    ps[:], lhsT=w_t[:],
                    rhs=f8_t[:, :, h * 512:(h + 1) * 512],
                    start=True, stop=True, perf_mode=DR)
                nc.vector.tensor_copy(out=sb[:, h * 512:(h + 1) * 512],
                                      in_=ps[:])
            return sb

        # -------- per-item state carried into the next item (LN epilogue)
        deferred_stats = []  # closures emitting residual-stt + square ops
        pend = []       # [(i, stats, h_sb)] awaiting the stats->A/B chain
        pend_apply = []  # [(i, h_sb, A, B)] awaiting LN apply + store

        def ln_epi_chain():
            """Stats -> mean/var -> rstd (DVE Newton) -> A/B. No ACT, and
            the only PE op (stats colsum) has its inputs long ready, so the
            PE stream never blocks on this chain."""
            if not pend:
                return
            i, stats, h_sb = pend.pop()
            ps_st = psum.tile([1, 8], F32, name="ps_st", tag="work", bufs=4)
            nc.tensor.matmul(ps_st[:], lhsT=ones_col[:], rhs=stats[:],
                             start=True, stop=True)
            st = work.tile([1, 8], F32, name="st", tag="st")
            nc.vector.tensor_copy(out=st[:], in_=ps_st[:])
            # cols: s*4 + t*2 + k (k=0 sum, k=1 sumsq) -> tot over t
            st_r = st[:].rearrange("p (a b) -> p a b", a=2)  # a=s, b=(t,k)
            tot = work.tile([1, 4], F32, name="tot", tag="tot")
            nc.vector.tensor_add(
                out=tot[:].rearrange("p (a b) -> p a b", a=2),
                in0=st_r[:, :, 0:2],
                in1=st_r[:, :, 2:4])
            # tot = [s0_sum, s0_sq, s1_sum, s1_sq] -> moments = tot / NTOT
            mom = work.tile([1, 4], F32, name="mom", tag="mom")
            nc.vector.tensor_scalar(out=mom[:], in0=tot[:],
                                    scalar1=1.0 / NTOT, scalar2=None,
                                    op0=OP.mult)
            mom_r = mom[:].rearrange("p (a b) -> p a b", a=2)
            mu_ap = mom_r[:, :, 0]
            m2_ap = mom_r[:, :, 1]
            musq = work.tile([1, 2], F32, name="musq", tag="musq")
            nc.vector.tensor_tensor(out=musq[:], in0=mu_ap, in1=mu_ap,
                                    op=OP.mult)
            var = work.tile([1, 2], F32, name="var", tag="var")
            nc.vector.scalar_tensor_tensor(
                out=var[:], in0=musq[:], scalar=-1.0, in1=m2_ap,
                op0=OP.mult, op1=OP.add)
            nc.vector.tensor_scalar(out=var[:], in0=var[:], scalar1=EPS,
                                    scalar2=None, op0=OP.add)
            # rstd = var^-0.5 via Newton (all-DVE; var is ~[0.3, 3] so the
            # constant seed converges: err 30% -> 6% -> 0.3% -> 1e-5)
            mr = work.tile([1, 4], F32, name="mr", tag="mr")
            y = mr[:, 0:2]
            nc.vector.memset(y, 0.92)
            t1 = work.tile([1, 2], F32, name="t1", tag="t1")
            for _ in range(3):
                nc.vector.tensor_tensor(out=t1[:], in0=y, in1=y, op=OP.mult)
                nc.vector.tensor_tensor(out=t1[:], in0=var[:], in1=t1[:],
                                        op=OP.mult)
                nc.vector.tensor_scalar(out=t1[:], in0=t1[:], scalar1=-0.5,
                                        scalar2=1.5, op0=OP.mult, op1=OP.add)
                nc.vector.tensor_tensor(out=y, in0=y, in1=t1[:], op=OP.mult)
            nc.vector.tensor_scalar(out=mr[:, 2:4], in0=mu_ap,
                                    scalar1=-1.0, scalar2=None, op0=OP.mult)
            # broadcast [1,4] -> [128,4] on GpSimd (PE stays out of it)
            mrb = work.tile([128, 4], F32, name="mrb", tag="mrb")
            nc.gpsimd.partition_broadcast(out_ap=mrb[:], in_ap=mr[:])
            # A = lnw * rstd ; Bb = lnb + (-mu) * A
            AB = []
            for s in range(2):
                Asb = work.tile([128, 2], F32, name="Asb", tag=f"A{s}")
                nc.vector.tensor_scalar(
                    out=Asb[:], in0=lnw[:, s, :], scalar1=mrb[:, s:s + 1],
                    scalar2=None, op0=OP.mult)
                Bsb = work.tile([128, 2], F32, name="Bsb", tag=f"B{s}")
                nc.vector.scalar_tensor_tensor(
                    out=Bsb[:], in0=Asb[:], scalar=mrb[:, 2 + s:3 + s],
                    in1=lnb[:, s, :], op0=OP.mult, op1=OP.add)
                AB.append((Asb, Bsb))
            pend_apply.append((i, h_sb, AB))

        def ln_epi_apply():
            if not pend_apply:
                return
            i, h_sb, AB = pend_apply.pop()
            for s in range(2):
                Asb, Bsb = AB[s]
                for t in range(2):
                    o_t = work.tile([128, N], F32, name="o_t", tag="o_t",
                                    bufs=4)
                    nc.vector.tensor_scalar(
                        out=o_t[:], in0=h_sb[s][:, t, :],
                        scalar1=Asb[:, t:t + 1], scalar2=Bsb[:, t:t + 1],
                        op0=OP.mult, op1=OP.add)
                    nc.sync.dma_start(out=out_d[s][i, t], in_=o_t[:])

        for i in range(IPC):
            # ---- input DMAs ----
            fb = []
            f8 = []
            for s in range(2):
                t = inp.tile([128, 2, N], BF16, name=f"fb{s}", tag=f"fb{s}")
                nc.sync.dma_start(
                    out=t[:], in_=fb_d[s][i].rearrange("c p n -> p c n"))
                fb.append(t)
                t8 = inp.tile([128, 2, N], FP8, name=f"f8_{s}", tag=f"f8_{s}")
                nc.sync.dma_start(
                    out=t8[:], in_=f8_d[s][i].rearrange("c p n -> p c n"))
                f8.append(t8)

            stats = work.tile([128, 8], F32, name="stats", tag="stats")
            h_sb = []

            for s in range(2):
                # ================= attention for output stream s ==========
                q_sb = conv_qk(wq[1 - s], f8[1 - s], "q_sb")
                k_sb = conv_qk(wk[s], f8[s], "k_sb")

                # vT[m, c] via DoubleRow: stationary = f8 slice pair
                vt_sb = work.tile([128, 8, 256], FP8, name="vt_sb", tag="vt")
                for half in range(4):
                    ps_vt = psum.tile([128, 512], F32, name="ps_vt",
                                      tag="work", bufs=4)
                    for jj in range(2):
                        j = half * 2 + jj
                        nc.tensor.matmul(
                            ps_vt[:, jj * 256:(jj + 1) * 256],
                            lhsT=f8[s][:, :, j * 128:(j + 1) * 128],
                            rhs=wv[s][:],
                            start=True, stop=True, perf_mode=DR)
                    nc.vector.tensor_copy(
                        out=vt_sb[:, half * 2:(half + 1) * 2, :]
                        .rearrange("p a b -> p (a b)"),
                        in_=ps_vt[:])

                # flush the previous section's residual/square stat ops
                # here: the S^T/PV phase ahead has no DVE work, so theyueue
                # behind the vT casts harmlessly
                while deferred_stats:
                    deferred_stats.pop(0)()
                if s == 1:
                    # previous item's LN apply lands between this attention's
                    # casts and its tail ops on the DVE queue
                    ln_epi_apply()

                # S^T -> exp(fp8) ; PV accumulates DoubleRow chunk-pairs
                pv_ps = [psum.tile([128, N], F32, name=f"pv{t}", tag="pv")
                         for t in range(2)]
                expS = work.tile([128, 8, N], FP8, name="expS", tag="expS")
                for j in range(8):
                    for h in range(2):
                        ps_s = psum.tile([128, 512], F32, name="ps_s",
                                         tag="work", bufs=4)
                        nc.tensor.matmul(
                            ps_s[:],
                            lhsT=k_sb[:, j * 128:(j + 1) * 128],
                            rhs=q_sb[:, h * 512:(h + 1) * 512],
                            start=True, stop=True)
                        nc.scalar.activation(
                            out=expS[:, j, h * 512:(h + 1) * 512],
                            in_=ps_s[:], func=AF.Exp, scale=EXP_SCALE)
                    if j % 2 == 1:
                        jp = j // 2  # chunk pair (2jp, 2jp+1) ready
                        for t in range(2):
                            for h in range(2):
                                nc.tensor.matmul(
                                    pv_ps[t][:, h * 512:(h + 1) * 512],
                                    lhsT=vt_sb[:, 2 * jp:2 * jp + 2,
                                               t * 128:(t + 1) * 128],
                                    rhs=expS[:, 2 * jp:2 * jp + 2,
                                             h * 512:(h + 1) * 512],
                                    start=(jp == 0), stop=(jp == 3),
                                    perf_mode=DR)

                # denominator: ones.T @ E accumulated over chunk pairs,
                # result rows are all equal to den[n]; then reciprocal.
                rden = work.tile([128, N], F32, name="rden", tag="rden")
                for h in range(2):
                    ps_bc = psum.tile([128, 512], F32, name="ps_bc",
                                      tag="work", bufs=4)
                    for jp in range(4):
                        nc.tensor.matmul(
                            ps_bc[:],
                            lhsT=ones8[:],
                            rhs=expS[:, 2 * jp:2 * jp + 2,
                                     h * 512:(h + 1) * 512],
                            start=(jp == 0), stop=(jp == 3), perf_mode=DR)
                    nc.vector.reciprocal_approx_fast(
                        out=rden[:, h * 512:(h + 1) * 512], in_=ps_bc[:])

                # normalize PV -> attn (bf16, feeds fuse matmul)
                attn_sb = work.tile([128, 2, N], BF16, name="attn_sb",
                                    tag="attn")
                for t in range(2):
                    for h in range(2):
                        sl = slice(h * 512, (h + 1) * 512)
                        nc.vector.scalar_tensor_tensor(
                            out=attn_sb[:, t, sl], in0=pv_ps[t][:, sl],
                            scalar=1.0 / WSCALE, in1=rden[:, sl],
                            op0=OP.mult, op1=OP.mult)

                # ================= fuse + residual + LN stats =============
                # f-half chunks first (no attn dependency), then attn half.
                ps_f = {}
                for t in range(2):
                    for h in range(2):
                        p = psum.tile([128, 512], F32, name="ps_f",
                                      tag="work", bufs=4)
                        ps_f[(t, h)] = p
                        for kc in range(2):
                            nc.tensor.matmul(
                                p[:],
                                lhsT=wf[:, kc, t * 128:(t + 1) * 128],
                                rhs=fb[s][:, kc, h * 512:(h + 1) * 512],
                                start=(kc == 0), stop=False)
                h_t = work.tile([128, 2, N], BF16, name="h_t", tag=f"h{s}",
                                bufs=2)
                g_t = work.tile([128, 2, N], BF16, name="g_t", tag="g_t",
                                bufs=3)
                for t in range(2):
                    for h in range(2):
                        p = ps_f[(t, h)]
                        for kc in range(2, 4):
                            nc.tensor.matmul(
                                p[:],
                                lhsT=wf[:, kc, t * 128:(t + 1) * 128],
                                rhs=attn_sb[:, kc - 2, h * 512:(h + 1) * 512],
                                start=False, stop=(kc == 3))
                        nc.scalar.activation(
                            out=g_t[:, t, h * 512:(h + 1) * 512], in_=p[:],
                            func=AF.Relu, bias=fbias[:, t:t + 1], scale=1.0)
                # residual (bf16) + LN sum/sumsq accum -- deferred past
                # the next section's conv casts so they don't clog the DVE
                # queue ahead of psum-freeing casts
                def emit_stats(s=s, g_t=g_t, h_t=h_t, fb_s=fb[s],
                               stats=stats):
                    for t in range(2):
                        c0 = s * 4 + t * 2
                        nc.vector.scalar_tensor_tensor(
                            out=h_t[:, t, :], in0=g_t[:, t, :], scalar=1.0,
                            in1=fb_s[:, t, :], op0=OP.mult, op1=OP.add,
                            accum_out=stats[:, c0:c0 + 1])
                        dum = work.tile([128, N], BF16, name="dum", tag="dum")
                        nc.vector.scalar_tensor_tensor(
                            out=dum[:], in0=h_t[:, t, :], scalar=1.0,
                            in1=h_t[:, t, :], op0=OP.mult, op1=OP.mult,
                            accum_out=stats[:, c0 + 1:c0 + 2])
                deferred_stats.append(emit_stats)
                h_sb.append(h_t)

                if s == 0:
                    # previous item's LN chain hides behind this item's work
                    ln_epi_chain()

            pend.append((i, stats, h_sb))

        while deferred_stats:
            deferred_stats.pop(0)()
        ln_epi_chain()
        ln_epi_apply()

        psum.release()
        work.release()
        inp.release()
        consts.release()

    nc.compile()
    return nc


_NC_CACHE = None


def _get_nc():
    global _NC_CACHE
    if _NC_CACHE is None:
        _NC_CACHE = _build()
    return _NC_CACHE


def kernel(fs, fi, qs_w, ks_w, vs_w, qi_w, ki_w, vi_w,
           fuse_w, fuse_b, ln_s_w, ln_s_b, ln_i_w, ln_i_b):
    global LAST_RESULT
    fs = np.asarray(fs, np.float32)
    fi = np.asarray(fi, np.float32)

    def prep_f(x):
        # (B, C, H, W) -> per-core [IPC, 2, 128, N]
        return x.reshape(NCORES, IPC, 2, 128, N)

    def prep_w_qk(w):  # (128, 256) -> lhsT layout [128p, 2kc, 128m] * 32
        wt = np.ascontiguousarray(np.asarray(w, np.float32).T) * WSCALE
        return np.ascontiguousarray(
            wt.reshape(2, 128, 128).transpose(1, 0, 2)).astype(
                ml_dtypes.float8_e4m3)

    def prep_w_v(w):  # (256, 256) -> rhs layout [128p, 2kc, 256c] * 32
        wt = np.ascontiguousarray(np.asarray(w, np.float32).T) * WSCALE
        return np.ascontiguousarray(
            wt.reshape(2, 128, 256).transpose(1, 0, 2)).astype(
                ml_dtypes.float8_e4m3)

    fs_sh = prep_f(fs)
    fi_sh = prep_f(fi)
    fs_bf = fs_sh.astype(ml_dtypes.bfloat16)
    fi_bf = fi_sh.astype(ml_dtypes.bfloat16)
    fs_q8 = fs_sh.astype(ml_dtypes.float8_e4m3)
    fi_q8 = fi_sh.astype(ml_dtypes.float8_e4m3)

    wq0 = prep_w_qk(qs_w)
    wq1 = prep_w_qk(qi_w)
    wk0 = prep_w_qk(ks_w)
    wk1 = prep_w_qk(ki_w)
    wv0 = prep_w_v(vs_w)
    wv1 = prep_w_v(vi_w)
    wfuse = np.ascontiguousarray(
        np.asarray(fuse_w, np.float32).T.reshape(4, 128, 256)
        .transpose(1, 0, 2)).astype(ml_dtypes.bfloat16)
    fuseb = np.ascontiguousarray(
        np.asarray(fuse_b, np.float32).reshape(2, 128).T)
    lnw = np.ascontiguousarray(
        np.stack([np.asarray(ln_s_w, np.float32).reshape(256),
                  np.asarray(ln_i_w, np.float32).reshape(256)])
        .reshape(2, 2, 128).transpose(2, 0, 1))
    lnb = np.ascontiguousarray(
        np.stack([np.asarray(ln_s_b, np.float32).reshape(256),
                  np.asarray(ln_i_b, np.float32).reshape(256)])
        .reshape(2, 2, 128).transpose(2, 0, 1))

    in_maps = []
    for c in range(NCORES):
        in_maps.append({
            "fsb": np.ascontiguousarray(fs_bf[c]),
            "fib": np.ascontiguousarray(fi_bf[c]),
            "fs8": np.ascontiguousarray(fs_q8[c]),
            "fi8": np.ascontiguousarray(fi_q8[c]),
            "wq0": wq0, "wq1": wq1, "wk0": wk0, "wk1": wk1,
            "wv0": wv0, "wv1": wv1, "wfuse": wfuse, "fuseb": fuseb,
            "lnw": lnw, "lnb": lnb,
        })

    nc = _get_nc()
    res = run_bass_kernel_spmd(nc, in_maps, core_ids=list(range(NCORES)),
                               **RUN_KWARGS)
    LAST_RESULT = res

    fs_out = np.empty((NCORES, IPC, 2, 128, N), np.float32)
    fi_out = np.empty((NCORES, IPC, 2, 128, N), np.float32)
    for c in range(NCORES):
        fs_out[c] = res.results[c]["out0"]
        fi_out[c] = res.results[c]["out1"]
    fs_out = fs_out.reshape(B, C, 32, 32)
    fi_out = fi_out.reshape(B, C, 32, 32)
    return fs_out, fi_out


# revision 13
# speedup vs baseline: 1.5243x; 1.0061x over previous
"""Trainium2 Bass kernel for nn_CrossAttention2d (B=32, C=256, INNER=128, H=W=32).

Sharding: pure data parallel — batch 32 split as 4 items per core across 8
NeuronCores; all weights replicated. No collectives.

Per item (N = H*W = 1024 tokens, C = 256 channels, D = 128 inner):
  attention for output stream s (s=0 -> fs side, s=1 -> fi side):
      q = wq[1-s] @ f[1-s], k = wk[s] @ f[s]   (fp8 DoubleRow, x32 prescale)
      vT[m, c] = (wv[s] @ f[s]).T   -- computed directly via DoubleRow with
                 f-slices as the stationary operand
      S^T[m, n] = sum_d k[d, m] q[d, n]        (bf16 PE, m-tiles of 128)
      E = exp(S^T / (1024 sqrt(D)))            (ACT, psum -> fp8 sbuf)
      O_un[c, n] = sum_m vT[m, c] E[m, n]      (fp8 DoubleRow, 4 chunk-pairs)
      den[n] via ones.T @ E (fp8 DoubleRow) broadcast to 128 rows
      attn = O_un * (1/32) * (1/den)           (DVE scalar_tensor_tensor)
  fuse: g = relu(Wf @ [f[s]; attn] + b)        (bf16 PE + ACT)
  h = g + f[s] (bf16 residual); LayerNorm over all (C,N) of h; LN stats via
  DVE accum_out + PE partition-reduce; out = h * A + B (DVE tensor_scalar).
  The LN scalar epilogue of item i is emitted inside item i+1 (software
  pipelining) so its serial tiny-op chain hides behind PE work.

Matmul convention: out[M, N] = lhsT.T @ rhs, lhsT = [K<=128, M<=128] (K on
partitions), rhs = [K, N<=512], out in PSUM f32 (one bank per matmul).
DoubleRow: lhsT [Ki, 2, M], rhs [Ki, 2, N] fp8 -> contracts 2*Ki.
PSUM: "pv" tag 2x[128,1024] (4 banks) + "work" tag 4x[128,512] (4 banks).
"""

import numpy as np
import ml_dtypes

import concourse.bacc as bacc
import concourse.bass as bass
import concourse.tile as tile
from concourse import mybir
from concourse.bass_utils import run_bass_kernel_spmd

F32 = mybir.dt.float32
BF16 = mybir.dt.bfloat16
FP8 = mybir.dt.float8e4
DR = mybir.MatmulPerfMode.DoubleRow
AF = mybir.ActivationFunctionType
OP = mybir.AluOpType

B, C, D, N = 32, 256, 128, 1024
NCORES = 8
IPC = B // NCORES  # items per core = 4
WSCALE = 32.0  # fp8 weight prescale (w*32 keeps N(0,0.02) in e4m3 range)
EXP_SCALE = (1.0 / float(np.sqrt(D))) / (WSCALE * WSCALE)
EPS = 1e-5
NTOT = float(C * N)  # layernorm element count per item/stream

# test.py can set {"trace": True}; harness path leaves this empty.
RUN_KWARGS = {}
LAST_RESULT = None


def _build():
    nc = bacc.Bacc("TRN2", target_bir_lowering=False, debug=False,
                   num_devices=NCORES)

    # ---- DRAM I/O (per-core shapes) ----
    fb_d = [nc.dram_tensor(n_, [IPC, 2, 128, N], BF16, kind="ExternalInput")
            for n_ in ("fsb", "fib")]
    f8_d = [nc.dram_tensor(n_, [IPC, 2, 128, N], FP8, kind="ExternalInput")
            for n_ in ("fs8", "fi8")]
    wq_d = [nc.dram_tensor(n_, [128, 2, 128], FP8, kind="ExternalInput")
            for n_ in ("wq0", "wq1")]
    wk_d = [nc.dram_tensor(n_, [128, 2, 128], FP8, kind="ExternalInput")
            for n_ in ("wk0", "wk1")]
    wv_d = [nc.dram_tensor(n_, [128, 2, 256], FP8, kind="ExternalInput")
            for n_ in ("wv0", "wv1")]
    wf_d = nc.dram_tensor("wfuse", [128, 4, 256], BF16, kind="ExternalInput")
    fb_bias_d = nc.dram_tensor("fuseb", [128, 2], F32, kind="ExternalInput")
    lnw_d = nc.dram_tensor("lnw", [128, 2, 2], F32, kind="ExternalInput")
    lnb_d = nc.dram_tensor("lnb", [128, 2, 2], F32, kind="ExternalInput")
    out_d = [nc.dram_tensor(n_, [IPC, 2, 128, N], F32, kind="ExternalOutput")
             for n_ in ("out0", "out1")]

    with tile.TileContext(nc) as tc:
        consts = tc.alloc_tile_pool(name="consts", bufs=1)
        inp = tc.alloc_tile_pool(name="inp", bufs=2)
        work = tc.alloc_tile_pool(name="work", bufs=2)
        psum = tc.alloc_tile_pool(name="psum", bufs=2, space="PSUM")

        # ---- load constants ----
        wq = [consts.tile([128, 2, 128], FP8, name=f"wq{s}", tag=f"wq{s}")
              for s in range(2)]
        wk = [consts.tile([128, 2, 128], FP8, name=f"wk{s}", tag=f"wk{s}")
              for s in range(2)]
        wv = [consts.tile([128, 2, 256], FP8, name=f"wv{s}", tag=f"wv{s}")
              for s in range(2)]
        wf = consts.tile([128, 4, 256], BF16, name="wf", tag="wf")
        fbias = consts.tile([128, 2], F32, name="fbias", tag="fbias")
        lnw = consts.tile([128, 2, 2], F32, name="lnw", tag="lnw")
        lnb = consts.tile([128, 2, 2], F32, name="lnb", tag="lnb")
        ones8 = consts.tile([128, 2, 128], FP8, name="ones8", tag="ones8")
        ones_col = consts.tile([128, 1], F32, name="ones_col", tag="ones_col")
        ones_row = consts.tile([1, 128], F32, name="ones_row", tag="ones_row")
        for s in range(2):
            nc.sync.dma_start(out=wq[s][:], in_=wq_d[s][:])
            nc.sync.dma_start(out=wk[s][:], in_=wk_d[s][:])
            nc.sync.dma_start(out=wv[s][:], in_=wv_d[s][:])
        nc.sync.dma_start(out=wf[:], in_=wf_d[:])
        nc.sync.dma_start(out=fbias[:], in_=fb_bias_d[:])
        nc.sync.dma_start(out=lnw[:], in_=lnw_d[:])
        nc.sync.dma_start(out=lnb[:], in_=lnb_d[:])
        nc.vector.memset(ones8[:], 1.0)
        nc.vector.memset(ones_col[:], 1.0)
        nc.vector.memset(ones_row[:], 1.0)

        def conv_qk(w_t, f8_t, name):
            """[128, N] = (32w).T @ f via fp8 DoubleRow; bf16 sbuf out."""
            sb = work.tile([128, N], BF16, name=name, tag=name)
            for h in range(2):
                ps = psum.tile([128, 512], F32, name=f"ps_{name}", tag="work",
                               bufs=4)
                nc.tensor.matmul(
                    ps[:], lhsT=w_t[:],
                    rhs=f8_t[:, :, h * 512:(h + 1) * 512],
                    start=True, stop=True, perf_mode=DR)
                nc.vector.tensor_copy(out=sb[:, h * 512:(h + 1) * 512],
                                      in_=ps[:])
            return sb

        # -------- per-item state carried into the next item (LN epilogue)
        deferred_stats = []  # closures emitting residual-stt ops (DVE)
        deferred_sq = []     # closures emitting sumsq ops (ACT)
        pend = []       # [(i, stats, h_sb)] awaiting the stats->A/B chain
        pend_apply = []  # [(i, h_sb, A, B)] awaiting LN apply + store

        def ln_epi_chain():
            """Stats -> mean/var -> rstd (DVE Newton) -> A/B. No ACT, and
            the only PE op (stats colsum) has its inputs long ready, so the
            PE stream never blocks on this chain."""
            if not pend:
                return
            i, stats, h_sb = pend.pop()
            ps_st = psum.tile([1, 8], F32, name="ps_st", tag="work", bufs=4)
            nc.tensor.matmul(ps_st[:], lhsT=ones_col[:], rhs=stats[:],
                             start=True, stop=True)
            st = work.tile([1, 8], F32, name="st", tag="st")
            nc.vector.tensor_copy(out=st[:], in_=ps_st[:])
            # cols: s*4 + t*2 + k (k=0 sum, k=1 sumsq) -> tot over t
            st_r = st[:].rearrange("p (a b) -> p a b", a=2)  # a=s, b=(t,k)
            tot = work.tile([1, 4], F32, name="tot", tag="tot")
            nc.vector.tensor_add(
                out=tot[:].rearrange("p (a b) -> p a b", a=2),
                in0=st_r[:, :, 0:2],
                in1=st_r[:, :, 2:4])
            # tot = [s0_sum, s0_sq, s1_sum, s1_sq] -> moments = tot / NTOT
            mom = work.tile([1, 4], F32, name="mom", tag="mom")
            nc.vector.tensor_scalar(out=mom[:], in0=tot[:],
                                    scalar1=1.0 / NTOT, scalar2=None,
                                    op0=OP.mult)
            mom_r = mom[:].rearrange("p (a b) -> p a b", a=2)
            mu_ap = mom_r[:, :, 0]
            m2_ap = mom_r[:, :, 1]
            musq = work.tile([1, 2], F32, name="musq", tag="musq")
            nc.vector.tensor_tensor(out=musq[:], in0=mu_ap, in1=mu_ap,
                                    op=OP.mult)
            var = work.tile([1, 2], F32, name="var", tag="var")
            nc.vector.scalar_tensor_tensor(
                out=var[:], in0=musq[:], scalar=-1.0, in1=m2_ap,
                op0=OP.mult, op1=OP.add)
            nc.vector.tensor_scalar(out=var[:], in0=var[:], scalar1=EPS,
                                    scalar2=None, op0=OP.add)
            # rstd = var^-0.5 via Newton (all-DVE; var is ~[0.3, 3] so the
            # constant seed converges: err 30% -> 6% -> 0.3% -> 1e-5)
            mr = work.tile([1, 4], F32, name="mr", tag="mr")
            y = mr[:, 0:2]
            nc.vector.memset(y, 0.92)
            t1 = work.tile([1, 2], F32, name="t1", tag="t1")
            for _ in range(3):
                nc.vector.tensor_tensor(out=t1[:], in0=y, in1=y, op=OP.mult)
                nc.vector.tensor_tensor(out=t1[:], in0=var[:], in1=t1[:],
                                        op=OP.mult)
                nc.vector.tensor_scalar(out=t1[:], in0=t1[:], scalar1=-0.5,
                                        scalar2=1.5, op0=OP.mult, op1=OP.add)
                nc.vector.tensor_tensor(out=y, in0=y, in1=t1[:], op=OP.mult)
            nc.vector.tensor_scalar(out=mr[:, 2:4], in0=mu_ap,
                                    scalar1=-1.0, scalar2=None, op0=OP.mult)
            # broadcast [1,4] -> [128,4] on GpSimd (PE stays out of it)
            mrb = work.tile([128, 4], F32, name="mrb", tag="mrb")
            nc.gpsimd.partition_broadcast(out_ap=mrb[:], in_ap=mr[:])
            # A = lnw * rstd ; Bb = lnb + (-mu) * A
            AB = []
            for s in range(2):
                Asb = work.tile([128, 2], F32, name="Asb", tag=f"A{s}")
                nc.vector.tensor_scalar(
                    out=Asb[:], in0=lnw[:, s, :], scalar1=mrb[:, s:s + 1],
                    scalar2=None, op0=OP.mult)
                Bsb = work.tile([128, 2], F32, name="Bsb", tag=f"B{s}")
                nc.vector.scalar_tensor_tensor(
                    out=Bsb[:], in0=Asb[:], scalar=mrb[:, 2 + s:3 + s],
                    in1=lnb[:, s, :], op0=OP.mult, op1=OP.add)
                AB.append((Asb, Bsb))
            pend_apply.append((i, h_sb, AB))

        def ln_epi_apply():
            if not pend_apply:
                return
            i, h_sb, AB = pend_apply.pop()
            for s in range(2):
                Asb, Bsb = AB[s]
                for t in range(2):
                    o_t = work.tile([128, N], F32, name="o_t", tag="o_t",
                                    bufs=4)
                    nc.vector.tensor_scalar(
                        out=o_t[:], in0=h_sb[s][:, t, :],
                        scalar1=Asb[:, t:t + 1], scalar2=Bsb[:, t:t + 1],
                        op0=OP.mult, op1=OP.add)
                    nc.sync.dma_start(out=out_d[s][i, t], in_=o_t[:])

        for i in range(IPC):
            # ---- input DMAs ----
            fb = []
            f8 = []
            for s in range(2):
                t = inp.tile([128, 2, N], BF16, name=f"fb{s}", tag=f"fb{s}")
                nc.sync.dma_start(
                    out=t[:], in_=fb_d[s][i].rearrange("c p n -> p c n"))
                fb.append(t)
                t8 = inp.tile([128, 2, N], FP8, name=f"f8_{s}", tag=f"f8_{s}")
                nc.sync.dma_start(
                    out=t8[:], in_=f8_d[s][i].rearrange("c p n -> p c n"))
                f8.append(t8)

            stats = work.tile([128, 8], F32, name="stats", tag="stats")
            h_sb = []

            for s in range(2):
                # ================= attention for output stream s ==========
                q_sb = conv_qk(wq[1 - s], f8[1 - s], "q_sb")
                k_sb = conv_qk(wk[s], f8[s], "k_sb")

                # vT[m, c] via DoubleRow: stationary = f8 slice pair
                vt_sb = work.tile([128, 8, 256], FP8, name="vt_sb", tag="vt")
                for half in range(4):
                    ps_vt = psum.tile([128, 512], F32, name="ps_vt",
                                      tag="work", bufs=4)
                    for jj in range(2):
                        j = half * 2 + jj
                        nc.tensor.matmul(
                            ps_vt[:, jj * 256:(jj + 1) * 256],
                            lhsT=f8[s][:, :, j * 128:(j + 1) * 128],
                            rhs=wv[s][:],
                            start=True, stop=True, perf_mode=DR)
                    nc.vector.tensor_copy(
                        out=vt_sb[:, half * 2:(half + 1) * 2, :]
                        .rearrange("p a b -> p (a b)"),
                        in_=ps_vt[:])

                # flush the previous section's residual/square stat ops
                # here: the S^T/PV phase ahead has no DVE work, so theyueue
                # behind the vT casts harmlessly
                while deferred_stats:
                    deferred_stats.pop(0)()
                if s == 1:
                    # previous item's LN apply lands between this attention's
                    # casts and its tail ops on the DVE queue
                    ln_epi_apply()

                # S^T -> exp(fp8) ; PV accumulates DoubleRow chunk-pairs
                pv_ps = [psum.tile([128, N], F32, name=f"pv{t}", tag="pv")
                         for t in range(2)]
                expS = work.tile([128, 8, N], FP8, name="expS", tag="expS")
                for j in range(8):
                    for h in range(2):
                        ps_s = psum.tile([128, 512], F32, name="ps_s",
                                         tag="work", bufs=4)
                        nc.tensor.matmul(
                            ps_s[:],
                            lhsT=k_sb[:, j * 128:(j + 1) * 128],
                            rhs=q_sb[:, h * 512:(h + 1) * 512],
                            start=True, stop=True)
                        nc.scalar.activation(
                            out=expS[:, j, h * 512:(h + 1) * 512],
                            in_=ps_s[:], func=AF.Exp, scale=EXP_SCALE)
                    if j % 2 == 1:
                        jp = j // 2  # chunk pair (2jp, 2jp+1) ready
                        for t in range(2):
                            for h in range(2):
                                nc.tensor.matmul(
                                    pv_ps[t][:, h * 512:(h + 1) * 512],
                                    lhsT=vt_sb[:, 2 * jp:2 * jp + 2,
                                               t * 128:(t + 1) * 128],
                                    rhs=expS[:, 2 * jp:2 * jp + 2,
                                             h * 512:(h + 1) * 512],
                                    start=(jp == 0), stop=(jp == 3),
                                    perf_mode=DR)

                # previous section's sumsq lands in ACT's idle tail here
                while deferred_sq:
                    deferred_sq.pop(0)()

                # denominator: ones.T @ E accumulated over chunk pairs,
                # result rows are all equal to den[n]; then reciprocal.
                rden = work.tile([128, N], F32, name="rden", tag="rden")
                for h in range(2):
                    ps_bc = psum.tile([128, 512], F32, name="ps_bc",
                                      tag="work", bufs=4)
                    for jp in range(4):
                        nc.tensor.matmul(
                            ps_bc[:],
                            lhsT=ones8[:],
                            rhs=expS[:, 2 * jp:2 * jp + 2,
                                     h * 512:(h + 1) * 512],
                            start=(jp == 0), stop=(jp == 3), perf_mode=DR)
                    nc.vector.reciprocal_approx_fast(
                        out=rden[:, h * 512:(h + 1) * 512], in_=ps_bc[:])

                # normalize PV -> attn (bf16, feeds fuse matmul)
                attn_sb = work.tile([128, 2, N], BF16, name="attn_sb",
                                    tag="attn")
                for t in range(2):
                    for h in range(2):
                        sl = slice(h * 512, (h + 1) * 512)
                        nc.vector.scalar_tensor_tensor(
                            out=attn_sb[:, t, sl], in0=pv_ps[t][:, sl],
                            scalar=1.0 / WSCALE, in1=rden[:, sl],
                            op0=OP.mult, op1=OP.mult)

                # ================= fuse + residual + LN stats =============
                # f-half chunks first (no attn dependency), then attn half.
                ps_f = {}
                for t in range(2):
                    for h in range(2):
                        p = psum.tile([128, 512], F32, name="ps_f",
                                      tag="work", bufs=4)
                        ps_f[(t, h)] = p
                        for kc in range(2):
                            nc.tensor.matmul(
                                p[:],
                                lhsT=wf[:, kc, t * 128:(t + 1) * 128],
                                rhs=fb[s][:, kc, h * 512:(h + 1) * 512],
                                start=(kc == 0), stop=False)
                h_t = work.tile([128, 2, N], BF16, name="h_t", tag=f"h{s}",
                                bufs=2)
                g_t = work.tile([128, 2, N], BF16, name="g_t", tag="g_t",
                                bufs=3)
                for t in range(2):
                    for h in range(2):
                        p = ps_f[(t, h)]
                        for kc in range(2, 4):
                            nc.tensor.matmul(
                                p[:],
                                lhsT=wf[:, kc, t * 128:(t + 1) * 128],
                                rhs=attn_sb[:, kc - 2, h * 512:(h + 1) * 512],
                                start=False, stop=(kc == 3))
                        nc.scalar.activation(
                            out=g_t[:, t, h * 512:(h + 1) * 512], in_=p[:],
                            func=AF.Relu, bias=fbias[:, t:t + 1], scale=1.0)
                # residual (bf16) + LN sum/sumsq accum -- deferred past
                # the next section's conv casts so they don't clog the DVE
                # queue ahead of psum-freeing casts
                def emit_stats(s=s, g_t=g_t, h_t=h_t, fb_s=fb[s],
                               stats=stats):
                    for t in range(2):
                        c0 = s * 4 + t * 2
                        nc.vector.scalar_tensor_tensor(
                            out=h_t[:, t, :], in0=g_t[:, t, :], scalar=1.0,
                            in1=fb_s[:, t, :], op0=OP.mult, op1=OP.add,
                            accum_out=stats[:, c0:c0 + 1])
                def emit_sq(s=s, h_t=h_t, stats=stats):
                    for t in range(2):
                        c0 = s * 4 + t * 2
                        dum = work.tile([128, N], BF16, name="dum", tag="dum")
                        nc.scalar.activation(
                            out=dum[:], in_=h_t[:, t, :], func=AF.Square,
                            accum_out=stats[:, c0 + 1:c0 + 2])
                deferred_stats.append(emit_stats)
                deferred_sq.append(emit_sq)
                h_sb.append(h_t)

                if s == 0:
                    # previous item's LN chain hides behind this item's work
                    ln_epi_chain()

            pend.append((i, stats, h_sb))

        while deferred_stats:
            deferred_stats.pop(0)()
        while deferred_sq:
            deferred_sq.pop(0)()
        ln_epi_chain()
        ln_epi_apply()

        psum.release()
        work.release()
        inp.release()
        consts.release()

    nc.compile()
    return nc


_NC_CACHE = None


def _get_nc():
    global _NC_CACHE
    if _NC_CACHE is None:
        _NC_CACHE = _build()
    return _NC_CACHE


def kernel(fs, fi, qs_w, ks_w, vs_w, qi_w, ki_w, vi_w,
           fuse_w, fuse_b, ln_s_w, ln_s_b, ln_i_w, ln_i_b):
    global LAST_RESULT
    fs = np.asarray(fs, np.float32)
    fi = np.asarray(fi, np.float32)

    def prep_f(x):
        # (B, C, H, W) -> per-core [IPC, 2, 128, N]
        return x.reshape(NCORES, IPC, 2, 128, N)

    def prep_w_qk(w):  # (128, 256) -> lhsT layout [128p, 2kc, 128m] * 32
        wt = np.ascontiguousarray(np.asarray(w, np.float32).T) * WSCALE
        return np.ascontiguousarray(
            wt.reshape(2, 128, 128).transpose(1, 0, 2)).astype(
                ml_dtypes.float8_e4m3)

    def prep_w_v(w):  # (256, 256) -> rhs layout [128p, 2kc, 256c] * 32
        wt = np.ascontiguousarray(np.asarray(w, np.float32).T) * WSCALE
        return np.ascontiguousarray(
            wt.reshape(2, 128, 256).transpose(1, 0, 2)).astype(
                ml_dtypes.float8_e4m3)

    fs_sh = prep_f(fs)
    fi_sh = prep_f(fi)
    fs_bf = fs_sh.astype(ml_dtypes.bfloat16)
    fi_bf = fi_sh.astype(ml_dtypes.bfloat16)
    fs_q8 = fs_sh.astype(ml_dtypes.float8_e4m3)
    fi_q8 = fi_sh.astype(ml_dtypes.float8_e4m3)

    wq0 = prep_w_qk(qs_w)
    wq1 = prep_w_qk(qi_w)
    wk0 = prep_w_qk(ks_w)
    wk1 = prep_w_qk(ki_w)
    wv0 = prep_w_v(vs_w)
    wv1 = prep_w_v(vi_w)
    wfuse = np.ascontiguousarray(
        np.asarray(fuse_w, np.float32).T.reshape(4, 128, 256)
        .transpose(1, 0, 2)).astype(ml_dtypes.bfloat16)
    fuseb = np.ascontiguousarray(
        np.asarray(fuse_b, np.float32).reshape(2, 128).T)
    lnw = np.ascontiguousarray(
        np.stack([np.asarray(ln_s_w, np.float32).reshape(256),
                  np.asarray(ln_i_w, np.float32).reshape(256)])
        .reshape(2, 2, 128).transpose(2, 0, 1))
    lnb = np.ascontiguousarray(
        np.stack([np.asarray(ln_s_b, np.float32).reshape(256),
                  np.asarray(ln_i_b, np.float32).reshape(256)])
        .reshape(2, 2, 128).transpose(2, 0, 1))

    in_maps = []
    for c in range(NCORES):
        in_maps.append({
            "fsb": np.ascontiguousarray(fs_bf[c]),
            "fib": np.ascontiguousarray(fi_bf[c]),
            "fs8": np.ascontiguousarray(fs_q8[c]),
            "fi8": np.ascontiguousarray(fi_q8[c]),
            "wq0": wq0, "wq1": wq1, "wk0": wk0, "wk1": wk1,
            "wv0": wv0, "wv1": wv1, "wfuse": wfuse, "fuseb": fuseb,
            "lnw": lnw, "lnb": lnb,
        })

    nc = _get_nc()
    res = run_bass_kernel_spmd(nc, in_maps, core_ids=list(range(NCORES)),
                               **RUN_KWARGS)
    LAST_RESULT = res

    fs_out = np.empty((NCORES, IPC, 2, 128, N), np.float32)
    fi_out = np.empty((NCORES, IPC, 2, 128, N), np.float32)
    for c in range(NCORES):
        fs_out[c] = res.results[c]["out0"]
        fi_out[c] = res.results[c]["out1"]
    fs_out = fs_out.reshape(B, C, 32, 32)
    fi_out = fi_out.reshape(B, C, 32, 32)
    return fs_out, fi_out


# revision 14
# speedup vs baseline: 1.5753x; 1.0335x over previous
"""Trainium2 Bass kernel for nn_CrossAttention2d (B=32, C=256, INNER=128, H=W=32).

Sharding: pure data parallel — batch 32 split as 4 items per core across 8
NeuronCores; all weights replicated. No collectives.

Per item (N = H*W = 1024 tokens, C = 256 channels, D = 128 inner):
  attention for output stream s (s=0 -> fs side, s=1 -> fi side):
      q = wq[1-s] @ f[1-s], k = wk[s] @ f[s]   (fp8 DoubleRow, x32 prescale)
      vT[m, c] = (wv[s] @ f[s]).T   -- computed directly via DoubleRow with
                 f-slices as the stationary operand
      S^T[m, n] = sum_d k[d, m] q[d, n]        (bf16 PE, m-tiles of 128)
      E = exp(S^T / (1024 sqrt(D)))            (ACT, psum -> fp8 sbuf)
      O_un[c, n] = sum_m vT[m, c] E[m, n]      (fp8 DoubleRow, 4 chunk-pairs)
      den[n] via ones.T @ E (fp8 DoubleRow) broadcast to 128 rows
      attn = O_un * (1/32) * (1/den)           (DVE scalar_tensor_tensor)
  fuse: g = relu(Wf @ [f[s]; attn] + b)        (bf16 PE + ACT)
  h = g + f[s] (bf16 residual); LayerNorm over all (C,N) of h; LN stats via
  DVE accum_out + PE partition-reduce; out = h * A + B (DVE tensor_scalar).
  The LN scalar epilogue of item i is emitted inside item i+1 (software
  pipelining) so its serial tiny-op chain hides behind PE work.

Matmul convention: out[M, N] = lhsT.T @ rhs, lhsT = [K<=128, M<=128] (K on
partitions), rhs = [K, N<=512], out in PSUM f32 (one bank per matmul).
DoubleRow: lhsT [Ki, 2, M], rhs [Ki, 2, N] fp8 -> contracts 2*Ki.
PSUM: "pv" tag 2x[128,1024] (4 banks) + "work" tag 4x[128,512] (4 banks).
"""

import numpy as np
import ml_dtypes

import concourse.bacc as bacc
import concourse.bass as bass
import concourse.tile as tile
from concourse import mybir
from concourse.bass_utils import run_bass_kernel_spmd

F32 = mybir.dt.float32
BF16 = mybir.dt.bfloat16
FP8 = mybir.dt.float8e4
DR = mybir.MatmulPerfMode.DoubleRow
AF = mybir.ActivationFunctionType
OP = mybir.AluOpType

B, C, D, N = 32, 256, 128, 1024
NCORES = 8
IPC = B // NCORES  # items per core = 4
WSCALE = 32.0  # fp8 weight prescale (w*32 keeps N(0,0.02) in e4m3 range)
EXP_SCALE = (1.0 / float(np.sqrt(D))) / (WSCALE * WSCALE)
EPS = 1e-5
NTOT = float(C * N)  # layernorm element count per item/stream

# test.py can set {"trace": True}; harness path leaves this empty.
RUN_KWARGS = {}
LAST_RESULT = None


def _build():
    nc = bacc.Bacc("TRN2", target_bir_lowering=False, debug=False,
                   num_devices=NCORES)

    # ---- DRAM I/O (per-core shapes) ----
    fb_d = [nc.dram_tensor(n_, [IPC, 128, 2, N], BF16, kind="ExternalInput")
            for n_ in ("fsb", "fib")]
    f8_d = [nc.dram_tensor(n_, [IPC, 128, 2, N], FP8, kind="ExternalInput")
            for n_ in ("fs8", "fi8")]
    wq_d = [nc.dram_tensor(n_, [128, 2, 128], FP8, kind="ExternalInput")
            for n_ in ("wq0", "wq1")]
    wk_d = [nc.dram_tensor(n_, [128, 2, 128], FP8, kind="ExternalInput")
            for n_ in ("wk0", "wk1")]
    wv_d = [nc.dram_tensor(n_, [128, 2, 256], FP8, kind="ExternalInput")
            for n_ in ("wv0", "wv1")]
    wf_d = nc.dram_tensor("wfuse", [128, 4, 256], BF16, kind="ExternalInput")
    wfa_d = nc.dram_tensor("wfusea", [128, 2, 256], FP8, kind="ExternalInput")
    fb_bias_d = nc.dram_tensor("fuseb", [128, 2], F32, kind="ExternalInput")
    lnw_d = nc.dram_tensor("lnw", [128, 2, 2], F32, kind="ExternalInput")
    lnb_d = nc.dram_tensor("lnb", [128, 2, 2], F32, kind="ExternalInput")
    out_d = [nc.dram_tensor(n_, [IPC, 2, 128, N], F32, kind="ExternalOutput")
             for n_ in ("out0", "out1")]

    with tile.TileContext(nc) as tc:
        consts = tc.alloc_tile_pool(name="consts", bufs=1)
        inp = tc.alloc_tile_pool(name="inp", bufs=2)
        work = tc.alloc_tile_pool(name="work", bufs=2)
        psum = tc.alloc_tile_pool(name="psum", bufs=2, space="PSUM")

        # ---- load constants ----
        wq = [consts.tile([128, 2, 128], FP8, name=f"wq{s}", tag=f"wq{s}")
              for s in range(2)]
        wk = [consts.tile([128, 2, 128], FP8, name=f"wk{s}", tag=f"wk{s}")
              for s in range(2)]
        wv = [consts.tile([128, 2, 256], FP8, name=f"wv{s}", tag=f"wv{s}")
              for s in range(2)]
        wf = consts.tile([128, 4, 256], BF16, name="wf", tag="wf")
        wfa = consts.tile([128, 2, 256], FP8, name="wfa", tag="wfa")
        fbias = consts.tile([128, 2], F32, name="fbias", tag="fbias")
        lnw = consts.tile([128, 2, 2], F32, name="lnw", tag="lnw")
        lnb = consts.tile([128, 2, 2], F32, name="lnb", tag="lnb")
        ones8 = consts.tile([128, 2, 128], FP8, name="ones8", tag="ones8")
        ones_col = consts.tile([128, 1], F32, name="ones_col", tag="ones_col")
        ones_row = consts.tile([1, 128], F32, name="ones_row", tag="ones_row")
        for s in range(2):
            nc.gpsimd.dma_start(out=wq[s][:], in_=wq_d[s][:])
            nc.gpsimd.dma_start(out=wk[s][:], in_=wk_d[s][:])
            nc.gpsimd.dma_start(out=wv[s][:], in_=wv_d[s][:])
        nc.gpsimd.dma_start(out=wf[:], in_=wf_d[:])
        nc.gpsimd.dma_start(out=wfa[:], in_=wfa_d[:])
        nc.gpsimd.dma_start(out=fbias[:], in_=fb_bias_d[:])
        nc.gpsimd.dma_start(out=lnw[:], in_=lnw_d[:])
        nc.gpsimd.dma_start(out=lnb[:], in_=lnb_d[:])
        nc.vector.memset(ones8[:], 1.0)
        nc.vector.memset(ones_col[:], 1.0)
        nc.vector.memset(ones_row[:], 1.0)

        def conv_qk(w_t, f8_t, name):
            """[128, N] = (32w).T @ f via fp8 DoubleRow; bf16 sbuf out."""
            sb = work.tile([128, N], BF16, name=name, tag=name)
            for h in range(2):
                ps = psum.tile([128, 512], F32, name=f"ps_{name}", tag="work",
                               bufs=4)
                nc.tensor.matmul(
                    ps[:], lhsT=w_t[:],
                    rhs=f8_t[:, :, h * 512:(h + 1) * 512],
                    start=True, stop=True, perf_mode=DR)
                nc.vector.tensor_copy(out=sb[:, h * 512:(h + 1) * 512],
                                      in_=ps[:])
            return sb

        # -------- per-item state carried into the next item (LN epilogue)
        deferred_stats = []  # closures emitting residual-stt ops (DVE)
        deferred_sq = []     # closures emitting sumsq ops (ACT)
        pend = []       # [(i, stats, h_sb)] awaiting the stats->A/B chain
        pend_apply = []  # [(i, h_sb, A, B)] awaiting LN apply + store

        def ln_epi_chain():
            """Stats -> mean/var -> rstd (DVE Newton) -> A/B. No ACT, and
            the only PE op (stats colsum) has its inputs long ready, so the
            PE stream never blocks on this chain."""
            if not pend:
                return
            i, stats, h_sb = pend.pop()
            ps_st = psum.tile([1, 8], F32, name="ps_st", tag="work", bufs=4)
            nc.tensor.matmul(ps_st[:], lhsT=ones_col[:], rhs=stats[:],
                             start=True, stop=True)
            st = work.tile([1, 8], F32, name="st", tag="st")
            nc.vector.tensor_copy(out=st[:], in_=ps_st[:])
            # cols: s*4 + t*2 + k (k=0 sum, k=1 sumsq) -> tot over t
            st_r = st[:].rearrange("p (a b) -> p a b", a=2)  # a=s, b=(t,k)
            tot = work.tile([1, 4], F32, name="tot", tag="tot")
            nc.vector.tensor_add(
                out=tot[:].rearrange("p (a b) -> p a b", a=2),
                in0=st_r[:, :, 0:2],
                in1=st_r[:, :, 2:4])
            # tot = [s0_sum, s0_sq, s1_sum, s1_sq] -> moments = tot / NTOT
            mom = work.tile([1, 4], F32, name="mom", tag="mom")
            nc.vector.tensor_scalar(out=mom[:], in0=tot[:],
                                    scalar1=1.0 / NTOT, scalar2=None,
                                    op0=OP.mult)
            mom_r = mom[:].rearrange("p (a b) -> p a b", a=2)
            mu_ap = mom_r[:, :, 0]
            m2_ap = mom_r[:, :, 1]
            musq = work.tile([1, 2], F32, name="musq", tag="musq")
            nc.vector.tensor_tensor(out=musq[:], in0=mu_ap, in1=mu_ap,
                                    op=OP.mult)
            var = work.tile([1, 2], F32, name="var", tag="var")
            nc.vector.scalar_tensor_tensor(
                out=var[:], in0=musq[:], scalar=-1.0, in1=m2_ap,
                op0=OP.mult, op1=OP.add)
            nc.vector.tensor_scalar(out=var[:], in0=var[:], scalar1=EPS,
                                    scalar2=None, op0=OP.add)
            # rstd = var^-0.5 via Newton (all-DVE; var is ~[0.3, 3] so the
            # constant seed converges: err 30% -> 6% -> 0.3% -> 1e-5)
            mr = work.tile([1, 4], F32, name="mr", tag="mr")
            y = mr[:, 0:2]
            nc.vector.memset(y, 0.92)
            t1 = work.tile([1, 2], F32, name="t1", tag="t1")
            for _ in range(3):
                nc.vector.tensor_tensor(out=t1[:], in0=y, in1=y, op=OP.mult)
                nc.vector.tensor_tensor(out=t1[:], in0=var[:], in1=t1[:],
                                        op=OP.mult)
                nc.vector.tensor_scalar(out=t1[:], in0=t1[:], scalar1=-0.5,
                                        scalar2=1.5, op0=OP.mult, op1=OP.add)
                nc.vector.tensor_tensor(out=y, in0=y, in1=t1[:], op=OP.mult)
            nc.vector.tensor_scalar(out=mr[:, 2:4], in0=mu_ap,
                                    scalar1=-1.0, scalar2=None, op0=OP.mult)
            # broadcast [1,4] -> [128,4] on GpSimd (PE stays out of it)
            mrb = work.tile([128, 4], F32, name="mrb", tag="mrb")
            nc.gpsimd.partition_broadcast(out_ap=mrb[:], in_ap=mr[:])
            # A = lnw * rstd ; Bb = lnb + (-mu) * A
            AB = []
            for s in range(2):
                Asb = work.tile([128, 2], F32, name="Asb", tag=f"A{s}")
                nc.vector.tensor_scalar(
                    out=Asb[:], in0=lnw[:, s, :], scalar1=mrb[:, s:s + 1],
                    scalar2=None, op0=OP.mult)
                Bsb = work.tile([128, 2], F32, name="Bsb", tag=f"B{s}")
                nc.vector.scalar_tensor_tensor(
                    out=Bsb[:], in0=Asb[:], scalar=mrb[:, 2 + s:3 + s],
                    in1=lnb[:, s, :], op0=OP.mult, op1=OP.add)
                AB.append((Asb, Bsb))
            pend_apply.append((i, h_sb, AB))

        def ln_epi_apply():
            if not pend_apply:
                return
            i, h_sb, AB = pend_apply.pop()
            for s in range(2):
                Asb, Bsb = AB[s]
                for t in range(2):
                    o_t = work.tile([128, N], F32, name="o_t", tag="o_t",
                                    bufs=4)
                    nc.vector.tensor_scalar(
                        out=o_t[:], in0=h_sb[s][:, t, :],
                        scalar1=Asb[:, t:t + 1], scalar2=Bsb[:, t:t + 1],
                        op0=OP.mult, op1=OP.add)
                    nc.sync.dma_start(out=out_d[s][i, t], in_=o_t[:])

        for i in range(IPC):
            # ---- input DMAs ----
            fb = []
            f8 = []
            for s in range(2):
                t = inp.tile([128, 2, N], BF16, name=f"fb{s}", tag=f"fb{s}")
                nc.sync.dma_start(out=t[:], in_=fb_d[s][i])
                fb.append(t)
                t8 = inp.tile([128, 2, N], FP8, name=f"f8_{s}", tag=f"f8_{s}")
                nc.sync.dma_start(out=t8[:], in_=f8_d[s][i])
                f8.append(t8)

            stats = work.tile([128, 8], F32, name="stats", tag="stats")
            h_sb = []

            for s in range(2):
                # ================= attention for output stream s ==========
                q_sb = conv_qk(wq[1 - s], f8[1 - s], "q_sb")
                k_sb = conv_qk(wk[s], f8[s], "k_sb")

                # vT[m, c] via DoubleRow: stationary = f8 slice pair
                vt_sb = work.tile([128, 8, 256], FP8, name="vt_sb", tag="vt")
                for half in range(4):
                    ps_vt = psum.tile([128, 512], F32, name="ps_vt",
                                      tag="work", bufs=4)
                    for jj in range(2):
                        j = half * 2 + jj
                        nc.tensor.matmul(
                            ps_vt[:, jj * 256:(jj + 1) * 256],
                            lhsT=f8[s][:, :, j * 128:(j + 1) * 128],
                            rhs=wv[s][:],
                            start=True, stop=True, perf_mode=DR)
                    nc.vector.tensor_copy(
                        out=vt_sb[:, half * 2:(half + 1) * 2, :]
                        .rearrange("p a b -> p (a b)"),
                        in_=ps_vt[:])

                # flush the previous section's residual/square stat ops
                # here: the S^T/PV phase ahead has no DVE work, so theyueue
                # behind the vT casts harmlessly
                while deferred_stats:
                    deferred_stats.pop(0)()
                if s == 1:
                    # previous item's LN apply lands between this attention's
                    # casts and its tail ops on the DVE queue
                    ln_epi_apply()

                # S^T -> exp(fp8) ; PV accumulates DoubleRow chunk-pairs
                pv_ps = [psum.tile([128, N], F32, name=f"pv{t}", tag="pv")
                         for t in range(2)]
                expS = work.tile([128, 8, N], FP8, name="expS", tag="expS")
                for j in range(8):
                    for h in range(2):
                        ps_s = psum.tile([128, 512], F32, name="ps_s",
                                         tag="work", bufs=4)
                        nc.tensor.matmul(
                            ps_s[:],
                            lhsT=k_sb[:, j * 128:(j + 1) * 128],
                            rhs=q_sb[:, h * 512:(h + 1) * 512],
                            start=True, stop=True)
                        nc.scalar.activation(
                            out=expS[:, j, h * 512:(h + 1) * 512],
                            in_=ps_s[:], func=AF.Exp, scale=EXP_SCALE)
                    if j % 2 == 1:
                        jp = j // 2  # chunk pair (2jp, 2jp+1) ready
                        for t in range(2):
                            for h in range(2):
                                nc.tensor.matmul(
                                    pv_ps[t][:, h * 512:(h + 1) * 512],
                                    lhsT=vt_sb[:, 2 * jp:2 * jp + 2,
                                               t * 128:(t + 1) * 128],
                                    rhs=expS[:, 2 * jp:2 * jp + 2,
                                             h * 512:(h + 1) * 512],
                                    start=(jp == 0), stop=(jp == 3),
                                    perf_mode=DR)

                # previous section's sumsq lands in ACT's idle tail here
                while deferred_sq:
                    deferred_sq.pop(0)()

                # denominator: ones.T @ E accumulated over chunk pairs,
                # result rows are all equal to den[n]; then reciprocal.
                rden = work.tile([128, N], F32, name="rden", tag="rden")
                for h in range(2):
                    ps_bc = psum.tile([128, 512], F32, name="ps_bc",
                                      tag="work", bufs=4)
                    for jp in range(4):
                        nc.tensor.matmul(
                            ps_bc[:],
                            lhsT=ones8[:],
                            rhs=expS[:, 2 * jp:2 * jp + 2,
                                     h * 512:(h + 1) * 512],
                            start=(jp == 0), stop=(jp == 3), perf_mode=DR)
                    nc.vector.reciprocal_approx_fast(
                        out=rden[:, h * 512:(h + 1) * 512], in_=ps_bc[:])

                # normalize PV -> attn (bf16, feeds fuse matmul)
                attn_sb = work.tile([128, 2, N], FP8, name="attn_sb",
                                    tag="attn")
                for t in range(2):
                    for h in range(2):
                        sl = slice(h * 512, (h + 1) * 512)
                        nc.vector.tensor_tensor(
                            out=attn_sb[:, t, sl], in0=pv_ps[t][:, sl],
                            in1=rden[:, sl], op=OP.mult)

                # ================= fuse + residual + LN stats =============
                # f-half chunks first (no attn dependency), then attn half.
                ps_f = {}
                for t in range(2):
                    for h in range(2):
                        p = psum.tile([128, 512], F32, name="ps_f",
                                      tag="work", bufs=4)
                        ps_f[(t, h)] = p
                        for kc in range(2):
                            nc.tensor.matmul(
                                p[:],
                                lhsT=wf[:, kc, t * 128:(t + 1) * 128],
                                rhs=fb[s][:, kc, h * 512:(h + 1) * 512],
                                start=(kc == 0), stop=False)
                h_t = work.tile([128, 2, N], BF16, name="h_t", tag=f"h{s}",
                                bufs=2)
                g_t = work.tile([128, 2, N], BF16, name="g_t", tag="g_t",
                                bufs=3)
                for t in range(2):
                    for h in range(2):
                        p = ps_f[(t, h)]
                        nc.tensor.matmul(
                            p[:],
                            lhsT=wfa[:, :, t * 128:(t + 1) * 128],
                            rhs=attn_sb[:, :, h * 512:(h + 1) * 512],
                            start=False, stop=True, perf_mode=DR)
                        nc.scalar.activation(
                            out=g_t[:, t, h * 512:(h + 1) * 512], in_=p[:],
                            func=AF.Relu, bias=fbias[:, t:t + 1],
                            scale=1.0 / (WSCALE * WSCALE))
                # residual (bf16) + LN sum/sumsq accum -- deferred past
                # the next section's conv casts so they don't clog the DVE
                # queue ahead of psum-freeing casts
                def emit_stats(s=s, g_t=g_t, h_t=h_t, fb_s=fb[s],
                               stats=stats):
                    for t in range(2):
                        c0 = s * 4 + t * 2
                        nc.vector.scalar_tensor_tensor(
                            out=h_t[:, t, :], in0=g_t[:, t, :], scalar=1.0,
                            in1=fb_s[:, t, :], op0=OP.mult, op1=OP.add,
                            accum_out=stats[:, c0:c0 + 1])
                def emit_sq(s=s, h_t=h_t, stats=stats):
                    for t in range(2):
                        c0 = s * 4 + t * 2
                        dum = work.tile([128, N], BF16, name="dum", tag="dum")
                        nc.scalar.activation(
                            out=dum[:], in_=h_t[:, t, :], func=AF.Square,
                            accum_out=stats[:, c0 + 1:c0 + 2])
                deferred_stats.append(emit_stats)
                deferred_sq.append(emit_sq)
                h_sb.append(h_t)

                if s == 0:
                    # previous item's LN chain hides behind this item's work
                    ln_epi_chain()

            pend.append((i, stats, h_sb))

        while deferred_stats:
            deferred_stats.pop(0)()
        while deferred_sq:
            deferred_sq.pop(0)()
        ln_epi_chain()
        ln_epi_apply()

        psum.release()
        work.release()
        inp.release()
        consts.release()

    nc.compile()
    return nc


_NC_CACHE = None


def _get_nc():
    global _NC_CACHE
    if _NC_CACHE is None:
        _NC_CACHE = _build()
    return _NC_CACHE


def kernel(fs, fi, qs_w, ks_w, vs_w, qi_w, ki_w, vi_w,
           fuse_w, fuse_b, ln_s_w, ln_s_b, ln_i_w, ln_i_b):
    global LAST_RESULT
    fs = np.asarray(fs, np.float32)
    fi = np.asarray(fi, np.float32)

    def prep_f(x):
        # (B, C, H, W) -> per-core [IPC, 128, 2, N] (partition-major so the
        # on-chip DMA is fully contiguous)
        x = x.reshape(NCORES, IPC, 2, 128, N)
        return np.ascontiguousarray(x.transpose(0, 1, 3, 2, 4))

    def prep_w_qk(w):  # (128, 256) -> lhsT layout [128p, 2kc, 128m] * 32
        wt = np.ascontiguousarray(np.asarray(w, np.float32).T) * WSCALE
        return np.ascontiguousarray(
            wt.reshape(2, 128, 128).transpose(1, 0, 2)).astype(
                ml_dtypes.float8_e4m3)

    def prep_w_v(w):  # (256, 256) -> rhs layout [128p, 2kc, 256c] * 32
        wt = np.ascontiguousarray(np.asarray(w, np.float32).T) * WSCALE
        return np.ascontiguousarray(
            wt.reshape(2, 128, 256).transpose(1, 0, 2)).astype(
                ml_dtypes.float8_e4m3)

    fs_sh = prep_f(fs)
    fi_sh = prep_f(fi)
    fs_bf = fs_sh.astype(ml_dtypes.bfloat16)
    fi_bf = fi_sh.astype(ml_dtypes.bfloat16)
    fs_q8 = fs_sh.astype(ml_dtypes.float8_e4m3)
    fi_q8 = fi_sh.astype(ml_dtypes.float8_e4m3)

    wq0 = prep_w_qk(qs_w)
    wq1 = prep_w_qk(qi_w)
    wk0 = prep_w_qk(ks_w)
    wk1 = prep_w_qk(ki_w)
    wv0 = prep_w_v(vs_w)
    wv1 = prep_w_v(vi_w)
    wfuse_t = np.ascontiguousarray(
        np.asarray(fuse_w, np.float32).T.reshape(4, 128, 256)
        .transpose(1, 0, 2))
    wfuse = (wfuse_t * (WSCALE * WSCALE)).astype(ml_dtypes.bfloat16)
    wfusea = np.ascontiguousarray(
        (wfuse_t[:, 2:4, :] * WSCALE)).astype(ml_dtypes.float8_e4m3)
    fuseb = np.ascontiguousarray(
        np.asarray(fuse_b, np.float32).reshape(2, 128).T)
    lnw = np.ascontiguousarray(
        np.stack([np.asarray(ln_s_w, np.float32).reshape(256),
                  np.asarray(ln_i_w, np.float32).reshape(256)])
        .reshape(2, 2, 128).transpose(2, 0, 1))
    lnb = np.ascontiguousarray(
        np.stack([np.asarray(ln_s_b, np.float32).reshape(256),
                  np.asarray(ln_i_b, np.float32).reshape(256)])
        .reshape(2, 2, 128).transpose(2, 0, 1))

    in_maps = []
    for c in range(NCORES):
        in_maps.append({
            "fsb": np.ascontiguousarray(fs_bf[c]),
            "fib": np.ascontiguousarray(fi_bf[c]),
            "fs8": np.ascontiguousarray(fs_q8[c]),
            "fi8": np.ascontiguousarray(fi_q8[c]),
            "wq0": wq0, "wq1": wq1, "wk0": wk0, "wk1": wk1,
            "wv0": wv0, "wv1": wv1, "wfuse": wfuse, "wfusea": wfusea,
            "fuseb": fuseb,
            "lnw": lnw, "lnb": lnb,
        })

    nc = _get_nc()
    res = run_bass_kernel_spmd(nc, in_maps, core_ids=list(range(NCORES)),
                               **RUN_KWARGS)
    LAST_RESULT = res

    fs_out = np.empty((NCORES, IPC, 2, 128, N), np.float32)
    fi_out = np.empty((NCORES, IPC, 2, 128, N), np.float32)
    for c in range(NCORES):
        fs_out[c] = res.results[c]["out0"]
        fi_out[c] = res.results[c]["out1"]
    fs_out = fs_out.reshape(B, C, 32, 32)
    fi_out = fi_out.reshape(B, C, 32, 32)
    return fs_out, fi_out
